# revision 1
# baseline (speedup 1.0000x reference)
"""ONIMemoryHub kernel for 8 Trainium2 NeuronCores (Bass/Tile).

Sharding: data-parallel over batch for the query side; episodic store and
semantic memory sharded across cores for the key/value projections, with
AllGathers of the projected (normalized, pre-scaled) keys/values.

kernel(**inputs) takes FULL inputs (as produced by reference.setup_inputs())
and returns the FULL [4096, 2048] output.
"""
import math

import numpy as np

import concourse.bass as bass
import concourse.mybir as mybir
import concourse.tile as tile
from concourse import bacc
from concourse.bass_utils import run_bass_kernel_spmd
from concourse.masks import make_identity

AF = mybir.ActivationFunctionType
AXL = mybir.AxisListType
ALU = mybir.AluOpType

NCORES = 8
B, H, N, M, S = 4096, 2048, 4096, 16384, 64
BL, NL, ML = B // NCORES, N // NCORES, M // NCORES   # 512, 512, 2048
HT = H // 128                                        # 16 h-tiles
P = 128
NBT = BL // P                                        # 4 b-tiles
EP_K = 8
SEM_K = 4
LN_EPS = 1e-5
RECENCY = 0.01   # 1 - RECENCY_DECAY

F32 = mybir.dt.float32
F32R = mybir.dt.float32r
BF16 = mybir.dt.bfloat16
U32 = mybir.dt.uint32

# dtype knobs (iterate on these for perf; F32 = exact)
SIM_DT = F32     # sim matmul inputs (qT/qsT/keys)
PROJ_DT = F32    # q/qs/ek/ks projection inputs
VAL_DT = F32     # value-side + output projections


def R(ap):
    """Reinterpret an f32 AP as f32r: 1 cycle/row on PE for free dim >=256."""
    return ap.bitcast(F32R)


def build():
    nc = bacc.Bacc("TRN2", target_bir_lowering=False, debug=False,
                   num_devices=NCORES)

    def din(name, shape, dt=F32):
        return nc.dram_tensor(name, shape, dt, kind="ExternalInput").ap()

    # per-core slices: host-split bf16 pairs, pre-transposed to tile layout
    query_pk = din("query_pk", [1, P, 2, HT, 512], BF16)
    ep_pk = din("ep_pk", [1, P, 2, HT, 512], BF16)
    semk_pk = din("semk_pk", [ML // 512, P, 2, HT, 512], BF16)
    ep_imp_s = din("ep_imp_s", [NL])
    ep_ts_s = din("ep_ts_s", [NL])
    # replicated
    ep_imp = din("ep_imp", [N])
    ep_ts = din("ep_ts", [N])
    semv16 = din("semv16", [M, H], BF16)
    wq_pk = din("wq_pk", [HT, P, 2, HT, P], BF16)
    wek_pk = din("wek_pk", [HT, P, 2, HT, P], BF16)
    wsq_pk = din("wsq_pk", [HT, P, 2, HT, P], BF16)
    wsk_pk = din("wsk_pk", [HT, P, 2, HT, P], BF16)
    wev16 = din("wev16", [H, H], BF16)
    weo16 = din("weo16", [H, H], BF16)
    wso16 = din("wso16", [H, H], BF16)
    wro16 = din("wro16", [H, H], BF16)
    work_slots = din("work_slots", [S, H])
    gate_W1 = din("gate_W1", [H, 64])
    gate_b1 = din("gate_b1", [64])
    gate_W2 = din("gate_W2", [64, 3])
    gate_b2 = din("gate_b2", [3])
    ln_gamma = din("ln_gamma", [H])
    ln_beta = din("ln_beta", [H])

    out_s = nc.dram_tensor("out_s", [BL, H], F32, kind="ExternalOutput").ap()

    with tile.TileContext(nc) as tc:
        with (
            tc.tile_pool(name="cst", bufs=1) as cst,
            tc.tile_pool(name="big", bufs=1) as big,
            tc.tile_pool(name="rows", bufs=1) as rows,
            tc.tile_pool(name="s512", bufs=2) as s512p,
            tc.tile_pool(name="wcol", bufs=2) as wcolp,
            tc.tile_pool(name="wtile", bufs=3) as wtp,
            tc.tile_pool(name="sm", bufs=1) as sm,
            tc.tile_pool(name="tiny", bufs=2) as tiny,
            tc.tile_pool(name="simb", bufs=2) as simb,
            tc.tile_pool(name="gath", bufs=2) as gath,
            tc.tile_pool(name="ps_tr", bufs=2, space="PSUM") as ps_tr,
            tc.tile_pool(name="ps_mm", bufs=5, space="PSUM") as ps_mm,
            tc.tile_pool(name="ps_sml", bufs=1, space="PSUM") as ps_sml,
            tc.tile_pool(name="dram", bufs=1, space="DRAM") as dram,
        ):
            ident = cst.tile([P, P], F32)
            make_identity(nc, ident[:])
            ones_col = cst.tile([P, 1], F32)
            nc.vector.memset(ones_col[:], 1.0)

            # ---------- helpers ----------
            # big slot chains (explicit liveness via shared tags):
            #   xTin: epT -> skT(x4) -> qTin -> qsT
            #   kT  : ekT -> ksT(x4) -> qT
            def emit_split(dst_hi, dst_lo, src_f32, tmp32):
                """bf16 two-term split: hi = bf16(x), lo = bf16(x - hi)."""
                nc.scalar.activation(dst_hi, src_f32, AF.Copy)
                nc.gpsimd.tensor_copy(tmp32, dst_hi)
                nc.vector.tensor_tensor(out=tmp32, in0=src_f32, in1=tmp32,
                                        op=ALU.subtract)
                nc.scalar.activation(dst_lo, tmp32, AF.Copy)

            def load_transposed_pair(src_pk, chunk, name, tag):
                """host-pre-transposed bf16 pair chunk -> SBUF tile."""
                xP = big.tile([P, 2, HT, 512], BF16, tag=tag, name=name)
                nc.sync.dma_start(xP[:], src_pk[chunk])
                return xP

            def wcol_pair(w_pk, j):
                wcP = wcolp.tile([P, 2, HT, P], BF16, tag="wcp", name="wcp")
                nc.sync.dma_start(wcP[:], w_pk[j])
                return wcP

            def project3(xP, w_pk, name, tag, mode, pair_tag=None):
                """(x @ W).T via 3-term bf16 split matmuls; xP is a pair."""
                yT = yP = None
                if mode in ("f32", "both"):
                    yT = big.tile([P, HT, 512], F32, tag=tag, name=name)
                if mode in ("pair", "both"):
                    yP = big.tile([P, 2, HT, 512], BF16,
                                  tag=pair_tag or tag, name=name + "p")
                for j in range(HT):
                    wcP = wcol_pair(w_pk, j)
                    pst = ps_mm.tile([P, 512], F32, tag="mm", name="projps")
                    for hi in range(HT):
                        nc.tensor.matmul(
                            pst[:], wcP[:, 0, hi, :], xP[:, 0, hi, :],
                            start=(hi == 0), stop=False)
                        nc.tensor.matmul(
                            pst[:], wcP[:, 0, hi, :], xP[:, 1, hi, :],
                            start=False, stop=False)
                        nc.tensor.matmul(
                            pst[:], wcP[:, 1, hi, :], xP[:, 0, hi, :],
                            start=False, stop=(hi == HT - 1))
                    if mode == "f32":
                        nc.scalar.activation(yT[:, j, :], pst[:], AF.Copy)
                    else:
                        tmp32 = s512p.tile([P, 512], F32, tag="s512",
                                           name="spj32")
                        if mode == "both":
                            nc.vector.tensor_copy(yT[:, j, :], pst[:])
                        emit_split(yP[:, 0, j, :], yP[:, 1, j, :], pst[:],
                                   tmp32[:])
                if mode == "f32":
                    return yT
                if mode == "pair":
                    return yP
                return yT, yP

            def split_to_pair(xT, tag, name):
                """f32 [P, HT, 512] tile -> bf16 pair tile in another slot."""
                xP = big.tile([P, 2, HT, 512], BF16, tag=tag, name=name)
                for hi in range(HT):
                    tmp32 = s512p.tile([P, 512], F32, tag="s512", name="sps32")
                    emit_split(xP[:, 0, hi, :], xP[:, 1, hi, :], xT[:, hi, :],
                               tmp32[:])
                return xP

            def store_pair_to_ag(xT, ag_in):
                """split scaled f32 keys and store bf16 pair to AG input."""
                for hi in range(HT):
                    sth = s512p.tile([P, 512], BF16, tag="st16h", name="sth",
                                     bufs=8)
                    stl = s512p.tile([P, 512], BF16, tag="st16l", name="stl",
                                     bufs=8)
                    tmp32 = s512p.tile([P, 512], F32, tag="s512", name="spg32")
                    emit_split(sth[:], stl[:], xT[:, hi, :], tmp32[:])
                    nc.sync.dma_start(ag_in[0, hi * P:(hi + 1) * P, :], sth[:])
                    nc.sync.dma_start(ag_in[1, hi * P:(hi + 1) * P, :], stl[:])

            def inv_norm_row(xT, extra_row=None):
                """[1, 512] = (extra or 1)/max(||x_col||,1e-12); xT [128,HT,512]."""
                row = rows.tile([1, 512], F32, tag="nrow", name="nrow", bufs=2)
                sq = s512p.tile([P, 512], F32, tag="s512", name="sqn")
                psn = ps_sml.tile([1, 512], F32, tag="sml", name="npsum")
                for hi in range(HT):
                    nc.scalar.square(sq[:, :], xT[:, hi, :])
                    nc.tensor.matmul(
                        psn[:1, :], ones_col[:], sq[:, :],
                        start=(hi == 0), stop=(hi == HT - 1))
                nc.vector.tensor_copy(row[:1, :], psn[:1, :])
                nc.scalar.sqrt(row[:1, :], row[:1, :])
                nc.vector.tensor_scalar_max(row[:1, :], row[:1, :], 1e-12)
                nc.vector.reciprocal(row[:1, :], row[:1, :])
                if extra_row is not None:
                    nc.vector.tensor_mul(row[:1, :], row[:1, :], extra_row)
                return row

            def inv_norm_pair(xP):
                """inv norm of a bf16 pair tile (hi+lo recombined)."""
                row = rows.tile([1, 512], F32, tag="nrow", name="nrow", bufs=2)
                sq = s512p.tile([P, 512], F32, tag="s512", name="sqn")
                cmb = s512p.tile([P, 512], F32, tag="s512", name="cmb")
                psn = ps_sml.tile([1, 512], F32, tag="sml", name="npsum")
                for hi in range(HT):
                    nc.vector.scalar_tensor_tensor(
                        out=cmb[:, :], in0=xP[:, 1, hi, :], scalar=1.0,
                        in1=xP[:, 0, hi, :], op0=ALU.mult, op1=ALU.add)
                    nc.scalar.square(sq[:, :], cmb[:, :])
                    nc.tensor.matmul(
                        psn[:1, :], ones_col[:], sq[:, :],
                        start=(hi == 0), stop=(hi == HT - 1))
                nc.vector.tensor_copy(row[:1, :], psn[:1, :])
                nc.scalar.sqrt(row[:1, :], row[:1, :])
                nc.vector.tensor_scalar_max(row[:1, :], row[:1, :], 1e-12)
                nc.vector.reciprocal(row[:1, :], row[:1, :])
                return row

            def scale_cols(xT, scale_row):
                bc = s512p.tile([P, 512], F32, tag="s512", name="bcn")
                nc.gpsimd.partition_broadcast(bc[:, :], scale_row[:1, :])
                for hi in range(HT):
                    nc.vector.tensor_mul(xT[:, hi, :], xT[:, hi, :], bc[:, :])


            # ===================================================================
            # Phase M: sharded memory-side projections + AllGathers
            # ===================================================================
            ag_nek_in = dram.tile([2, H, NL], BF16, name="ag_nek_in")
            ag_nek_out = dram.tile([NCORES, 2, H, NL], BF16,
                                   addr_space="Shared", name="ag_nek_out")
            ag_ev_in = dram.tile([NL, H], BF16, name="ag_ev_in")
            ag_ev_out = dram.tile([N, H], BF16, addr_space="Shared",
                                  name="ag_ev_out")
            ag_nks_in = [dram.tile([2, H, 512], BF16, name=f"ag_nks_in{i}")
                         for i in range(ML // 512)]
            ag_nks_out = [dram.tile([NCORES, 2, H, 512], BF16,
                                    addr_space="Shared", name=f"ag_nks_out{i}")
                          for i in range(ML // 512)]

            # --- episodic: transpose slice, project keys/vals ---
            # --- work slots transposed + gate weights ---
            wsT = big.tile([P, HT, S], F32, name="wsT")
            for hi in range(HT):
                wsn = s512p.tile([S, 512], F32, tag="s512", name="wsn")
                nc.sync.dma_start(wsn[:S, :P], work_slots[:, hi * P:(hi + 1) * P])
                pst = ps_tr.tile([P, S], F32, tag="tr", name="wstp")
                nc.tensor.transpose(out=pst[:, :S], in_=wsn[:S, :P],
                                    identity=ident[:S, :S])
                nc.vector.tensor_copy(wsT[:, hi, :], pst[:, :S])
            gw1 = big.tile([P, HT, 64], F32, name="gw1")
            nc.sync.dma_start(gw1[:], gate_W1.rearrange("(hi p) c -> p hi c", p=P))
            gw2 = cst.tile([64, 3], F32, name="gw2")
            nc.sync.dma_start(gw2[:, :], gate_W2)

            epP = load_transposed_pair(ep_pk, 0, "epT", "xTin")
            ekT = project3(epP, wek_pk, "ekT", "kT", "f32")
            # ===================================================================
            # Phase W: episodic recency/importance weights
            # ===================================================================
            def rec_weight(imp_ap, ts_ap, shape, tagb):
                """(1+imp)*exp(-|1-ts|*RECENCY) elementwise; returns tile."""
                impt = rows.tile(shape, F32, tag=tagb + "i", name="impt")
                tst = rows.tile(shape, F32, tag=tagb + "t", name="tst")
                nc.sync.dma_start(impt[:shape[0], :], imp_ap)
                nc.sync.dma_start(tst[:shape[0], :], ts_ap)
                s = tst[:shape[0], :]
                nc.scalar.activation(s, s, AF.Copy, bias=0.0, scale=-1.0)
                nc.vector.tensor_scalar_add(s, s, 1.0)
                nc.scalar.activation(s, s, AF.Abs)
                nc.scalar.activation(s, s, AF.Exp, scale=-RECENCY)
                si = impt[:shape[0], :]
                nc.vector.tensor_scalar_add(si, si, 1.0)
                nc.vector.tensor_mul(si, si, s)
                return impt

            # global sum in [128, 32] layout
            wfull = rec_weight(ep_imp.rearrange("(p c) -> p c", p=P),
                               ep_ts.rearrange("(p c) -> p c", p=P),
                               [P, N // P], "wf")
            wpart = rows.tile([P, 1], F32, tag="wpart", name="wpart")
            nc.vector.reduce_sum(wpart[:, :], wfull[:, :], axis=AXL.X)
            pssum = ps_sml.tile([1, 512], F32, tag="sml", name="wsps")
            nc.tensor.matmul(pssum[:1, :1], ones_col[:], wpart[:, :],
                             start=True, stop=True)
            wsum = rows.tile([1, 1], F32, tag="wsum", name="wsum")
            nc.vector.tensor_copy(wsum[:1, :], pssum[:1, :1])
            nc.vector.tensor_scalar_add(wsum[:1, :], wsum[:1, :], 1e-8)
            nc.vector.reciprocal(wsum[:1, :], wsum[:1, :])
            # local slice weights [1, NL], normalized
            wloc = rec_weight(ep_imp_s[None, :], ep_ts_s[None, :], [1, NL], "wl")
            nc.vector.tensor_scalar(wloc[:1, :], wloc[:1, :], wsum[:1, :1], None,
                                    op0=ALU.mult)
            inv_ek = inv_norm_row(ekT, extra_row=wloc[:1, :])
            scale_cols(ekT, inv_ek)
            store_pair_to_ag(ekT, ag_nek_in)
            nc.gpsimd.collective_compute(
                "AllGather", ALU.bypass,
                replica_groups=[list(range(NCORES))],
                ins=[ag_nek_in.opt()], outs=[ag_nek_out.opt()])
            # e_vals natural layout [NL, H]; bf16 single term (values path)
            for jc in range(H // 512):
                psts = [ps_mm.tile([P, 512], F32, tag="mm", name=f"evps{i}")
                        for i in range(NL // P)]
                for hi in range(HT):
                    wt16 = wtp.tile([P, 512], BF16, tag="wt16", name="wt16")
                    nc.sync.dma_start(
                        wt16[:],
                        wev16[hi * P:(hi + 1) * P, jc * 512:(jc + 1) * 512])
                    for nt in range(NL // P):
                        ns = slice(nt * P, (nt + 1) * P)
                        nc.tensor.matmul(
                            psts[nt][:], epP[:, 0, hi, ns], wt16[:],
                            start=(hi == 0), stop=(hi == HT - 1))
                for nt in range(NL // P):
                    evs = s512p.tile([P, 512], BF16, tag="evo16", name="evout")
                    nc.vector.tensor_copy(evs[:], psts[nt][:])
                    nc.sync.dma_start(
                        ag_ev_in[nt * P:(nt + 1) * P, jc * 512:(jc + 1) * 512],
                        evs[:])
            nc.gpsimd.collective_compute(
                "AllGather", ALU.bypass,
                replica_groups=[list(range(NCORES))],
                ins=[ag_ev_in.opt()], outs=[ag_ev_out.opt()])

            # --- semantic keys: 4 chunks of 512 ---
            for mc in range(ML // 512):
                # chunk 0 prefetches into the idle "bl" slot ("xTin" is still
                # held by epP until e_vals completes)
                skP = load_transposed_pair(semk_pk, mc, f"skT{mc}",
                                           "bl" if mc == 0 else "xTin")
                ksT = project3(skP, wsk_pk, f"ksT{mc}", "kT", "f32")
                inv_ks = inv_norm_row(ksT)
                scale_cols(ksT, inv_ks)
                store_pair_to_ag(ksT, ag_nks_in[mc])
                nc.gpsimd.collective_compute(
                    "AllGather", ALU.bypass,
                    replica_groups=[list(range(NCORES))],
                    ins=[ag_nks_in[mc].opt()], outs=[ag_nks_out[mc].opt()])

            # ===================================================================
            # Phase Q: query-side projections + work/gate precompute
            # ===================================================================
            qTinP = load_transposed_pair(query_pk, 0, "qTin", "xTin")
            qT, qTp = project3(qTinP, wq_pk, "qT", "kT", "both",
                               pair_tag="bl")
            qsP = project3(qTp, wsq_pk, "qsT", "xTin", "pair")
            inv_q = inv_norm_row(qT)
            inv_qs = inv_norm_pair(qsP)

            # transpose inv rows -> per-partition [128, NBT] via DRAM bounce
            invq_p = cst.tile([P, NBT], F32, name="invq_p")
            invqs_p = cst.tile([P, NBT], F32, name="invqs_p")
            bounce = dram.tile([2, BL], F32, name="bounce")
            nc.sync.dma_start(bounce[0:1, :], inv_q[:1, :])
            nc.sync.dma_start(bounce[1:2, :], inv_qs[:1, :])
            nc.sync.dma_start(
                invq_p[:, :], bounce[0:1, :].rearrange("o (t p) -> (o p) t", p=P))
            nc.sync.dma_start(
                invqs_p[:, :], bounce[1:2, :].rearrange("o (t p) -> (o p) t", p=P))


            def bcast_row(dram_row, width, pool, tag, name):
                row = rows.tile([1, width], F32, tag="crow", name="crow", bufs=1)
                nc.sync.dma_start(row[:1, :], dram_row)
                t = pool.tile([P, width], F32, tag=tag, name=name)
                nc.gpsimd.partition_broadcast(t[:, :], row[:1, :])
                return t

            b1bc = bcast_row(gate_b1[None, :], 64, cst, "", "b1bc")
            b2bc = bcast_row(gate_b2[None, :], 3, cst, "", "b2bc")

            inv_sqrt_h = 1.0 / math.sqrt(H)
            ewT_pre = []
            gw_pre = []
            for bt in range(NBT):
                # gate
                psg = ps_sml.tile([P, 64], F32, tag="sml", name="psg")
                for hi in range(HT):
                    nc.tensor.matmul(
                        psg[:, :64], qT[:, hi, bt * P:(bt + 1) * P], gw1[:, hi, :],
                        start=(hi == 0), stop=(hi == HT - 1))
                hid = tiny.tile([P, 64], F32, tag="c64", name="hid")
                nc.vector.tensor_add(hid[:, :], psg[:, :64], b1bc[:, :])
                nc.scalar.activation(hid[:, :], hid[:, :], AF.Silu)
                psht = ps_tr.tile([64, P], F32, tag="tr", name="hidtp")
                nc.tensor.transpose(out=psht[:64, :], in_=hid[:, :],
                                    identity=ident[:])
                hidT = tiny.tile([64, P], F32, tag="c128", name="hidT")
                nc.vector.tensor_copy(hidT[:, :], psht[:64, :])
                psg2 = ps_sml.tile([P, 3], F32, tag="sml", name="psg2")
                nc.tensor.matmul(psg2[:, :3], hidT[:, :], gw2[:, :],
                                 start=True, stop=True)
                gl = cst.tile([P, 3], F32, name=f"gl{bt}")
                nc.vector.tensor_add(gl[:, :], psg2[:, :3], b2bc[:, :])
                gmax = tiny.tile([P, 1], F32, tag="c1", name="gmax")
                nc.vector.reduce_max(gmax[:, :], gl[:, :], axis=AXL.X)
                nc.vector.tensor_scalar_mul(gmax[:, :], gmax[:, :], -1.0)
                nc.scalar.activation(gl[:, :], gl[:, :], AF.Exp, bias=gmax[:, :1])
                gz = tiny.tile([P, 1], F32, tag="c1", name="gz")
                nc.vector.reduce_sum(gz[:, :], gl[:, :], axis=AXL.X)
                nc.vector.reciprocal(gz[:, :], gz[:, :])
                nc.vector.tensor_scalar(gl[:, :], gl[:, :], gz[:, :1], None,
                                        op0=ALU.mult)
                gw_pre.append(gl)

                # work attention probs (transposed, pre-scaled by gate0)
                psw = ps_sml.tile([P, S], F32, tag="sml", name="pswk")
                for hi in range(HT):
                    nc.tensor.matmul(
                        psw[:, :S], qT[:, hi, bt * P:(bt + 1) * P], wsT[:, hi, :],
                        start=(hi == 0), stop=(hi == HT - 1))
                wmax = tiny.tile([P, 1], F32, tag="c1", name="wmax")
                nc.vector.reduce_max(wmax[:, :], psw[:, :S], axis=AXL.X)
                nc.vector.tensor_scalar_mul(wmax[:, :], wmax[:, :], -inv_sqrt_h)
                ew = tiny.tile([P, S], F32, tag="c64", name="ew")
                nc.scalar.activation(ew[:, :], psw[:, :S], AF.Exp,
                                     bias=wmax[:, :1], scale=inv_sqrt_h)
                zw = tiny.tile([P, 1], F32, tag="c1", name="zw")
                nc.vector.reduce_sum(zw[:, :], ew[:, :], axis=AXL.X)
                nc.vector.reciprocal(zw[:, :], zw[:, :])
                # fold softmax normalization AND gate weight 0 into ew
                nc.vector.tensor_tensor(out=zw[:, :], in0=zw[:, :],
                                        in1=gl[:, 0:1], op=ALU.mult)
                nc.vector.tensor_scalar(ew[:, :], ew[:, :], zw[:, :1], None,
                                        op0=ALU.mult)
                pset = ps_tr.tile([S, P], F32, tag="tr", name="ewtp")
                nc.tensor.transpose(out=pset[:S, :], in_=ew[:, :],
                                    identity=ident[:])
                ewT = cst.tile([S, P], F32, name=f"ewT{bt}")
                nc.vector.tensor_copy(ewT[:, :], pset[:S, :])
                ewT_pre.append(ewT)

            # ===================================================================
            # Phase S: similarity + per-chunk top-8 candidates
            # ===================================================================
            cand_v_e = [big.tile([P, (N // 512) * 8], F32, tag=f"cve{bt}",
                                 name=f"cve{bt}") for bt in range(NBT)]
            cand_i_e = [big.tile([P, (N // 512) * 8], F32, tag=f"cie{bt}",
                                 name=f"cie{bt}") for bt in range(NBT)]
            cand_v_s = [big.tile([P, (M // 512) * 8], F32, tag=f"cvs{bt}",
                                 name=f"cvs{bt}") for bt in range(NBT)]
            cand_i_s = [big.tile([P, (M // 512) * 8], F32, tag=f"cis{bt}",
                                 name=f"cis{bt}") for bt in range(NBT)]

            def sim_chunk(xP, kd, r, sub, ch, cand_v, cand_i, base):
                """sims of all 4 b-tiles vs bf16-pair keys kd[r, :, h, :]."""
                psts = [ps_mm.tile([P, 512], F32, tag="mm", name=f"simps{i}")
                        for i in range(NBT)]
                for hi in range(HT):
                    kth = s512p.tile([P, 512], BF16, tag="st16h", name="kth",
                                     bufs=8)
                    ktl = s512p.tile([P, 512], BF16, tag="st16l", name="ktl",
                                     bufs=8)
                    nc.sync.dma_start(
                        kth[:], kd[r, 0, hi * P:(hi + 1) * P, :])
                    nc.sync.dma_start(
                        ktl[:], kd[r, 1, hi * P:(hi + 1) * P, :])
                    for bt in range(NBT):
                        bs = slice(bt * P, (bt + 1) * P)
                        nc.tensor.matmul(
                            psts[bt][:], xP[:, 0, hi, bs], kth[:],
                            start=(hi == 0), stop=False)
                        nc.tensor.matmul(
                            psts[bt][:], xP[:, 0, hi, bs], ktl[:],
                            start=False, stop=False)
                        nc.tensor.matmul(
                            psts[bt][:], xP[:, 1, hi, bs], kth[:],
                            start=False, stop=(hi == HT - 1))
                for bt in range(NBT):
                    sc = simb.tile([P, 512], F32, tag="simc", name="simc",
                                   bufs=4)
                    nc.scalar.activation(sc[:], psts[bt][:], AF.Copy)
                    mx = simb.tile([P, 8], F32, tag="mx", name="mx")
                    mi = simb.tile([P, 8], U32, tag="mi", name="mi")
                    nc.vector.max(out=mx[:], in_=sc[:])
                    nc.vector.max_index(out=mi[:], in_max=mx[:], in_values=sc[:])
                    nc.vector.tensor_copy(cand_v[bt][:, ch * 8:(ch + 1) * 8],
                                          mx[:])
                    mif = simb.tile([P, 8], F32, tag="mif", name="mif")
                    nc.vector.tensor_copy(mif[:], mi[:])
                    nc.vector.tensor_scalar_add(
                        cand_i[bt][:, ch * 8:(ch + 1) * 8], mif[:],
                        float(base))

            def topk_attend(cand_v, cand_i, k, inv_p, bt, vals_dram, gscale,
                            acc_tag):
                """Merged top-k -> softmax (x gscale) -> gather + weighted sum."""
                top8 = tiny.tile([P, 8], F32, tag="c8", name="top8")
                nc.vector.max(out=top8[:], in_=cand_v[:])
                idxf = tiny.tile([P, 8], F32, tag="c8", name="idxf")
                eqm = sm.tile([P, 256], F32, tag="eqm", name="eqm")
                for kk in range(k):
                    w = cand_v.shape[-1]
                    nc.vector.tensor_scalar(
                        eqm[:, :w], cand_v[:], top8[:, kk:kk + 1], None,
                        op0=ALU.is_equal)
                    nc.vector.tensor_tensor(
                        out=eqm[:, :w], in0=eqm[:, :w], in1=cand_i[:], op=ALU.mult)
                    nc.vector.reduce_sum(idxf[:, kk:kk + 1], eqm[:, :w], axis=AXL.X)
                idxu = tiny.tile([P, 8], U32, tag="c8u", name="idxu")
                nc.vector.tensor_copy(idxu[:, :k], idxf[:, :k])
                sc8 = tiny.tile([P, 8], F32, tag="c8", name="sc8")
                nc.vector.tensor_scalar(
                    sc8[:, :k], top8[:, :k], inv_p[:, bt:bt + 1], None,
                    op0=ALU.mult)
                negm = tiny.tile([P, 1], F32, tag="c1", name="negm")
                nc.vector.tensor_scalar_mul(negm[:, :], sc8[:, 0:1], -1.0)
                nc.scalar.activation(sc8[:, :k], sc8[:, :k], AF.Exp,
                                     bias=negm[:, :1])
                zs = tiny.tile([P, 1], F32, tag="c1", name="zs")
                nc.vector.reduce_sum(zs[:, :], sc8[:, :k], axis=AXL.X)
                nc.vector.reciprocal(zs[:, :], zs[:, :])
                nc.vector.tensor_scalar(zs[:, :], zs[:, :], gscale, None,
                                        op0=ALU.mult)
                nc.vector.tensor_scalar(sc8[:, :k], sc8[:, :k], zs[:, :1], None,
                                        op0=ALU.mult)
                acc = sm.tile([P, H], F32, tag=acc_tag, name="acc" + acc_tag)
                nc.vector.memset(acc[:, :], 0.0)
                for kk in range(k):
                    g = gath.tile([P, H], BF16, tag="g", name="g")
                    nc.gpsimd.indirect_dma_start(
                        out=g[:, :], out_offset=None, in_=vals_dram,
                        in_offset=bass.IndirectOffsetOnAxis(
                            ap=idxu[:, kk:kk + 1], axis=0))
                    nc.vector.scalar_tensor_tensor(
                        out=acc[:, :], in0=g[:, :], scalar=sc8[:, kk:kk + 1],
                        in1=acc[:, :], op0=ALU.mult, op1=ALU.add)
                return acc

            def transpose_into(dst, src):
                """dst [P, HT, P] view <- transpose of src [P, H]."""
                for hi in range(HT):
                    pst = ps_tr.tile([P, P], F32, tag="tr", name="trf")
                    nc.tensor.transpose(out=pst[:], in_=src[:, hi * P:(hi + 1) * P],
                                        identity=ident[:])
                    nc.vector.tensor_copy(dst[:, hi, :], pst[:])

            # episodic: one gathered buffer, rank-major global indices
            for ch in range(N // 512):
                sim_chunk(qTp, ag_nek_out, ch, 0, ch, cand_v_e, cand_i_e,
                          ch * 512)
            # episodic merge+gather+weighted-sum overlaps the semantic sims
            accT_e = big.tile([P, NBT, HT, P], BF16, tag="kT", name="accTe")
            for bt in range(NBT):
                acc_e = topk_attend(cand_v_e[bt][:], cand_i_e[bt][:], EP_K,
                                    invq_p, bt, ag_ev_out[:, :],
                                    gw_pre[bt][:, 1:2], "sl1")
                transpose_into(accT_e[:, bt], acc_e)
            # semantic: iterate AG-buffer-major so sims start as soon as the
            # first semantic AllGather lands (buffer i holds local chunk i of
            # every rank; global index = r * ML + i * 512 + local)
            ch = 0
            for i in range(ML // 512):
                for r in range(NCORES):
                    sim_chunk(qsP, ag_nks_out[i], r, 0, ch, cand_v_s,
                              cand_i_s, r * ML + i * 512)
                    ch += 1

            # ===================================================================
            # Phase F: per-b-tile merge, softmax, gather-attend, blend, out
            # ===================================================================


            # Pass 1: per-b-tile merged top-k, gathers, weighted sums; store
            # transposed accumulators for all b-tiles (weights read once below).
            accT_s = big.tile([P, NBT, HT, P], BF16, tag="xTin", name="accTs")
            bl_all = big.tile([P, NBT, H], F32, tag="bl", name="bl_all")
            # Pass 2a: bl = gate0*w_out + acc_e @ W_eo (PE) — overlaps the
            # semantic merge below, which is DVE/DMA-bound.
            for jc in range(H // 512):
                cs = slice(jc * 512, (jc + 1) * 512)
                wsn = s512p.tile([S, 512], F32, tag="s512", name="wsn2")
                nc.sync.dma_start(wsn[:S, :], work_slots[:, cs])
                psos = [ps_mm.tile([P, 512], F32, tag="mm", name=f"pso{i}")
                        for i in range(NBT)]
                for bt in range(NBT):
                    nc.tensor.matmul(psos[bt][:], ewT_pre[bt][:, :],
                                     wsn[:S, :], start=True, stop=False)
                for hi in range(HT):
                    wt = wtp.tile([P, 512], BF16, tag="wt", name="wto")
                    nc.sync.dma_start(wt[:], weo16[hi * P:(hi + 1) * P, cs])
                    for bt in range(NBT):
                        nc.tensor.matmul(
                            psos[bt][:], accT_e[:, bt, hi, :], wt[:],
                            start=False, stop=(hi == HT - 1))
                for bt in range(NBT):
                    nc.vector.tensor_copy(bl_all[:, bt, cs], psos[bt][:])
            # Pass 1s: semantic merge + gathers (DVE/DMA)
            for bt in range(NBT):
                acc_s = topk_attend(cand_v_s[bt][:], cand_i_s[bt][:], SEM_K,
                                    invqs_p, bt, semv16, gw_pre[bt][:, 2:3],
                                    "sl2")
                transpose_into(accT_s[:, bt], acc_s)
            # Pass 2b: bl += acc_s @ W_so
            for jc in range(H // 512):
                cs = slice(jc * 512, (jc + 1) * 512)
                psos = [ps_mm.tile([P, 512], F32, tag="mm", name=f"psob{i}")
                        for i in range(NBT)]
                for hi in range(HT):
                    wt = wtp.tile([P, 512], BF16, tag="wt", name="wtob")
                    nc.sync.dma_start(wt[:], wso16[hi * P:(hi + 1) * P, cs])
                    for bt in range(NBT):
                        nc.tensor.matmul(
                            psos[bt][:], accT_s[:, bt, hi, :], wt[:],
                            start=(hi == 0), stop=(hi == HT - 1))
                for bt in range(NBT):
                    nc.vector.tensor_add(bl_all[:, bt, cs],
                                         bl_all[:, bt, cs], psos[bt][:])


            # Pass 3: xo = bl @ W_ro (W_ro read once)
            blT = big.tile([P, NBT, HT, P], BF16, tag="kT", name="blT")
            for bt in range(NBT):
                transpose_into(blT[:, bt], bl_all[:, bt, :])
            xo_all = big.tile([P, NBT, H], F32, tag="xTin", name="xo_all")
            for jc in range(H // 512):
                cs = slice(jc * 512, (jc + 1) * 512)
                psos = [ps_mm.tile([P, 512], F32, tag="mm", name=f"psro{i}")
                        for i in range(NBT)]
                for hi in range(HT):
                    wt = wtp.tile([P, 512], BF16, tag="wt", name="wtro")
                    nc.sync.dma_start(
                        wt[:], wro16[hi * P:(hi + 1) * P, cs])
                    for bt in range(NBT):
                        nc.tensor.matmul(
                            psos[bt][:], blT[:, bt, hi, :], wt[:],
                            start=(hi == 0), stop=(hi == HT - 1))
                for bt in range(NBT):
                    nc.vector.tensor_copy(xo_all[:, bt, cs], psos[bt][:])

            # Pass 4: layernorm + affine + output (gamma/beta loaded once)
            gbc = sm.tile([P, H], F32, tag="sl1", name="gbc")
            bbc = sm.tile([P, H], F32, tag="sl2", name="bbc")
            for jc in range(H // 512):
                cs = slice(jc * 512, (jc + 1) * 512)
                grow = rows.tile([1, 512], F32, tag="crow", name="grow", bufs=1)
                nc.sync.dma_start(grow[:1, :], ln_gamma[None, cs])
                nc.gpsimd.partition_broadcast(gbc[:, cs], grow[:1, :])
                brow = rows.tile([1, 512], F32, tag="crow", name="brow", bufs=1)
                nc.sync.dma_start(brow[:1, :], ln_beta[None, cs])
                nc.gpsimd.partition_broadcast(bbc[:, cs], brow[:1, :])
            for bt in range(NBT):
                xo = xo_all[:, bt, :]
                mu = tiny.tile([P, 1], F32, tag="c1", name="mu")
                nc.vector.reduce_sum(mu[:, :], xo, axis=AXL.X)
                nc.vector.tensor_scalar_mul(mu[:, :], mu[:, :], -1.0 / H)
                nc.vector.tensor_scalar(xo, xo, mu[:, :1], None, op0=ALU.add)
                vs = tiny.tile([P, 1], F32, tag="c1", name="vs")
                for jc in range(H // 512):
                    sqc = s512p.tile([P, 512], F32, tag="s512", name="sqc")
                    vc = tiny.tile([P, 1], F32, tag="c64", name="vc")
                    nc.scalar.activation(sqc[:, :],
                                         xo[:, jc * 512:(jc + 1) * 512],
                                         AF.Square, accum_out=vc[:, :1])
                    if jc == 0:
                        nc.vector.tensor_copy(vs[:, :], vc[:, :1])
                    else:
                        nc.vector.tensor_add(vs[:, :], vs[:, :], vc[:, :1])
                nc.vector.tensor_scalar_mul(vs[:, :], vs[:, :], 1.0 / H)
                nc.vector.tensor_scalar_add(vs[:, :], vs[:, :], LN_EPS)
                nc.scalar.sqrt(vs[:, :], vs[:, :])
                nc.vector.reciprocal(vs[:, :], vs[:, :])
                nc.vector.tensor_scalar(xo, xo, vs[:, :1], None, op0=ALU.mult)
                nc.vector.tensor_mul(xo, xo, gbc[:, :])
                nc.vector.tensor_add(xo, xo, bbc[:, :])
                nc.sync.dma_start(out_s[bt * P:(bt + 1) * P, :], xo)

    nc.finalize()
    return nc


_NC_CACHE = None
LAST_EXEC_NS = None


def _pack_xpair(x):
    """[R,H] f32 -> [R//512, P, 2, HT, 512] bf16 pair, pre-transposed to
    the on-chip tile layout: pk[ch, p, half, hi, r] = split(x)[half][
    ch*512+r, hi*128+p]."""
    hi_, lo_ = _split_bf16(x)
    def lay(a):
        return a.reshape(-1, HT, P).transpose(2, 1, 0)   # [P, HT, R]
    pk = np.stack([lay(hi_), lay(lo_)], axis=1)          # [P, 2, HT, R]
    R = x.shape[0]
    return np.ascontiguousarray(
        np.stack([pk[..., i * 512:(i + 1) * 512]
                  for i in range(R // 512)], axis=0))


def _pack_wpair(w):
    """[H,H] f32 -> [HT, P, 2, HT, P] bf16 pair in wcP tile layout:
    packed[j, p, half, hi, c] = split(W)[half][hi*128+p, j*128+c]."""
    hi_, lo_ = _split_bf16(w)
    def lay(a):
        # [hi, p, j, c] -> [j, p, hi, c]
        return np.ascontiguousarray(
            a.reshape(HT, P, HT, P).transpose(2, 1, 0, 3))
    return np.ascontiguousarray(
        np.stack([lay(hi_), lay(lo_)], axis=2))


def _split_bf16(x):
    """two-term bf16 decomposition: x ~= hi + lo to ~16 mantissa bits."""
    import ml_dtypes
    bf = ml_dtypes.bfloat16
    x = np.ascontiguousarray(np.asarray(x), dtype=np.float32)
    hi = x.astype(bf)
    lo = (x - hi.astype(np.float32)).astype(bf)
    return hi, lo


def kernel(**inputs) -> np.ndarray:
    global _NC_CACHE
    if _NC_CACHE is None:
        _NC_CACHE = build()
    nc = _NC_CACHE

    def arr(x):
        return np.ascontiguousarray(np.asarray(x), dtype=np.float32)


    wq_pk = _pack_wpair(inputs["W_query"])
    wek_pk = _pack_wpair(inputs["W_ek"])
    wsq_pk = _pack_wpair(inputs["W_sq"])
    wsk_pk = _pack_wpair(inputs["W_sk"])
    wev16, _ = _split_bf16(inputs["W_ev"])
    weo16, _ = _split_bf16(inputs["W_eo"])
    wso16, _ = _split_bf16(inputs["W_so"])
    wro16, _ = _split_bf16(inputs["W_ro"])
    semv16, _ = _split_bf16(inputs["sem_values"])

    in_maps = []
    for c in range(NCORES):
        in_maps.append({
            "query_pk": _pack_xpair(inputs["query"][c * BL:(c + 1) * BL]),
            "ep_pk": _pack_xpair(inputs["ep_store"][c * NL:(c + 1) * NL]),
            "semk_pk": _pack_xpair(inputs["sem_keys"][c * ML:(c + 1) * ML]),
            "ep_imp_s": arr(inputs["ep_importance"][c * NL:(c + 1) * NL]),
            "ep_ts_s": arr(inputs["ep_timestamps"][c * NL:(c + 1) * NL]),
            "ep_imp": arr(inputs["ep_importance"]),
            "ep_ts": arr(inputs["ep_timestamps"]),
            "semv16": semv16,
            "wq_pk": wq_pk,
            "wek_pk": wek_pk,
            "wsq_pk": wsq_pk,
            "wsk_pk": wsk_pk,
            "wev16": wev16,
            "weo16": weo16,
            "wso16": wso16,
            "wro16": wro16,
            "work_slots": arr(inputs["work_slots"]),
            "gate_W1": arr(inputs["gate_W1"]),
            "gate_b1": arr(inputs["gate_b1"]),
            "gate_W2": arr(inputs["gate_W2"]),
            "gate_b2": arr(inputs["gate_b2"]),
            "ln_gamma": arr(inputs["ln_gamma"]),
            "ln_beta": arr(inputs["ln_beta"]),
        })
    res = run_bass_kernel_spmd(nc, in_maps, core_ids=list(range(NCORES)))
    return np.concatenate([res.results[c]["out_s"] for c in range(NCORES)],
                          axis=0)



# revision 44
# speedup vs baseline: 1.0312x; 1.0312x over previous
"""ONIMemoryHub kernel for 8 Trainium2 NeuronCores (Bass/Tile).

Sharding: data-parallel over batch for the query side; episodic store and
semantic memory sharded across cores for the key/value projections, with
AllGathers of the projected (normalized, pre-scaled) keys/values.

Schedule notes (v2): the PE instruction stream is kept free of stalls by
emitting off-engine work (top-k merges, norms, layernorm) interleaved
between matmul blocks whose inputs are already resident:
  - projection column norms are fused into the projection evacuation
  - gate/work blocks interleave with the episodic sim chunks
  - episodic merges/transposes interleave with the semantic sim chunks
  - semantic merges interleave with the W_eo output pass
  - W_so/W_ro passes run b-tile-major with a streaming layernorm so the
    kernel tail is one b-tile's LN instead of a full LN pass.

kernel(**inputs) takes FULL inputs (as produced by reference.setup_inputs())
and returns the FULL [4096, 2048] output.
"""
import math

import numpy as np

import concourse.bass as bass
import concourse.mybir as mybir
import concourse.tile as tile
from concourse import bacc
from concourse.bass_utils import run_bass_kernel_spmd
from concourse.masks import make_identity

AF = mybir.ActivationFunctionType
AXL = mybir.AxisListType
ALU = mybir.AluOpType

NCORES = 8
B, H, N, M, S = 4096, 2048, 4096, 16384, 64
BL, NL, ML = B // NCORES, N // NCORES, M // NCORES   # 512, 512, 2048
HT = H // 128                                        # 16 h-tiles
P = 128
NBT = BL // P                                        # 4 b-tiles
EP_K = 8
SEM_K = 4
LN_EPS = 1e-5
RECENCY = 0.01   # 1 - RECENCY_DECAY

F32 = mybir.dt.float32
BF16 = mybir.dt.bfloat16
U32 = mybir.dt.uint32


def build():
    nc = bacc.Bacc("TRN2", target_bir_lowering=False, debug=False,
                   num_devices=NCORES)

    def din(name, shape, dt=F32):
        return nc.dram_tensor(name, shape, dt, kind="ExternalInput").ap()

    # per-core slices: host-split bf16 pairs, pre-transposed to tile layout
    query_pk = din("query_pk", [1, P, 2, HT, 512], BF16)
    ep_pk = din("ep_pk", [1, P, 2, HT, 512], BF16)
    semk_pk = din("semk_pk", [ML // 512, P, 2, HT, 512], BF16)
    ep_imp_s = din("ep_imp_s", [NL])
    ep_ts_s = din("ep_ts_s", [NL])
    # replicated
    ep_imp = din("ep_imp", [N])
    ep_ts = din("ep_ts", [N])
    semv16 = din("semv16", [M, H], BF16)
    wq_pk = din("wq_pk", [HT, P, 2, HT, P], BF16)
    wek_pk = din("wek_pk", [HT, P, 2, HT, P], BF16)
    wsq_pk = din("wsq_pk", [HT, P, 2, HT, P], BF16)
    wsk_pk = din("wsk_pk", [HT, P, 2, HT, P], BF16)
    wev16 = din("wev16", [H, H], BF16)
    weo16 = din("weo16", [H, H], BF16)
    wso16 = din("wso16", [H, H], BF16)
    wro16 = din("wro16", [H, H], BF16)
    ws16 = din("ws16", [S, H], BF16)
    gwk_pk = din("gwk_pk", [P, 2, HT, 128], BF16)
    gate_b1 = din("gate_b1", [64])
    gate_W2 = din("gate_W2", [64, 3])
    gate_b2 = din("gate_b2", [3])
    ln_gamma = din("ln_gamma", [H])
    ln_beta = din("ln_beta", [H])

    out_s = nc.dram_tensor("out_s", [BL, H], F32, kind="ExternalOutput").ap()

    with tile.TileContext(nc) as tc:
        with (
            tc.tile_pool(name="cst", bufs=1) as cst,
            tc.tile_pool(name="big", bufs=1) as big,
            tc.tile_pool(name="rows", bufs=1) as rows,
            tc.tile_pool(name="s512", bufs=2) as s512p,
            tc.tile_pool(name="wcol", bufs=2) as wcolp,
            tc.tile_pool(name="wtile", bufs=2) as wtp,
            tc.tile_pool(name="sm", bufs=2) as sm,
            tc.tile_pool(name="tiny", bufs=2) as tiny,
            tc.tile_pool(name="simb", bufs=2) as simb,
            tc.tile_pool(name="gath", bufs=2) as gath,
            tc.tile_pool(name="ps_mm", bufs=7, space="PSUM") as ps_mm,
            tc.tile_pool(name="ps_sml", bufs=1, space="PSUM") as ps_sml,
            tc.tile_pool(name="dram", bufs=1, space="DRAM") as dram,
        ):
            ident = cst.tile([P, P], F32)
            make_identity(nc, ident[:])
            ident16 = cst.tile([P, P], BF16)
            nc.scalar.activation(ident16[:], ident[:], AF.Copy)
            ones_col = cst.tile([P, 1], F32)
            nc.vector.memset(ones_col[:], 1.0)

            # ---------- helpers ----------
            # big slot chains (explicit liveness via shared tags):
            #   xTin: epT -> skT(x4) -> qsT -> accTs
            #   kT  : ekT -> ksT(x4) -> accTe -> blT
            #   bl  : qTp -> bl_all
            def emit_split(dst_hi, dst_lo, src_f32, tmp32):
                """bf16 two-term split: hi = bf16(x), lo = bf16(x - hi)."""
                nc.scalar.activation(dst_hi, src_f32, AF.Copy)
                nc.gpsimd.tensor_copy(tmp32, dst_hi)
                nc.vector.tensor_tensor(out=tmp32, in0=src_f32, in1=tmp32,
                                        op=ALU.subtract)
                nc.scalar.activation(dst_lo, tmp32, AF.Copy)

            def wcol_pair(w_pk, j):
                wcP = wcolp.tile([P, 2, HT, P], BF16, tag="wcp", name="wcp")
                nc.sync.dma_start(wcP[:], w_pk[j])
                return wcP

            def norm_row_finish(psn, extra_row=None):
                """[1,512] inv-norm row from accumulated sum-of-squares."""
                row = rows.tile([1, 512], F32, tag="nrow", name="nrow", bufs=2)
                nc.vector.tensor_copy(row[:1, :], psn[:1, :])
                nc.scalar.sqrt(row[:1, :], row[:1, :])
                nc.vector.tensor_scalar_max(row[:1, :], row[:1, :], 1e-12)
                nc.vector.reciprocal(row[:1, :], row[:1, :])
                if extra_row is not None:
                    nc.vector.tensor_mul(row[:1, :], row[:1, :], extra_row)
                return row

            def project3(xP, w_pk, name, tag, mode, mid_emit=None,
                         first_wc=None):
                """(x @ W).T via 3-term bf16 split matmuls; xP is a pair.

                mode "f32": returns (yT, psn) — f32 tile + sum-of-squares
                psum row (norm fused into the evacuation).
                mode "pair": returns (yP, psn) — bf16 pair tile + norm psum.
                mid_emit() is called after the j==3 block so a prefetch DMA
                can ride the SP queue behind the first few weight columns.
                """
                psn = ps_sml.tile([1, 512], F32, tag="sml", name="npsum")
                if mode == "f32":
                    yT = big.tile([P, HT, 512], F32, tag=tag, name=name)
                else:
                    yP = big.tile([P, 2, HT, 512], BF16, tag=tag, name=name)
                # norm accumulation runs at lag 1 so the PE never waits on the
                # ACT square of the chunk it just produced.
                sqs = [None] * HT
                for j in range(HT):
                    wcP = first_wc if (j == 0 and first_wc is not None) \
                        else wcol_pair(w_pk, j)
                    pst = ps_mm.tile([P, 512], F32, tag="mm", name="projps")
                    for hi in range(HT):
                        nc.tensor.matmul(
                            pst[:], wcP[:, 0, hi, :], xP[:, 0, hi, :],
                            start=(hi == 0), stop=False)
                        nc.tensor.matmul(
                            pst[:], wcP[:, 0, hi, :], xP[:, 1, hi, :],
                            start=False, stop=False)
                        nc.tensor.matmul(
                            pst[:], wcP[:, 1, hi, :], xP[:, 0, hi, :],
                            start=False, stop=(hi == HT - 1))
                    sq = s512p.tile([P, 512], F32, tag="sqn", name="sqn")
                    nc.scalar.square(sq[:, :], pst[:])
                    sqs[j] = sq
                    if mode == "f32":
                        nc.scalar.activation(yT[:, j, :], pst[:], AF.Copy)
                    else:
                        tmp32 = s512p.tile([P, 512], F32, tag="s512",
                                           name="spj32")
                        emit_split(yP[:, 0, j, :], yP[:, 1, j, :], pst[:],
                                   tmp32[:])
                    if j >= 1:
                        nc.tensor.matmul(psn[:1, :], ones_col[:],
                                         sqs[j - 1][:, :],
                                         start=(j == 1), stop=False)
                    if mid_emit is not None and j in (3, 6, 9, 12):
                        mid_emit((j - 3) // 3)
                nc.tensor.matmul(psn[:1, :], ones_col[:], sqs[HT - 1][:, :],
                                 start=False, stop=True)
                if mode == "f32":
                    return yT, psn
                return yP, psn

            def store_pair_to_ag(xT, ag_in):
                """split scaled f32 keys and store bf16 pair to AG input."""
                for hi in range(HT):
                    sth = s512p.tile([P, 512], BF16, tag="st16h", name="sth",
                                     bufs=5)
                    stl = s512p.tile([P, 512], BF16, tag="st16l", name="stl",
                                     bufs=5)
                    tmp32 = s512p.tile([P, 512], F32, tag="s512", name="spg32")
                    emit_split(sth[:], stl[:], xT[:, hi, :], tmp32[:])
                    nc.sync.dma_start(ag_in[0, hi * P:(hi + 1) * P, :], sth[:])
                    nc.sync.dma_start(ag_in[1, hi * P:(hi + 1) * P, :], stl[:])

            def scale_cols(xT, scale_row):
                bc = s512p.tile([P, 512], F32, tag="s512", name="bcn")
                nc.gpsimd.partition_broadcast(bc[:, :], scale_row[:1, :])
                for hi in range(HT):
                    nc.vector.tensor_mul(xT[:, hi, :], xT[:, hi, :], bc[:, :])

            # ===================================================================
            # Phase M: sharded memory-side projections + AllGathers
            # ===================================================================
            ag_nek_in = dram.tile([2, H, NL], BF16, name="ag_nek_in")
            ag_nek_out = dram.tile([NCORES, 2, H, NL], BF16,
                                   addr_space="Shared", name="ag_nek_out")
            ag_ev_in = dram.tile([NL, H], BF16, name="ag_ev_in")
            ag_ev_out = dram.tile([N, H], BF16, addr_space="Shared",
                                  name="ag_ev_out")
            ag_nks_in = [dram.tile([2, H, 512], BF16, name=f"ag_nks_in{i}")
                         for i in range(ML // 512)]
            ag_nks_out = [dram.tile([NCORES, 2, H, 512], BF16,
                                    addr_space="Shared", name=f"ag_nks_out{i}")
                          for i in range(ML // 512)]

            # semantic-key chunk loads: double-buffered on alternating big
            # slots (bl/xTin), emitted via project3 mid_emit hooks so each
            # 12.6us DMA hides under the previous projection.
            skPs = [None] * (ML // 512)
            qTin_box = [None]

            def load_sk(mc, piece):
                """quarter-piece prefetch of a semantic-key chunk."""
                if piece == 0:
                    skPs[mc] = big.tile([P, 2, HT, 512], BF16,
                                        tag="bl" if mc % 2 == 0 else "xTin",
                                        name=f"skT{mc}")
                hs = slice(piece * 4, (piece + 1) * 4)
                nc.sync.dma_start(skPs[mc][:, :, hs], semk_pk[mc, :, :, hs])

            def load_qTin(piece):
                if piece == 0:
                    qTin_box[0] = big.tile([P, 2, HT, 512], BF16, tag="bl",
                                           name="qTin")
                hs = slice(piece * 4, (piece + 1) * 4)
                nc.sync.dma_start(qTin_box[0][:, :, hs], query_pk[0, :, :, hs])

            # First weight column rides the DMA queue ahead of epP so the very
            # first matmul chain starts as soon as epP's first piece lands;
            # epP is split so early hi-tiles arrive (and compute) first.
            wc_ek0 = wcol_pair(wek_pk, 0)
            epP = big.tile([P, 2, HT, 512], BF16, tag="xTin", name="epT")
            nc.sync.dma_start(epP[:, :, :HT // 4], ep_pk[0, :, :, :HT // 4])
            nc.sync.dma_start(epP[:, :, HT // 4:HT // 2],
                              ep_pk[0, :, :, HT // 4:HT // 2])
            nc.sync.dma_start(epP[:, :, HT // 2:], ep_pk[0, :, :, HT // 2:])

            # ---- episodic recency/importance weights (off-PE, overlaps ekT)
            def rec_weight(imp_ap, ts_ap, shape, tagb):
                """(1+imp)*exp(-|1-ts|*RECENCY) elementwise; returns tile."""
                impt = rows.tile(shape, F32, tag=tagb + "i", name="impt")
                tst = rows.tile(shape, F32, tag=tagb + "t", name="tst")
                nc.sync.dma_start(impt[:shape[0], :], imp_ap)
                nc.sync.dma_start(tst[:shape[0], :], ts_ap)
                s = tst[:shape[0], :]
                nc.scalar.activation(s, s, AF.Copy, bias=0.0, scale=-1.0)
                nc.vector.tensor_scalar_add(s, s, 1.0)
                nc.scalar.activation(s, s, AF.Abs)
                nc.scalar.activation(s, s, AF.Exp, scale=-RECENCY)
                si = impt[:shape[0], :]
                nc.vector.tensor_scalar_add(si, si, 1.0)
                nc.vector.tensor_mul(si, si, s)
                return impt

            wfull = rec_weight(ep_imp.rearrange("(p c) -> p c", p=P),
                               ep_ts.rearrange("(p c) -> p c", p=P),
                               [P, N // P], "wf")
            wpart = rows.tile([P, 1], F32, tag="wpart", name="wpart")
            nc.vector.reduce_sum(wpart[:, :], wfull[:, :], axis=AXL.X)
            pssum = ps_sml.tile([1, 512], F32, tag="sml", name="wsps")
            nc.tensor.matmul(pssum[:1, :1], ones_col[:], wpart[:, :],
                             start=True, stop=True)
            wsum = rows.tile([1, 1], F32, tag="wsum", name="wsum")
            nc.vector.tensor_copy(wsum[:1, :], pssum[:1, :1])
            nc.vector.tensor_scalar_add(wsum[:1, :], wsum[:1, :], 1e-8)
            nc.vector.reciprocal(wsum[:1, :], wsum[:1, :])
            wloc = rec_weight(ep_imp_s[None, :], ep_ts_s[None, :], [1, NL], "wl")
            nc.vector.tensor_scalar(wloc[:1, :], wloc[:1, :], wsum[:1, :1], None,
                                    op0=ALU.mult)

            # ---- episodic keys: project (norm fused), scale, store, AG;
            # skT0's load rides behind the early ek weight columns.
            ekT, psn_ek = project3(epP, wek_pk, "ekT", "kT", "f32",
                                   mid_emit=lambda p: load_sk(0, p),
                                   first_wc=wc_ek0)
            inv_ek = norm_row_finish(psn_ek, extra_row=wloc[:1, :])
            scale_cols(ekT, inv_ek)
            store_pair_to_ag(ekT, ag_nek_in)
            nc.gpsimd.collective_compute(
                "AllGather", ALU.bypass,
                replica_groups=[list(range(NCORES))],
                ins=[ag_nek_in.opt()], outs=[ag_nek_out.opt()])

            # ---- e_vals natural layout [NL, H]; bf16 single term
            for jc in range(H // 512):
                psts = [ps_mm.tile([P, 512], F32, tag="mm", name=f"evps{i}")
                        for i in range(NL // P)]
                for hi in range(HT):
                    wt16 = wtp.tile([P, 512], BF16, tag="wt16", name="wt16",
                                    bufs=2)
                    nc.sync.dma_start(
                        wt16[:],
                        wev16[hi * P:(hi + 1) * P, jc * 512:(jc + 1) * 512])
                    for nt in range(NL // P):
                        ns = slice(nt * P, (nt + 1) * P)
                        nc.tensor.matmul(
                            psts[nt][:], epP[:, 0, hi, ns], wt16[:],
                            start=(hi == 0), stop=(hi == HT - 1))
                for nt in range(NL // P):
                    evs = s512p.tile([P, 512], BF16, tag="evo16", name="evout", bufs=1)
                    nc.vector.tensor_copy(evs[:], psts[nt][:])
                    nc.sync.dma_start(
                        ag_ev_in[nt * P:(nt + 1) * P, jc * 512:(jc + 1) * 512],
                        evs[:])
            nc.gpsimd.collective_compute(
                "AllGather", ALU.bypass,
                replica_groups=[list(range(NCORES))],
                ins=[ag_ev_in.opt()], outs=[ag_ev_out.opt()])

            # ---- semantic keys: 4 chunks of 512 (loads via mid_emit hooks)
            for mc in range(ML // 512):
                nxt = (lambda p, m=mc + 1: load_sk(m, p)) \
                    if mc + 1 < ML // 512 else load_qTin
                ksT, psn_ks = project3(skPs[mc], wsk_pk, f"ksT{mc}", "kT",
                                       "f32", mid_emit=nxt)
                inv_ks = norm_row_finish(psn_ks)
                scale_cols(ksT, inv_ks)
                store_pair_to_ag(ksT, ag_nks_in[mc])
                nc.gpsimd.collective_compute(
                    "AllGather", ALU.bypass,
                    replica_groups=[list(range(NCORES))],
                    ins=[ag_nks_in[mc].opt()], outs=[ag_nks_out[mc].opt()])

            # ===================================================================
            # Phase Q: query-side projections (norms fused)
            # ===================================================================
            qTinP = qTin_box[0]
            qTp, psn_q = project3(qTinP, wq_pk, "qT", "kT", "pair")
            inv_q = norm_row_finish(psn_q)
            qsP, psn_qs = project3(qTp, wsq_pk, "qsT", "xTin", "pair")
            inv_qs = norm_row_finish(psn_qs)

            # transpose inv rows -> per-partition [128, NBT] via DRAM bounce
            invq_p = cst.tile([P, NBT], F32, name="invq_p")
            invqs_p = cst.tile([P, NBT], F32, name="invqs_p")
            bounce = dram.tile([2, BL], F32, name="bounce")
            nc.sync.dma_start(bounce[0:1, :], inv_q[:1, :])
            nc.sync.dma_start(bounce[1:2, :], inv_qs[:1, :])
            nc.sync.dma_start(
                invq_p[:, :], bounce[0:1, :].rearrange("o (t p) -> (o p) t", p=P))
            nc.sync.dma_start(
                invqs_p[:, :], bounce[1:2, :].rearrange("o (t p) -> (o p) t", p=P))

            def bcast_row(dram_row, width, pool, tag, name, dt=F32):
                row = rows.tile([1, width], F32, tag="crow", name="crow", bufs=1)
                nc.sync.dma_start(row[:1, :], dram_row)
                src = row[:1, :]
                if dt != F32:
                    row16 = rows.tile([1, width], dt, tag="crow16",
                                      name="crow16", bufs=1)
                    nc.scalar.activation(row16[:1, :], row[:1, :], AF.Copy)
                    src = row16[:1, :]
                t = pool.tile([P, width], dt, tag=tag, name=name, bufs=1)
                nc.gpsimd.partition_broadcast(t[:, :], src)
                return t

            b1bc = bcast_row(gate_b1[None, :], 64, cst, "b1bc", "b1bc")
            b2bc = bcast_row(gate_b2[None, :], 3, cst, "b2bc", "b2bc")
            # gate+work concatenated projection weights (bf16 pair); rides the
            # wcol ring slot freed after the last qs weight column.
            gwk = wcolp.tile([P, 2, HT, 128], BF16, tag="wcp", name="gwk")
            nc.sync.dma_start(gwk[:], gwk_pk)
            gw2 = cst.tile([64, 3], F32, name="gw2")
            nc.sync.dma_start(gw2[:, :], gate_W2)

            inv_sqrt_h = 1.0 / math.sqrt(H)
            ewT_pre = [None] * NBT
            gw_pre = [None] * NBT

            gate_st = [None] * NBT

            def emit_gate_a(bt):
                """Gate/work stage A: fused matmul (cols 0:64 gate hidden,
                64:128 work logits; stationary q bf16-hi, moving bf16 pair of
                hstack(gate_W1, work_slots.T)) + the off-PE softmax chain."""
                psg = ps_sml.tile([P, 128], F32, tag="sml", name="psg")
                bs = slice(bt * P, (bt + 1) * P)
                for hi in range(HT):
                    nc.tensor.matmul(
                        psg[:, :], qTp[:, 0, hi, bs], gwk[:, 0, hi, :],
                        start=(hi == 0), stop=False)
                    nc.tensor.matmul(
                        psg[:, :], qTp[:, 0, hi, bs], gwk[:, 1, hi, :],
                        start=False, stop=(hi == HT - 1))
                hid = tiny.tile([P, 64], F32, tag="c64", name="hid")
                nc.vector.tensor_add(hid[:, :], psg[:, :64], b1bc[:, :])
                nc.scalar.activation(hid[:, :], hid[:, :], AF.Silu)
                wmax = tiny.tile([P, 1], F32, tag="c1", name="wmax")
                nc.vector.reduce_max(wmax[:, :], psg[:, 64:], axis=AXL.X)
                nc.vector.tensor_scalar_mul(wmax[:, :], wmax[:, :], -inv_sqrt_h)
                ew = tiny.tile([P, S], F32, tag="cew", name="ew")
                nc.scalar.activation(ew[:, :], psg[:, 64:], AF.Exp,
                                     bias=wmax[:, :1], scale=inv_sqrt_h)
                zw = tiny.tile([P, 1], F32, tag="czw", name="zw")
                nc.vector.reduce_sum(zw[:, :], ew[:, :], axis=AXL.X)
                nc.vector.reciprocal(zw[:, :], zw[:, :])
                gate_st[bt] = (hid, ew, zw)

            def emit_gate_b(bt):
                """Gate/work stage B: transposes + gate MLP tail; its PE ops
                depend only on stage-A results finished a sim chunk ago."""
                hid, ew, zw = gate_st[bt]
                psht = ps_sml.tile([64, P], F32, tag="sml", name="hidtp")
                nc.tensor.transpose(out=psht[:64, :], in_=hid[:, :],
                                    identity=ident[:])
                hidT = tiny.tile([64, P], F32, tag="c128", name="hidT")
                nc.vector.tensor_copy(hidT[:, :], psht[:64, :])
                psg2 = ps_sml.tile([P, 3], F32, tag="sml", name="psg2")
                nc.tensor.matmul(psg2[:, :3], hidT[:, :], gw2[:, :],
                                 start=True, stop=True)
                gl = cst.tile([P, 3], F32, name=f"gl{bt}")
                nc.vector.tensor_add(gl[:, :], psg2[:, :3], b2bc[:, :])
                gmax = tiny.tile([P, 1], F32, tag="c1", name="gmax")
                nc.vector.reduce_max(gmax[:, :], gl[:, :], axis=AXL.X)
                nc.vector.tensor_scalar_mul(gmax[:, :], gmax[:, :], -1.0)
                nc.scalar.activation(gl[:, :], gl[:, :], AF.Exp, bias=gmax[:, :1])
                gz = tiny.tile([P, 1], F32, tag="c1", name="gz")
                nc.vector.reduce_sum(gz[:, :], gl[:, :], axis=AXL.X)
                nc.vector.reciprocal(gz[:, :], gz[:, :])
                nc.vector.tensor_scalar(gl[:, :], gl[:, :], gz[:, :1], None,
                                        op0=ALU.mult)
                gw_pre[bt] = gl
                # fold softmax normalization AND gate weight 0 into ew
                nc.vector.tensor_tensor(out=zw[:, :], in0=zw[:, :],
                                        in1=gl[:, 0:1], op=ALU.mult)
                nc.vector.tensor_scalar(ew[:, :], ew[:, :], zw[:, :1], None,
                                        op0=ALU.mult)

            def emit_gate_c(bt):
                """Gate/work stage C: transpose of the folded work probs."""
                _, ew, _ = gate_st[bt]
                pset = ps_sml.tile([S, P], F32, tag="sml", name="ewtp")
                nc.tensor.transpose(out=pset[:S, :], in_=ew[:, :],
                                    identity=ident[:])
                ewT = cst.tile([S, P], BF16, name=f"ewT{bt}")
                nc.vector.tensor_copy(ewT[:, :], pset[:S, :])
                ewT_pre[bt] = ewT

            # ===================================================================
            # Phase S: similarity + per-chunk top-8 candidates
            # ===================================================================
            cand_v_e = [big.tile([P, (N // 512) * 8], F32, tag=f"cve{bt}",
                                 name=f"cve{bt}") for bt in range(NBT)]
            cand_i_e = [big.tile([P, (N // 512) * 8], F32, tag=f"cie{bt}",
                                 name=f"cie{bt}") for bt in range(NBT)]
            cand_v_s = [big.tile([P, (M // 512) * 8], F32, tag=f"cvs{bt}",
                                 name=f"cvs{bt}") for bt in range(NBT)]
            cand_i_s = [big.tile([P, (M // 512) * 8], F32, tag=f"cis{bt}",
                                 name=f"cis{bt}") for bt in range(NBT)]

            def sim_chunk(xP, kd, r, ch, cand_v, cand_i, base):
                """sims of all 4 b-tiles vs bf16-pair keys kd[r, :, h, :]."""
                psts = [ps_mm.tile([P, 512], F32, tag="mm", name=f"simps{i}")
                        for i in range(NBT)]
                for hi in range(HT):
                    kth = s512p.tile([P, 512], BF16, tag="st16h", name="kth",
                                     bufs=5)
                    ktl = s512p.tile([P, 512], BF16, tag="st16l", name="ktl",
                                     bufs=5)
                    nc.sync.dma_start(
                        kth[:], kd[r, 0, hi * P:(hi + 1) * P, :])
                    nc.sync.dma_start(
                        ktl[:], kd[r, 1, hi * P:(hi + 1) * P, :])
                    for bt in range(NBT):
                        bs = slice(bt * P, (bt + 1) * P)
                        nc.tensor.matmul(
                            psts[bt][:], xP[:, 0, hi, bs], kth[:],
                            start=(hi == 0), stop=False)
                        nc.tensor.matmul(
                            psts[bt][:], xP[:, 0, hi, bs], ktl[:],
                            start=False, stop=False)
                        nc.tensor.matmul(
                            psts[bt][:], xP[:, 1, hi, bs], kth[:],
                            start=False, stop=(hi == HT - 1))
                for bt in range(NBT):
                    sc = simb.tile([P, 512], F32, tag="simc", name="simc",
                                   bufs=2)
                    nc.scalar.activation(sc[:], psts[bt][:], AF.Copy)
                    mx = simb.tile([P, 8], F32, tag="mx", name="mx")
                    mi = simb.tile([P, 8], U32, tag="mi", name="mi")
                    nc.vector.max(out=mx[:], in_=sc[:])
                    nc.vector.max_index(out=mi[:], in_max=mx[:], in_values=sc[:])
                    nc.vector.tensor_copy(cand_v[bt][:, ch * 8:(ch + 1) * 8],
                                          mx[:])
                    mif = simb.tile([P, 8], F32, tag="mif", name="mif")
                    nc.vector.tensor_copy(mif[:], mi[:])
                    nc.vector.tensor_scalar_add(
                        cand_i[bt][:, ch * 8:(ch + 1) * 8], mif[:],
                        float(base))

            def topk_attend(cand_v, cand_i, k, inv_p, bt, vals_dram, gscale,
                            acc_tag, bufs=2):
                """Merged top-k -> softmax (x gscale) -> gather + weighted sum."""
                top8 = tiny.tile([P, 8], F32, tag="c8", name="top8")
                nc.vector.max(out=top8[:], in_=cand_v[:])
                idxf = tiny.tile([P, 8], F32, tag="c8", name="idxf")
                eqm = s512p.tile([P, 256], F32, tag="sqn", name="eqm")
                for kk in range(k):
                    w = cand_v.shape[-1]
                    nc.vector.tensor_scalar(
                        eqm[:, :w], cand_v[:], top8[:, kk:kk + 1], None,
                        op0=ALU.is_equal)
                    nc.vector.tensor_tensor(
                        out=eqm[:, :w], in0=eqm[:, :w], in1=cand_i[:], op=ALU.mult)
                    nc.vector.reduce_sum(idxf[:, kk:kk + 1], eqm[:, :w], axis=AXL.X)
                idxu = tiny.tile([P, 8], U32, tag="c8u", name="idxu")
                nc.vector.tensor_copy(idxu[:, :k], idxf[:, :k])
                sc8 = tiny.tile([P, 8], F32, tag="c8", name="sc8")
                nc.vector.tensor_scalar(
                    sc8[:, :k], top8[:, :k], inv_p[:, bt:bt + 1], None,
                    op0=ALU.mult)
                negm = tiny.tile([P, 1], F32, tag="c1", name="negm")
                nc.vector.tensor_scalar_mul(negm[:, :], sc8[:, 0:1], -1.0)
                nc.scalar.activation(sc8[:, :k], sc8[:, :k], AF.Exp,
                                     bias=negm[:, :1])
                zs = tiny.tile([P, 1], F32, tag="c1", name="zs")
                nc.vector.reduce_sum(zs[:, :], sc8[:, :k], axis=AXL.X)
                nc.vector.reciprocal(zs[:, :], zs[:, :])
                nc.vector.tensor_scalar(zs[:, :], zs[:, :], gscale, None,
                                        op0=ALU.mult)
                nc.vector.tensor_scalar(sc8[:, :k], sc8[:, :k], zs[:, :1], None,
                                        op0=ALU.mult)
                acc = sm.tile([P, H], BF16, tag=acc_tag, name="acc" + acc_tag,
                              bufs=bufs)
                nc.vector.memset(acc[:, :], 0.0)
                for kk in range(k):
                    g = gath.tile([P, H], BF16, tag="g", name="g")
                    nc.gpsimd.indirect_dma_start(
                        out=g[:, :], out_offset=None, in_=vals_dram,
                        in_offset=bass.IndirectOffsetOnAxis(
                            ap=idxu[:, kk:kk + 1], axis=0))
                    nc.vector.scalar_tensor_tensor(
                        out=acc[:, :], in0=g[:, :], scalar=sc8[:, kk:kk + 1],
                        in1=acc[:, :], op0=ALU.mult, op1=ALU.add)
                return acc

            def transpose_into(dst, src, dt=BF16):
                """dst [P, HT, P] view <- transpose of src [P, H]."""
                idn = ident if dt == F32 else ident16
                for hi in range(HT):
                    pst = ps_mm.tile([P, P], dt, tag="mm", name="trf")
                    nc.tensor.transpose(out=pst[:], in_=src[:, hi * P:(hi + 1) * P],
                                        identity=idn[:])
                    nc.scalar.activation(dst[:, hi, :], pst[:], AF.Copy)

            # episodic sims: one gathered buffer, rank-major global indices;
            # gate/work stages (off-PE-latency-heavy) interleave with chunks
            # so each stage's PE ops only see dependencies already finished.
            gbc2 = [None] * (H // 512)
            bbc2 = [None] * (H // 512)
            for ch in range(N // 512):
                if 2 <= ch <= NBT + 1:
                    emit_gate_c(ch - 2)
                if 1 <= ch <= NBT:
                    emit_gate_b(ch - 1)
                if ch < NBT:
                    emit_gate_a(ch)
                if 4 <= ch < 4 + H // 512:
                    # LN gamma/beta broadcast tiles: the serial DMA<->POOL
                    # ping-pong hides under the remaining sim chunks.
                    jc = ch - 4
                    cs = slice(jc * 512, (jc + 1) * 512)
                    gbc2[jc] = bcast_row(ln_gamma[None, cs], 512, sm,
                                         f"gbc{jc}", f"gbc{jc}", BF16)
                    bbc2[jc] = bcast_row(ln_beta[None, cs], 512, sm,
                                         f"bbc{jc}", f"bbc{jc}", BF16)
                sim_chunk(qTp, ag_nek_out, ch, ch, cand_v_e, cand_i_e,
                          ch * 512)

            # episodic merges (DVE/gathers) overlap semantic sims (PE); the
            # accT_e transposes are emitted after a sem chunk each so the PE
            # queue never waits on a merge.
            accT_e = big.tile([P, NBT, HT, P], BF16, tag="kT", name="accTe")
            accT_s = big.tile([P, NBT, HT, P], BF16, tag="xTin", name="accTs")
            acc_e = [None] * NBT
            acc_s = [None] * NBT

            def emit_merge_e(bt):
                acc_e[bt] = topk_attend(cand_v_e[bt][:], cand_i_e[bt][:], EP_K,
                                        invq_p, bt, ag_ev_out[:, :],
                                        gw_pre[bt][:, 1:2], "sl1")

            def emit_merge_s(bt):
                acc_s[bt] = topk_attend(cand_v_s[bt][:], cand_i_s[bt][:], SEM_K,
                                        invqs_p, bt, semv16, gw_pre[bt][:, 2:3],
                                        "sl2", bufs=3)

            sem_seq = [(i, r) for i in range(ML // 512) for r in range(NCORES)]

            def emit_sem_chunk(ch):
                i, r = sem_seq[ch]
                sim_chunk(qsP, ag_nks_out[i], r, ch, cand_v_s, cand_i_s,
                          r * ML + i * 512)

            emit_merge_e(0)
            emit_merge_e(1)
            for ch in range(len(sem_seq)):
                emit_sem_chunk(ch)
                if ch < NBT:
                    transpose_into(accT_e[:, ch], acc_e[ch])
                    if ch + 2 < NBT:
                        emit_merge_e(ch + 2)

            # ===================================================================
            # Phase F: blend + output projections + streaming layernorm
            # ===================================================================
            bl_all = big.tile([P, NBT, H], BF16, tag="bl", name="bl_all")

            emit_merge_s(0)
            emit_merge_s(1)
            emit_merge_s(2)
            # Pass 2a: bl = gate0*w_out + acc_e @ W_eo (jc-major, weights read
            # once); ACT evacuates so DVE stays free for the semantic merges,
            # which run concurrently on DVE. The accT_s transposes interleave
            # between jc blocks: Ts_k lands right after merge k finishes, and
            # releasing acc_s[0] lets merge 3's ring slot allocate.
            for jc in range(H // 512):
                cs = slice(jc * 512, (jc + 1) * 512)
                wsn = s512p.tile([S, 512], BF16, tag="s512", name="wsn2")
                nc.sync.dma_start(wsn[:S, :], ws16[:, cs])
                psos = [ps_mm.tile([P, 512], F32, tag="mm", name=f"pso{i}")
                        for i in range(NBT)]
                for bt in range(NBT):
                    nc.tensor.matmul(psos[bt][:], ewT_pre[bt][:, :],
                                     wsn[:S, :], start=True, stop=False)
                for hq in range(HT // 4):
                    wt = wtp.tile([P, 4, 512], BF16, tag="wt", name="wto")
                    nc.sync.dma_start(
                        wt[:], weo16[hq * 512:(hq + 1) * 512, cs].rearrange(
                            "(q p) c -> p q c", p=P))
                    for q4 in range(4):
                        hi = hq * 4 + q4
                        for bt in range(NBT):
                            nc.tensor.matmul(
                                psos[bt][:], accT_e[:, bt, hi, :], wt[:, q4],
                                start=False, stop=(hi == HT - 1))
                for bt in range(NBT):
                    nc.scalar.activation(bl_all[:, bt, cs], psos[bt][:], AF.Copy)
                if jc < NBT:
                    transpose_into(accT_s[:, jc], acc_s[jc])
                if jc == 0:
                    emit_merge_s(3)

            # Pass 2b: bl += acc_s @ W_so (jc-major, weights read once)
            for jc in range(H // 512):
                cs = slice(jc * 512, (jc + 1) * 512)
                psob = [ps_mm.tile([P, 512], F32, tag="mm", name=f"psob{i}")
                        for i in range(NBT)]
                for hq in range(HT // 4):
                    wt = wtp.tile([P, 4, 512], BF16, tag="wt", name="wtob")
                    nc.sync.dma_start(
                        wt[:], wso16[hq * 512:(hq + 1) * 512, cs].rearrange(
                            "(q p) c -> p q c", p=P))
                    for q4 in range(4):
                        hi = hq * 4 + q4
                        for bt in range(NBT):
                            nc.tensor.matmul(
                                psob[bt][:], accT_s[:, bt, hi, :], wt[:, q4],
                                start=(hi == 0), stop=(hi == HT - 1))
                for bt in range(NBT):
                    nc.vector.tensor_add(bl_all[:, bt, cs],
                                         bl_all[:, bt, cs], psob[bt][:])

            # blT transposes (bl_all complete per bt only after P2b jc3)
            blT = big.tile([P, NBT, HT, P], BF16, tag="kT", name="blT")
            for bt in range(NBT):
                transpose_into(blT[:, bt], bl_all[:, bt, :])

            # Pass 3: xo = bl @ W_ro (jc-major) with streamed LN stats
            xo_all = big.tile([P, NBT, H], BF16, tag="xTin", name="xo_all")
            msum = [tiny.tile([P, 4], F32, tag=f"cms{i}", name=f"msum{i}",
                              bufs=1) for i in range(NBT)]
            vsum = [tiny.tile([P, 4], F32, tag=f"cvs{i}", name=f"vsum{i}",
                              bufs=1) for i in range(NBT)]
            ln_stats = [None] * NBT

            def emit_ln_stats(bt):
                """inv-std and -mu*inv-std per-partition scalars for one bt."""
                mu = tiny.tile([P, 1], F32, tag="c1", name="mu")
                nc.vector.reduce_sum(mu[:, :], msum[bt][:, :], axis=AXL.X)
                nc.vector.tensor_scalar_mul(mu[:, :], mu[:, :], 1.0 / H)
                vs = tiny.tile([P, 1], F32, tag=f"cvv{bt}", name="vs", bufs=1)
                nc.vector.reduce_sum(vs[:, :], vsum[bt][:, :], axis=AXL.X)
                nc.vector.tensor_scalar_mul(vs[:, :], vs[:, :], 1.0 / H)
                mu2 = tiny.tile([P, 1], F32, tag="c1", name="mu2")
                nc.vector.tensor_tensor(out=mu2[:, :], in0=mu[:, :],
                                        in1=mu[:, :], op=ALU.mult)
                nc.vector.tensor_tensor(out=vs[:, :], in0=vs[:, :],
                                        in1=mu2[:, :], op=ALU.subtract)
                nc.vector.tensor_scalar_add(vs[:, :], vs[:, :], LN_EPS)
                nc.scalar.sqrt(vs[:, :], vs[:, :])
                nc.vector.reciprocal(vs[:, :], vs[:, :])
                nmu = tiny.tile([P, 1], F32, tag=f"cnm{bt}", name="nmu",
                                bufs=1)
                nc.vector.tensor_tensor(out=nmu[:, :], in0=mu[:, :],
                                        in1=vs[:, :], op=ALU.mult)
                nc.vector.tensor_scalar_mul(nmu[:, :], nmu[:, :], -1.0)
                ln_stats[bt] = (vs, nmu)
            for jc in range(H // 512):
                cs = slice(jc * 512, (jc + 1) * 512)
                psro = [ps_mm.tile([P, 512], F32, tag="mm", name=f"psro{i}")
                        for i in range(NBT)]
                for hq in range(HT // 4):
                    wt = wtp.tile([P, 4, 512], BF16, tag="wt", name="wtro")
                    nc.sync.dma_start(
                        wt[:], wro16[hq * 512:(hq + 1) * 512, cs].rearrange(
                            "(q p) c -> p q c", p=P))
                    for q4 in range(4):
                        hi = hq * 4 + q4
                        for bt in range(NBT):
                            nc.tensor.matmul(
                                psro[bt][:], blT[:, bt, hi, :], wt[:, q4],
                                start=(hi == 0), stop=(hi == HT - 1))
                for bt in range(NBT):
                    nc.scalar.activation(xo_all[:, bt, cs], psro[bt][:],
                                         AF.Copy,
                                         accum_out=msum[bt][:, jc:jc + 1])
                    sqc = s512p.tile([P, 512], F32, tag="sqn", name="sqc")
                    nc.scalar.activation(sqc[:, :], psro[bt][:], AF.Square,
                                         accum_out=vsum[bt][:, jc:jc + 1])
                    if jc == H // 512 - 1:
                        emit_ln_stats(bt)

            # Pass 4: layernorm finals from streamed stats (short tail;
            # bf16 intermediates for 2x DVE throughput)
            for bt in range(NBT):
                vs, nmu = ln_stats[bt]
                for jc in range(H // 512):
                    cs = slice(jc * 512, (jc + 1) * 512)
                    on16 = s512p.tile([P, 512], BF16, tag="sqn", name="on16")
                    nc.vector.tensor_scalar(on16[:, :], xo_all[:, bt, cs],
                                            vs[:, :1], nmu[:, :1],
                                            op0=ALU.mult, op1=ALU.add)
                    nc.vector.tensor_mul(on16[:, :], on16[:, :], gbc2[jc][:, :])
                    on = s512p.tile([P, 512], F32, tag="s512", name="on")
                    nc.vector.tensor_add(on[:, :], on16[:, :], bbc2[jc][:, :])
                    nc.sync.dma_start(out_s[bt * P:(bt + 1) * P, cs], on[:])

    nc.finalize()
    return nc


_NC_CACHE = None
LAST_EXEC_NS = None


def _pack_xpair(x):
    """[R,H] f32 -> [R//512, P, 2, HT, 512] bf16 pair, pre-transposed to
    the on-chip tile layout: pk[ch, p, half, hi, r] = split(x)[half][
    ch*512+r, hi*128+p]."""
    hi_, lo_ = _split_bf16(x)
    def lay(a):
        return a.reshape(-1, HT, P).transpose(2, 1, 0)   # [P, HT, R]
    pk = np.stack([lay(hi_), lay(lo_)], axis=1)          # [P, 2, HT, R]
    R = x.shape[0]
    return np.ascontiguousarray(
        np.stack([pk[..., i * 512:(i + 1) * 512]
                  for i in range(R // 512)], axis=0))


def _pack_wpair(w):
    """[H,H] f32 -> [HT, P, 2, HT, P] bf16 pair in wcP tile layout:
    packed[j, p, half, hi, c] = split(W)[half][hi*128+p, j*128+c]."""
    hi_, lo_ = _split_bf16(w)
    def lay(a):
        # [hi, p, j, c] -> [j, p, hi, c]
        return np.ascontiguousarray(
            a.reshape(HT, P, HT, P).transpose(2, 1, 0, 3))
    return np.ascontiguousarray(
        np.stack([lay(hi_), lay(lo_)], axis=2))


def _pack_gwk(gate_W1, work_slots):
    """hstack(gate_W1 [H,64], work_slots.T [H,64]) -> [P, 2, HT, 128] pair:
    pk[p, half, hi, c] = split(gw)[half][hi*128+p, c]."""
    gw = np.hstack([np.asarray(gate_W1, np.float32),
                    np.ascontiguousarray(np.asarray(work_slots, np.float32).T)])
    hi_, lo_ = _split_bf16(gw)
    def lay(a):
        return a.reshape(HT, P, 128).transpose(1, 0, 2)   # [P, HT, 128]
    return np.ascontiguousarray(np.stack([lay(hi_), lay(lo_)], axis=1))


def _split_bf16(x):
    """two-term bf16 decomposition: x ~= hi + lo to ~16 mantissa bits."""
    import ml_dtypes
    bf = ml_dtypes.bfloat16
    x = np.ascontiguousarray(np.asarray(x), dtype=np.float32)
    hi = x.astype(bf)
    lo = (x - hi.astype(np.float32)).astype(bf)
    return hi, lo


def kernel(**inputs) -> np.ndarray:
    global _NC_CACHE
    if _NC_CACHE is None:
        _NC_CACHE = build()
    nc = _NC_CACHE

    def arr(x):
        return np.ascontiguousarray(np.asarray(x), dtype=np.float32)

    wq_pk = _pack_wpair(inputs["W_query"])
    wek_pk = _pack_wpair(inputs["W_ek"])
    wsq_pk = _pack_wpair(inputs["W_sq"])
    wsk_pk = _pack_wpair(inputs["W_sk"])
    wev16, _ = _split_bf16(inputs["W_ev"])
    weo16, _ = _split_bf16(inputs["W_eo"])
    wso16, _ = _split_bf16(inputs["W_so"])
    wro16, _ = _split_bf16(inputs["W_ro"])
    semv16, _ = _split_bf16(inputs["sem_values"])
    ws16, _ = _split_bf16(inputs["work_slots"])
    gwk_pk = _pack_gwk(inputs["gate_W1"], inputs["work_slots"])

    in_maps = []
    for c in range(NCORES):
        in_maps.append({
            "query_pk": _pack_xpair(inputs["query"][c * BL:(c + 1) * BL]),
            "ep_pk": _pack_xpair(inputs["ep_store"][c * NL:(c + 1) * NL]),
            "semk_pk": _pack_xpair(inputs["sem_keys"][c * ML:(c + 1) * ML]),
            "ep_imp_s": arr(inputs["ep_importance"][c * NL:(c + 1) * NL]),
            "ep_ts_s": arr(inputs["ep_timestamps"][c * NL:(c + 1) * NL]),
            "ep_imp": arr(inputs["ep_importance"]),
            "ep_ts": arr(inputs["ep_timestamps"]),
            "semv16": semv16,
            "wq_pk": wq_pk,
            "wek_pk": wek_pk,
            "wsq_pk": wsq_pk,
            "wsk_pk": wsk_pk,
            "wev16": wev16,
            "weo16": weo16,
            "wso16": wso16,
            "wro16": wro16,
            "ws16": ws16,
            "gwk_pk": gwk_pk,
            "gate_b1": arr(inputs["gate_b1"]),
            "gate_W2": arr(inputs["gate_W2"]),
            "gate_b2": arr(inputs["gate_b2"]),
            "ln_gamma": arr(inputs["ln_gamma"]),
            "ln_beta": arr(inputs["ln_beta"]),
        })
    res = run_bass_kernel_spmd(nc, in_maps, core_ids=list(range(NCORES)))
    return np.concatenate([res.results[c]["out_s"] for c in range(NCORES)],
                          axis=0)


# revision 57
# speedup vs baseline: 1.0371x; 1.0057x over previous
"""ONIMemoryHub kernel for 8 Trainium2 NeuronCores (Bass/Tile).

Sharding: data-parallel over batch for the query side; episodic store and
semantic memory sharded across cores for the key/value projections, with
AllGathers of the projected (normalized, pre-scaled) keys/values.

Schedule notes (v2): the PE instruction stream is kept free of stalls by
emitting off-engine work (top-k merges, norms, layernorm) interleaved
between matmul blocks whose inputs are already resident:
  - projection column norms are fused into the projection evacuation
  - gate/work blocks interleave with the episodic sim chunks
  - episodic merges/transposes interleave with the semantic sim chunks
  - semantic merges interleave with the W_eo output pass
  - W_so/W_ro passes run b-tile-major with a streaming layernorm so the
    kernel tail is one b-tile's LN instead of a full LN pass.

kernel(**inputs) takes FULL inputs (as produced by reference.setup_inputs())
and returns the FULL [4096, 2048] output.
"""
import math

import numpy as np

import concourse.bass as bass
import concourse.mybir as mybir
import concourse.tile as tile
from concourse import bacc
from concourse.bass_utils import run_bass_kernel_spmd
from concourse.masks import make_identity

AF = mybir.ActivationFunctionType
AXL = mybir.AxisListType
ALU = mybir.AluOpType

NCORES = 8
B, H, N, M, S = 4096, 2048, 4096, 16384, 64
BL, NL, ML = B // NCORES, N // NCORES, M // NCORES   # 512, 512, 2048
HT = H // 128                                        # 16 h-tiles
P = 128
NBT = BL // P                                        # 4 b-tiles
EP_K = 8
SEM_K = 4
LN_EPS = 1e-5
RECENCY = 0.01   # 1 - RECENCY_DECAY

F32 = mybir.dt.float32
BF16 = mybir.dt.bfloat16
U32 = mybir.dt.uint32


def build():
    nc = bacc.Bacc("TRN2", target_bir_lowering=False, debug=False,
                   num_devices=NCORES)

    def din(name, shape, dt=F32):
        return nc.dram_tensor(name, shape, dt, kind="ExternalInput").ap()

    # per-core slices: host-split bf16 pairs, pre-transposed to tile layout
    query_pk = din("query_pk", [1, P, 2, HT, 512], BF16)
    ep_pk = din("ep_pk", [1, P, 2, HT, 512], BF16)
    semk_pk = din("semk_pk", [ML // 512, P, 2, HT, 512], BF16)
    ep_imp_s = din("ep_imp_s", [NL])
    ep_ts_s = din("ep_ts_s", [NL])
    # replicated
    ep_imp = din("ep_imp", [N])
    ep_ts = din("ep_ts", [N])
    semv16 = din("semv16", [M, H], BF16)
    wq_pk = din("wq_pk", [HT, P, 2, HT, P], BF16)
    wek_pk = din("wek_pk", [HT, P, 2, HT, P], BF16)
    wsq_pk = din("wsq_pk", [HT, P, 2, HT, P], BF16)
    wsk_pk = din("wsk_pk", [HT, P, 2, HT, P], BF16)
    wev16 = din("wev16", [H, H], BF16)
    weo16 = din("weo16", [H, H], BF16)
    wso16 = din("wso16", [H, H], BF16)
    wro16 = din("wro16", [H, H], BF16)
    ws16 = din("ws16", [S, H], BF16)
    gwk_pk = din("gwk_pk", [P, 2, HT, 128], BF16)
    gate_b1 = din("gate_b1", [64])
    gate_W2 = din("gate_W2", [64, 3])
    gate_b2 = din("gate_b2", [3])
    ln_gamma = din("ln_gamma", [H])
    ln_beta = din("ln_beta", [H])

    out_s = nc.dram_tensor("out_s", [BL, H], BF16, kind="ExternalOutput").ap()

    with tile.TileContext(nc) as tc:
        with (
            tc.tile_pool(name="cst", bufs=1) as cst,
            tc.tile_pool(name="big", bufs=1) as big,
            tc.tile_pool(name="rows", bufs=1) as rows,
            tc.tile_pool(name="s512", bufs=2) as s512p,
            tc.tile_pool(name="wcol", bufs=2) as wcolp,
            tc.tile_pool(name="wtile", bufs=2) as wtp,
            tc.tile_pool(name="sm", bufs=2) as sm,
            tc.tile_pool(name="tiny", bufs=2) as tiny,
            tc.tile_pool(name="simb", bufs=2) as simb,
            tc.tile_pool(name="gath", bufs=2) as gath,
            tc.tile_pool(name="ps_mm", bufs=7, space="PSUM") as ps_mm,
            tc.tile_pool(name="ps_sml", bufs=1, space="PSUM") as ps_sml,
            tc.tile_pool(name="dram", bufs=1, space="DRAM") as dram,
        ):
            ident = cst.tile([P, P], F32)
            make_identity(nc, ident[:])
            ident16 = cst.tile([P, P], BF16)
            nc.scalar.activation(ident16[:], ident[:], AF.Copy)
            ones_col = cst.tile([P, 1], F32)
            nc.vector.memset(ones_col[:], 1.0)

            # ---------- helpers ----------
            # big slot chains (explicit liveness via shared tags):
            #   xTin: epT -> skT(x4) -> qsT -> accTs
            #   kT  : ekT -> ksT(x4) -> accTe -> blT
            #   bl  : qTp -> bl_all
            def emit_split(dst_hi, dst_lo, src_f32, tmp32):
                """bf16 two-term split: hi = bf16(x), lo = bf16(x - hi)."""
                nc.scalar.activation(dst_hi, src_f32, AF.Copy)
                nc.gpsimd.tensor_copy(tmp32, dst_hi)
                nc.vector.tensor_tensor(out=tmp32, in0=src_f32, in1=tmp32,
                                        op=ALU.subtract)
                nc.scalar.activation(dst_lo, tmp32, AF.Copy)

            def wcol_pair(w_pk, j):
                wcP = wcolp.tile([P, 2, HT, P], BF16, tag="wcp", name="wcp")
                nc.sync.dma_start(wcP[:], w_pk[j])
                return wcP

            def norm_row_finish(psn, extra_row=None):
                """[1,512] inv-norm row from accumulated sum-of-squares."""
                row = rows.tile([1, 512], F32, tag="nrow", name="nrow", bufs=2)
                nc.vector.tensor_copy(row[:1, :], psn[:1, :])
                nc.scalar.sqrt(row[:1, :], row[:1, :])
                nc.vector.tensor_scalar_max(row[:1, :], row[:1, :], 1e-12)
                nc.vector.reciprocal(row[:1, :], row[:1, :])
                if extra_row is not None:
                    nc.vector.tensor_mul(row[:1, :], row[:1, :], extra_row)
                return row

            def project3(xP, w_pk, name, tag, mode, mid_emit=None,
                         first_wc=None):
                """(x @ W).T via 3-term bf16 split matmuls; xP is a pair.

                mode "f32": returns (yT, psn) — f32 tile + sum-of-squares
                psum row (norm fused into the evacuation).
                mode "pair": returns (yP, psn) — bf16 pair tile + norm psum.
                mid_emit() is called after the j==3 block so a prefetch DMA
                can ride the SP queue behind the first few weight columns.
                """
                psn = ps_sml.tile([1, 512], F32, tag="sml", name="npsum")
                if mode == "f32":
                    yT = big.tile([P, HT, 512], F32, tag=tag, name=name)
                else:
                    yP = big.tile([P, 2, HT, 512], BF16, tag=tag, name=name)
                # norm accumulation runs at lag 1 so the PE never waits on the
                # ACT square of the chunk it just produced.
                sqs = [None] * HT
                for j in range(HT):
                    wcP = first_wc if (j == 0 and first_wc is not None) \
                        else wcol_pair(w_pk, j)
                    pst = ps_mm.tile([P, 512], F32, tag="mm", name="projps")
                    for hi in range(HT):
                        nc.tensor.matmul(
                            pst[:], wcP[:, 0, hi, :], xP[:, 0, hi, :],
                            start=(hi == 0), stop=False)
                        nc.tensor.matmul(
                            pst[:], wcP[:, 0, hi, :], xP[:, 1, hi, :],
                            start=False, stop=False)
                        nc.tensor.matmul(
                            pst[:], wcP[:, 1, hi, :], xP[:, 0, hi, :],
                            start=False, stop=(hi == HT - 1))
                    sq = s512p.tile([P, 512], F32, tag="sqn", name="sqn")
                    nc.scalar.square(sq[:, :], pst[:])
                    sqs[j] = sq
                    if mode == "f32":
                        nc.scalar.activation(yT[:, j, :], pst[:], AF.Copy)
                    else:
                        tmp32 = s512p.tile([P, 512], F32, tag="s512",
                                           name="spj32")
                        emit_split(yP[:, 0, j, :], yP[:, 1, j, :], pst[:],
                                   tmp32[:])
                    if j >= 1:
                        nc.tensor.matmul(psn[:1, :], ones_col[:],
                                         sqs[j - 1][:, :],
                                         start=(j == 1), stop=False)
                    if mid_emit is not None and j in (3, 6, 9, 12):
                        mid_emit((j - 3) // 3)
                nc.tensor.matmul(psn[:1, :], ones_col[:], sqs[HT - 1][:, :],
                                 start=False, stop=True)
                if mode == "f32":
                    return yT, psn
                return yP, psn

            def store_pair_to_ag(xT, ag_in):
                """split scaled f32 keys and store bf16 pair to AG input."""
                for hi in range(HT):
                    sth = s512p.tile([P, 512], BF16, tag="st16h", name="sth",
                                     bufs=4)
                    stl = s512p.tile([P, 512], BF16, tag="st16l", name="stl",
                                     bufs=4)
                    tmp32 = s512p.tile([P, 512], F32, tag="s512", name="spg32")
                    emit_split(sth[:], stl[:], xT[:, hi, :], tmp32[:])
                    # stores ride the ACT queue (which paces them via the
                    # splits), keeping the SP queue free for weight loads
                    nc.scalar.dma_start(ag_in[0, hi * P:(hi + 1) * P, :], sth[:])
                    nc.scalar.dma_start(ag_in[1, hi * P:(hi + 1) * P, :], stl[:])

            def scale_cols(xT, scale_row):
                bc = s512p.tile([P, 512], F32, tag="s512", name="bcn")
                nc.gpsimd.partition_broadcast(bc[:, :], scale_row[:1, :])
                for hi in range(HT):
                    nc.vector.tensor_mul(xT[:, hi, :], xT[:, hi, :], bc[:, :])

            # ===================================================================
            # Phase M: sharded memory-side projections + AllGathers
            # ===================================================================
            ag_nek_in = dram.tile([2, H, NL], BF16, name="ag_nek_in")
            ag_nek_out = dram.tile([NCORES, 2, H, NL], BF16,
                                   addr_space="Shared", name="ag_nek_out")
            ag_ev_in = dram.tile([NL, H], BF16, name="ag_ev_in")
            ag_ev_out = dram.tile([N, H], BF16, addr_space="Shared",
                                  name="ag_ev_out")
            ag_nks_in = [dram.tile([2, H, 512], BF16, name=f"ag_nks_in{i}")
                         for i in range(ML // 512)]
            ag_nks_out = [dram.tile([NCORES, 2, H, 512], BF16,
                                    addr_space="Shared", name=f"ag_nks_out{i}")
                          for i in range(ML // 512)]

            # semantic-key chunk loads: double-buffered on alternating big
            # slots (bl/xTin), emitted via project3 mid_emit hooks so each
            # 12.6us DMA hides under the previous projection.
            skPs = [None] * (ML // 512)
            qTin_box = [None]

            def load_sk(mc, piece):
                """quarter-piece prefetch of a semantic-key chunk."""
                if piece == 0:
                    skPs[mc] = big.tile([P, 2, HT, 512], BF16,
                                        tag="bl" if mc % 2 == 0 else "xTin",
                                        name=f"skT{mc}")
                hs = slice(piece * 4, (piece + 1) * 4)
                nc.sync.dma_start(skPs[mc][:, :, hs], semk_pk[mc, :, :, hs])

            def load_qTin(piece):
                if piece == 0:
                    qTin_box[0] = big.tile([P, 2, HT, 512], BF16, tag="bl",
                                           name="qTin")
                hs = slice(piece * 4, (piece + 1) * 4)
                nc.sync.dma_start(qTin_box[0][:, :, hs], query_pk[0, :, :, hs])

            # First weight column rides the DMA queue ahead of epP so the very
            # first matmul chain starts as soon as epP's first piece lands;
            # epP is split so early hi-tiles arrive (and compute) first.
            wc_ek0 = wcol_pair(wek_pk, 0)
            epP = big.tile([P, 2, HT, 512], BF16, tag="xTin", name="epT")
            nc.sync.dma_start(epP[:, :, :HT // 4], ep_pk[0, :, :, :HT // 4])
            nc.sync.dma_start(epP[:, :, HT // 4:HT // 2],
                              ep_pk[0, :, :, HT // 4:HT // 2])
            nc.sync.dma_start(epP[:, :, HT // 2:], ep_pk[0, :, :, HT // 2:])

            # ---- episodic recency/importance weights (off-PE, overlaps ekT)
            def rec_weight(imp_ap, ts_ap, shape, tagb):
                """(1+imp)*exp(-|1-ts|*RECENCY) elementwise; returns tile."""
                impt = rows.tile(shape, F32, tag=tagb + "i", name="impt")
                tst = rows.tile(shape, F32, tag=tagb + "t", name="tst")
                nc.sync.dma_start(impt[:shape[0], :], imp_ap)
                nc.sync.dma_start(tst[:shape[0], :], ts_ap)
                s = tst[:shape[0], :]
                nc.scalar.activation(s, s, AF.Copy, bias=0.0, scale=-1.0)
                nc.vector.tensor_scalar_add(s, s, 1.0)
                nc.scalar.activation(s, s, AF.Abs)
                nc.scalar.activation(s, s, AF.Exp, scale=-RECENCY)
                si = impt[:shape[0], :]
                nc.vector.tensor_scalar_add(si, si, 1.0)
                nc.vector.tensor_mul(si, si, s)
                return impt

            wfull = rec_weight(ep_imp.rearrange("(p c) -> p c", p=P),
                               ep_ts.rearrange("(p c) -> p c", p=P),
                               [P, N // P], "wf")
            wpart = rows.tile([P, 1], F32, tag="wpart", name="wpart")
            nc.vector.reduce_sum(wpart[:, :], wfull[:, :], axis=AXL.X)
            pssum = ps_sml.tile([1, 512], F32, tag="sml", name="wsps")
            nc.tensor.matmul(pssum[:1, :1], ones_col[:], wpart[:, :],
                             start=True, stop=True)
            wsum = rows.tile([1, 1], F32, tag="wsum", name="wsum")
            nc.vector.tensor_copy(wsum[:1, :], pssum[:1, :1])
            nc.vector.tensor_scalar_add(wsum[:1, :], wsum[:1, :], 1e-8)
            nc.vector.reciprocal(wsum[:1, :], wsum[:1, :])
            wloc = rec_weight(ep_imp_s[None, :], ep_ts_s[None, :], [1, NL], "wl")
            nc.vector.tensor_scalar(wloc[:1, :], wloc[:1, :], wsum[:1, :1], None,
                                    op0=ALU.mult)

            # ---- episodic keys: project (norm fused), scale, store, AG;
            # skT0's load rides behind the early ek weight columns.
            ekT, psn_ek = project3(epP, wek_pk, "ekT", "kT", "f32",
                                   mid_emit=lambda p: load_sk(0, p),
                                   first_wc=wc_ek0)
            # ---- e_vals natural layout [NL, H]; bf16 single term
            for jc in range(H // 512):
                psts = [ps_mm.tile([P, 512], F32, tag="mm", name=f"evps{i}")
                        for i in range(NL // P)]
                for h2 in range(HT // 2):
                    wt16 = wtp.tile([P, 2, 512], BF16, tag="wt16", name="wt16",
                                    bufs=2)
                    nc.sync.dma_start(
                        wt16[:],
                        wev16[h2 * 256:(h2 + 1) * 256,
                              jc * 512:(jc + 1) * 512].rearrange(
                                  "(q p) c -> p q c", p=P))
                    for q2 in range(2):
                        hi = h2 * 2 + q2
                        for nt in range(NL // P):
                            ns = slice(nt * P, (nt + 1) * P)
                            nc.tensor.matmul(
                                psts[nt][:], epP[:, 0, hi, ns], wt16[:, q2],
                                start=(hi == 0), stop=(hi == HT - 1))
                for nt in range(NL // P):
                    evs = s512p.tile([P, 512], BF16, tag="evo16", name="evout", bufs=1)
                    nc.vector.tensor_copy(evs[:], psts[nt][:])
                    nc.scalar.dma_start(
                        ag_ev_in[nt * P:(nt + 1) * P, jc * 512:(jc + 1) * 512],
                        evs[:])
            nc.gpsimd.collective_compute(
                "AllGather", ALU.bypass,
                replica_groups=[list(range(NCORES))],
                ins=[ag_ev_in.opt()], outs=[ag_ev_out.opt()])

            inv_ek = norm_row_finish(psn_ek, extra_row=wloc[:1, :])
            scale_cols(ekT, inv_ek)
            store_pair_to_ag(ekT, ag_nek_in)
            nc.gpsimd.collective_compute(
                "AllGather", ALU.bypass,
                replica_groups=[list(range(NCORES))],
                ins=[ag_nek_in.opt()], outs=[ag_nek_out.opt()])

            # ---- semantic keys: 4 chunks of 512 (loads via mid_emit hooks)
            for mc in range(ML // 512):
                nxt = (lambda p, m=mc + 1: load_sk(m, p)) \
                    if mc + 1 < ML // 512 else load_qTin
                ksT, psn_ks = project3(skPs[mc], wsk_pk, f"ksT{mc}", "kT",
                                       "f32", mid_emit=nxt)
                inv_ks = norm_row_finish(psn_ks)
                scale_cols(ksT, inv_ks)
                store_pair_to_ag(ksT, ag_nks_in[mc])
                nc.gpsimd.collective_compute(
                    "AllGather", ALU.bypass,
                    replica_groups=[list(range(NCORES))],
                    ins=[ag_nks_in[mc].opt()], outs=[ag_nks_out[mc].opt()])

            # ===================================================================
            # Phase Q: query-side projections (norms fused)
            # ===================================================================
            qTinP = qTin_box[0]
            qTp, psn_q = project3(qTinP, wq_pk, "qT", "kT", "pair")
            inv_q = norm_row_finish(psn_q)
            qsP, psn_qs = project3(qTp, wsq_pk, "qsT", "xTin", "pair")
            inv_qs = norm_row_finish(psn_qs)

            # transpose inv rows -> per-partition [128, NBT] via DRAM bounce
            invq_p = cst.tile([P, NBT], F32, name="invq_p")
            invqs_p = cst.tile([P, NBT], F32, name="invqs_p")
            bounce = dram.tile([2, BL], F32, name="bounce")
            nc.sync.dma_start(bounce[0:1, :], inv_q[:1, :])
            nc.sync.dma_start(bounce[1:2, :], inv_qs[:1, :])
            nc.sync.dma_start(
                invq_p[:, :], bounce[0:1, :].rearrange("o (t p) -> (o p) t", p=P))
            nc.sync.dma_start(
                invqs_p[:, :], bounce[1:2, :].rearrange("o (t p) -> (o p) t", p=P))

            def bcast_row(dram_row, width, pool, tag, name, dt=F32):
                row = rows.tile([1, width], F32, tag="crow", name="crow", bufs=1)
                nc.sync.dma_start(row[:1, :], dram_row)
                src = row[:1, :]
                if dt != F32:
                    row16 = rows.tile([1, width], dt, tag="crow16",
                                      name="crow16", bufs=1)
                    nc.scalar.activation(row16[:1, :], row[:1, :], AF.Copy)
                    src = row16[:1, :]
                t = pool.tile([P, width], dt, tag=tag, name=name, bufs=1)
                nc.gpsimd.partition_broadcast(t[:, :], src)
                return t

            b1bc = bcast_row(gate_b1[None, :], 64, cst, "b1bc", "b1bc")
            b2bc = bcast_row(gate_b2[None, :], 3, cst, "b2bc", "b2bc")
            # gate+work concatenated projection weights (bf16 pair); rides the
            # wcol ring slot freed after the last qs weight column.
            gwk = wcolp.tile([P, 2, HT, 128], BF16, tag="wcp", name="gwk")
            nc.sync.dma_start(gwk[:], gwk_pk)
            gw2 = cst.tile([64, 3], F32, name="gw2")
            nc.sync.dma_start(gw2[:, :], gate_W2)

            inv_sqrt_h = 1.0 / math.sqrt(H)
            ewT_pre = [None] * NBT
            gw_pre = [None] * NBT

            gate_st = [None] * NBT

            def emit_gate_a(bt):
                """Gate/work stage A: fused matmul (cols 0:64 gate hidden,
                64:128 work logits; stationary q bf16-hi, moving bf16 pair of
                hstack(gate_W1, work_slots.T)) + the off-PE softmax chain."""
                psg = ps_sml.tile([P, 128], F32, tag="sml", name="psg")
                bs = slice(bt * P, (bt + 1) * P)
                for hi in range(HT):
                    nc.tensor.matmul(
                        psg[:, :], qTp[:, 0, hi, bs], gwk[:, 0, hi, :],
                        start=(hi == 0), stop=False)
                    nc.tensor.matmul(
                        psg[:, :], qTp[:, 0, hi, bs], gwk[:, 1, hi, :],
                        start=False, stop=(hi == HT - 1))
                hid = tiny.tile([P, 64], F32, tag="c64", name="hid")
                nc.vector.tensor_add(hid[:, :], psg[:, :64], b1bc[:, :])
                nc.scalar.activation(hid[:, :], hid[:, :], AF.Silu)
                wmax = tiny.tile([P, 1], F32, tag="c1", name="wmax")
                nc.vector.reduce_max(wmax[:, :], psg[:, 64:], axis=AXL.X)
                nc.vector.tensor_scalar_mul(wmax[:, :], wmax[:, :], -inv_sqrt_h)
                ew = tiny.tile([P, S], F32, tag="cew", name="ew")
                nc.scalar.activation(ew[:, :], psg[:, 64:], AF.Exp,
                                     bias=wmax[:, :1], scale=inv_sqrt_h)
                zw = tiny.tile([P, 1], F32, tag="czw", name="zw")
                nc.vector.reduce_sum(zw[:, :], ew[:, :], axis=AXL.X)
                nc.vector.reciprocal(zw[:, :], zw[:, :])
                gate_st[bt] = (hid, ew, zw)

            def emit_gate_b(bt):
                """Gate/work stage B: transposes + gate MLP tail; its PE ops
                depend only on stage-A results finished a sim chunk ago."""
                hid, ew, zw = gate_st[bt]
                psht = ps_sml.tile([64, P], F32, tag="sml", name="hidtp")
                nc.tensor.transpose(out=psht[:64, :], in_=hid[:, :],
                                    identity=ident[:])
                hidT = tiny.tile([64, P], F32, tag="c128", name="hidT")
                nc.vector.tensor_copy(hidT[:, :], psht[:64, :])
                psg2 = ps_sml.tile([P, 3], F32, tag="sml", name="psg2")
                nc.tensor.matmul(psg2[:, :3], hidT[:, :], gw2[:, :],
                                 start=True, stop=True)
                gl = cst.tile([P, 3], F32, name=f"gl{bt}")
                nc.vector.tensor_add(gl[:, :], psg2[:, :3], b2bc[:, :])
                gmax = tiny.tile([P, 1], F32, tag="c1", name="gmax")
                nc.vector.reduce_max(gmax[:, :], gl[:, :], axis=AXL.X)
                nc.vector.tensor_scalar_mul(gmax[:, :], gmax[:, :], -1.0)
                nc.scalar.activation(gl[:, :], gl[:, :], AF.Exp, bias=gmax[:, :1])
                gz = tiny.tile([P, 1], F32, tag="c1", name="gz")
                nc.vector.reduce_sum(gz[:, :], gl[:, :], axis=AXL.X)
                nc.vector.reciprocal(gz[:, :], gz[:, :])
                nc.vector.tensor_scalar(gl[:, :], gl[:, :], gz[:, :1], None,
                                        op0=ALU.mult)
                gw_pre[bt] = gl
                # fold softmax normalization AND gate weight 0 into ew
                nc.vector.tensor_tensor(out=zw[:, :], in0=zw[:, :],
                                        in1=gl[:, 0:1], op=ALU.mult)
                nc.vector.tensor_scalar(ew[:, :], ew[:, :], zw[:, :1], None,
                                        op0=ALU.mult)

            def emit_gate_c(bt):
                """Gate/work stage C: transpose of the folded work probs."""
                _, ew, _ = gate_st[bt]
                pset = ps_sml.tile([S, P], F32, tag="sml", name="ewtp")
                nc.tensor.transpose(out=pset[:S, :], in_=ew[:, :],
                                    identity=ident[:])
                ewT = cst.tile([S, P], BF16, name=f"ewT{bt}")
                nc.vector.tensor_copy(ewT[:, :], pset[:S, :])
                ewT_pre[bt] = ewT

            # ===================================================================
            # Phase S: similarity + per-chunk top-8 candidates
            # ===================================================================
            cand_v_e = [big.tile([P, (N // 512) * 8], F32, tag=f"cve{bt}",
                                 name=f"cve{bt}") for bt in range(NBT)]
            cand_i_e = [big.tile([P, (N // 512) * 8], F32, tag=f"cie{bt}",
                                 name=f"cie{bt}") for bt in range(NBT)]
            cand_v_s = [big.tile([P, (M // 512) * 8], F32, tag=f"cvs{bt}",
                                 name=f"cvs{bt}") for bt in range(NBT)]
            cand_i_s = [big.tile([P, (M // 512) * 8], F32, tag=f"cis{bt}",
                                 name=f"cis{bt}") for bt in range(NBT)]

            def sim_chunk(xP, kd, r, ch, cand_v, cand_i, base):
                """sims of all 4 b-tiles vs bf16-pair keys kd[r, :, h, :]."""
                psts = [ps_mm.tile([P, 512], F32, tag="mm", name=f"simps{i}")
                        for i in range(NBT)]
                for hi in range(HT):
                    kth = s512p.tile([P, 512], BF16, tag="st16h", name="kth",
                                     bufs=4)
                    ktl = s512p.tile([P, 512], BF16, tag="st16l", name="ktl",
                                     bufs=4)
                    nc.sync.dma_start(
                        kth[:], kd[r, 0, hi * P:(hi + 1) * P, :])
                    nc.sync.dma_start(
                        ktl[:], kd[r, 1, hi * P:(hi + 1) * P, :])
                    for bt in range(NBT):
                        bs = slice(bt * P, (bt + 1) * P)
                        nc.tensor.matmul(
                            psts[bt][:], xP[:, 0, hi, bs], kth[:],
                            start=(hi == 0), stop=False)
                        nc.tensor.matmul(
                            psts[bt][:], xP[:, 0, hi, bs], ktl[:],
                            start=False, stop=False)
                        nc.tensor.matmul(
                            psts[bt][:], xP[:, 1, hi, bs], kth[:],
                            start=False, stop=(hi == HT - 1))
                for bt in range(NBT):
                    sc = simb.tile([P, 512], F32, tag="simc", name="simc",
                                   bufs=2)
                    nc.scalar.activation(sc[:], psts[bt][:], AF.Copy)
                    mx = simb.tile([P, 8], F32, tag="mx", name="mx")
                    mi = simb.tile([P, 8], U32, tag="mi", name="mi")
                    nc.vector.max(out=mx[:], in_=sc[:])
                    nc.vector.max_index(out=mi[:], in_max=mx[:], in_values=sc[:])
                    nc.vector.tensor_copy(cand_v[bt][:, ch * 8:(ch + 1) * 8],
                                          mx[:])
                    mif = simb.tile([P, 8], F32, tag="mif", name="mif")
                    nc.vector.tensor_copy(mif[:], mi[:])
                    nc.vector.tensor_scalar_add(
                        cand_i[bt][:, ch * 8:(ch + 1) * 8], mif[:],
                        float(base))

            def topk_attend(cand_v, cand_i, k, inv_p, bt, vals_dram, gscale,
                            acc_tag, bufs=2):
                """Merged top-k -> softmax (x gscale) -> gather + weighted sum."""
                top8 = tiny.tile([P, 8], F32, tag="c8", name="top8")
                nc.vector.max(out=top8[:], in_=cand_v[:])
                idxf = tiny.tile([P, 8], F32, tag="c8", name="idxf")
                eqm = s512p.tile([P, 256], F32, tag="sqn", name="eqm")
                for kk in range(k):
                    w = cand_v.shape[-1]
                    nc.vector.tensor_scalar(
                        eqm[:, :w], cand_v[:], top8[:, kk:kk + 1], None,
                        op0=ALU.is_equal)
                    nc.vector.tensor_tensor(
                        out=eqm[:, :w], in0=eqm[:, :w], in1=cand_i[:], op=ALU.mult)
                    nc.vector.reduce_sum(idxf[:, kk:kk + 1], eqm[:, :w], axis=AXL.X)
                idxu = tiny.tile([P, 8], U32, tag="c8u", name="idxu")
                nc.vector.tensor_copy(idxu[:, :k], idxf[:, :k])
                sc8 = tiny.tile([P, 8], F32, tag="c8", name="sc8")
                nc.vector.tensor_scalar(
                    sc8[:, :k], top8[:, :k], inv_p[:, bt:bt + 1], None,
                    op0=ALU.mult)
                negm = tiny.tile([P, 1], F32, tag="c1", name="negm")
                nc.vector.tensor_scalar_mul(negm[:, :], sc8[:, 0:1], -1.0)
                nc.scalar.activation(sc8[:, :k], sc8[:, :k], AF.Exp,
                                     bias=negm[:, :1])
                zs = tiny.tile([P, 1], F32, tag="c1", name="zs")
                nc.vector.reduce_sum(zs[:, :], sc8[:, :k], axis=AXL.X)
                nc.vector.reciprocal(zs[:, :], zs[:, :])
                nc.vector.tensor_scalar(zs[:, :], zs[:, :], gscale, None,
                                        op0=ALU.mult)
                nc.vector.tensor_scalar(sc8[:, :k], sc8[:, :k], zs[:, :1], None,
                                        op0=ALU.mult)
                acc = sm.tile([P, H], BF16, tag=acc_tag, name="acc" + acc_tag,
                              bufs=bufs)
                nc.vector.memset(acc[:, :], 0.0)
                for kk in range(k):
                    g = gath.tile([P, H], BF16, tag="g", name="g")
                    nc.gpsimd.indirect_dma_start(
                        out=g[:, :], out_offset=None, in_=vals_dram,
                        in_offset=bass.IndirectOffsetOnAxis(
                            ap=idxu[:, kk:kk + 1], axis=0))
                    nc.vector.scalar_tensor_tensor(
                        out=acc[:, :], in0=g[:, :], scalar=sc8[:, kk:kk + 1],
                        in1=acc[:, :], op0=ALU.mult, op1=ALU.add)
                return acc

            def transpose_into(dst, src, dt=BF16):
                """dst [P, HT, P] view <- transpose of src [P, H]."""
                idn = ident if dt == F32 else ident16
                for hi in range(HT):
                    pst = ps_mm.tile([P, P], dt, tag="mm", name="trf")
                    nc.tensor.transpose(out=pst[:], in_=src[:, hi * P:(hi + 1) * P],
                                        identity=idn[:])
                    nc.scalar.activation(dst[:, hi, :], pst[:], AF.Copy)

            # episodic sims: one gathered buffer, rank-major global indices;
            # gate/work stages (off-PE-latency-heavy) interleave with chunks
            # so each stage's PE ops only see dependencies already finished.
            gbc2 = [None] * (H // 512)
            bbc2 = [None] * (H // 512)
            for ch in range(N // 512):
                if 2 <= ch <= NBT + 1:
                    emit_gate_c(ch - 2)
                if 1 <= ch <= NBT:
                    emit_gate_b(ch - 1)
                if ch < NBT:
                    emit_gate_a(ch)
                if 4 <= ch < 4 + H // 512:
                    # LN gamma/beta broadcast tiles: the serial DMA<->POOL
                    # ping-pong hides under the remaining sim chunks.
                    jc = ch - 4
                    cs = slice(jc * 512, (jc + 1) * 512)
                    gbc2[jc] = bcast_row(ln_gamma[None, cs], 512, sm,
                                         f"gbc{jc}", f"gbc{jc}", BF16)
                    bbc2[jc] = bcast_row(ln_beta[None, cs], 512, sm,
                                         f"bbc{jc}", f"bbc{jc}", BF16)
                sim_chunk(qTp, ag_nek_out, ch, ch, cand_v_e, cand_i_e,
                          ch * 512)

            # episodic merges (DVE/gathers) overlap semantic sims (PE); the
            # accT_e transposes are emitted after a sem chunk each so the PE
            # queue never waits on a merge.
            accT_e = big.tile([P, NBT, HT, P], BF16, tag="kT", name="accTe")
            accT_s = big.tile([P, NBT, HT, P], BF16, tag="xTin", name="accTs")
            acc_e = [None] * NBT
            acc_s = [None] * NBT

            def emit_merge_e(bt):
                acc_e[bt] = topk_attend(cand_v_e[bt][:], cand_i_e[bt][:], EP_K,
                                        invq_p, bt, ag_ev_out[:, :],
                                        gw_pre[bt][:, 1:2], "sl1")

            def emit_merge_s(bt):
                acc_s[bt] = topk_attend(cand_v_s[bt][:], cand_i_s[bt][:], SEM_K,
                                        invqs_p, bt, semv16, gw_pre[bt][:, 2:3],
                                        "sl2", bufs=3)

            sem_seq = [(i, r) for i in range(ML // 512) for r in range(NCORES)]

            def emit_sem_chunk(ch):
                i, r = sem_seq[ch]
                sim_chunk(qsP, ag_nks_out[i], r, ch, cand_v_s, cand_i_s,
                          r * ML + i * 512)

            emit_merge_e(0)
            emit_merge_e(1)
            p2a_pre = {}
            for ch in range(len(sem_seq)):
                if ch == len(sem_seq) - 2:
                    # prefetch Pass 2a's first moving tiles so its opening
                    # matmuls don't wait on the DMA queue draining
                    wsn0 = s512p.tile([S, 512], BF16, tag="s512", name="wsn2")
                    nc.sync.dma_start(wsn0[:S, :], ws16[:, :512])
                    wt0 = wtp.tile([P, 4, 512], BF16, tag="wt", name="wto")
                    nc.sync.dma_start(
                        wt0[:], weo16[:512, :512].rearrange(
                            "(q p) c -> p q c", p=P))
                    p2a_pre["wsn"] = wsn0
                    p2a_pre["wt"] = wt0
                emit_sem_chunk(ch)
                if ch < NBT:
                    transpose_into(accT_e[:, ch], acc_e[ch])
                    if ch + 2 < NBT:
                        emit_merge_e(ch + 2)

            # ===================================================================
            # Phase F: blend + output projections + streaming layernorm
            # ===================================================================
            bl_all = big.tile([P, NBT, H], BF16, tag="bl", name="bl_all")

            emit_merge_s(0)
            emit_merge_s(1)
            emit_merge_s(2)
            # Pass 2a: bl = gate0*w_out + acc_e @ W_eo (jc-major, weights read
            # once); ACT evacuates so DVE stays free for the semantic merges,
            # which run concurrently on DVE. The accT_s transposes interleave
            # between jc blocks: Ts_k lands right after merge k finishes, and
            # releasing acc_s[0] lets merge 3's ring slot allocate.
            for jc in range(H // 512):
                cs = slice(jc * 512, (jc + 1) * 512)
                if jc == 0:
                    wsn = p2a_pre["wsn"]
                else:
                    wsn = s512p.tile([S, 512], BF16, tag="s512", name="wsn2")
                    nc.sync.dma_start(wsn[:S, :], ws16[:, cs])
                psos = [ps_mm.tile([P, 512], F32, tag="mm", name=f"pso{i}")
                        for i in range(NBT)]
                for bt in range(NBT):
                    nc.tensor.matmul(psos[bt][:], ewT_pre[bt][:, :],
                                     wsn[:S, :], start=True, stop=False)
                for hq in range(HT // 4):
                    if jc == 0 and hq == 0:
                        wt = p2a_pre["wt"]
                    else:
                        wt = wtp.tile([P, 4, 512], BF16, tag="wt", name="wto")
                        nc.sync.dma_start(
                            wt[:], weo16[hq * 512:(hq + 1) * 512, cs].rearrange(
                                "(q p) c -> p q c", p=P))
                    for q4 in range(4):
                        hi = hq * 4 + q4
                        for bt in range(NBT):
                            nc.tensor.matmul(
                                psos[bt][:], accT_e[:, bt, hi, :], wt[:, q4],
                                start=False, stop=(hi == HT - 1))
                for bt in range(NBT):
                    nc.scalar.activation(bl_all[:, bt, cs], psos[bt][:], AF.Copy)
                if 1 <= jc:
                    transpose_into(accT_s[:, jc - 1], acc_s[jc - 1])
                    if jc == H // 512 - 1:
                        transpose_into(accT_s[:, jc], acc_s[jc])
                if jc == 0:
                    emit_merge_s(3)

            # Pass 2b: bl += acc_s @ W_so (jc-major, weights read once)
            for jc in range(H // 512):
                cs = slice(jc * 512, (jc + 1) * 512)
                psob = [ps_mm.tile([P, 512], F32, tag="mm", name=f"psob{i}")
                        for i in range(NBT)]
                for hq in range(HT // 4):
                    wt = wtp.tile([P, 4, 512], BF16, tag="wt", name="wtob")
                    nc.sync.dma_start(
                        wt[:], wso16[hq * 512:(hq + 1) * 512, cs].rearrange(
                            "(q p) c -> p q c", p=P))
                    for q4 in range(4):
                        hi = hq * 4 + q4
                        for bt in range(NBT):
                            nc.tensor.matmul(
                                psob[bt][:], accT_s[:, bt, hi, :], wt[:, q4],
                                start=(hi == 0), stop=(hi == HT - 1))
                for bt in range(NBT):
                    nc.vector.tensor_add(bl_all[:, bt, cs],
                                         bl_all[:, bt, cs], psob[bt][:])

            # blT transposes (bl_all complete per bt only after P2b jc3)
            blT = big.tile([P, NBT, HT, P], BF16, tag="kT", name="blT")
            for bt in range(NBT):
                transpose_into(blT[:, bt], bl_all[:, bt, :])

            # Pass 3: xo = bl @ W_ro (jc-major) with streamed LN stats
            xo_all = big.tile([P, NBT, H], BF16, tag="xTin", name="xo_all")
            msum = [tiny.tile([P, 4], F32, tag=f"cms{i}", name=f"msum{i}",
                              bufs=1) for i in range(NBT)]
            vsum = [tiny.tile([P, 4], F32, tag=f"cvs{i}", name=f"vsum{i}",
                              bufs=1) for i in range(NBT)]
            ln_stats = [None] * NBT

            def emit_ln_stats(bt):
                """inv-std and -mu*inv-std per-partition scalars for one bt."""
                mu = tiny.tile([P, 1], F32, tag="c1", name="mu")
                nc.vector.reduce_sum(mu[:, :], msum[bt][:, :], axis=AXL.X)
                nc.vector.tensor_scalar_mul(mu[:, :], mu[:, :], 1.0 / H)
                vs = tiny.tile([P, 1], F32, tag=f"cvv{bt}", name="vs", bufs=1)
                nc.vector.reduce_sum(vs[:, :], vsum[bt][:, :], axis=AXL.X)
                nc.vector.tensor_scalar_mul(vs[:, :], vs[:, :], 1.0 / H)
                mu2 = tiny.tile([P, 1], F32, tag="c1", name="mu2")
                nc.vector.tensor_tensor(out=mu2[:, :], in0=mu[:, :],
                                        in1=mu[:, :], op=ALU.mult)
                nc.vector.tensor_tensor(out=vs[:, :], in0=vs[:, :],
                                        in1=mu2[:, :], op=ALU.subtract)
                nc.vector.tensor_scalar_add(vs[:, :], vs[:, :], LN_EPS)
                nc.scalar.sqrt(vs[:, :], vs[:, :])
                nc.vector.reciprocal(vs[:, :], vs[:, :])
                nmu = tiny.tile([P, 1], F32, tag=f"cnm{bt}", name="nmu",
                                bufs=1)
                nc.vector.tensor_tensor(out=nmu[:, :], in0=mu[:, :],
                                        in1=vs[:, :], op=ALU.mult)
                nc.vector.tensor_scalar_mul(nmu[:, :], nmu[:, :], -1.0)
                ln_stats[bt] = (vs, nmu)
            def emit_ln_final(bt):
                """Normalize + affine + store for one b-tile (bf16
                intermediates for 2x DVE throughput)."""
                vs, nmu = ln_stats[bt]
                for jc in range(H // 512):
                    cs = slice(jc * 512, (jc + 1) * 512)
                    on16 = s512p.tile([P, 512], BF16, tag="sqn", name="on16")
                    nc.vector.tensor_scalar(on16[:, :], xo_all[:, bt, cs],
                                            vs[:, :1], nmu[:, :1],
                                            op0=ALU.mult, op1=ALU.add)
                    nc.vector.tensor_mul(on16[:, :], on16[:, :], gbc2[jc][:, :])
                    on = s512p.tile([P, 512], BF16, tag="s512", name="on")
                    nc.vector.tensor_add(on[:, :], on16[:, :], bbc2[jc][:, :])
                    nc.sync.dma_start(out_s[bt * P:(bt + 1) * P, cs], on[:])

            for jc in range(H // 512):
                cs = slice(jc * 512, (jc + 1) * 512)
                psro = [ps_mm.tile([P, 512], F32, tag="mm", name=f"psro{i}")
                        for i in range(NBT)]
                for hq in range(HT // 4):
                    wt = wtp.tile([P, 4, 512], BF16, tag="wt", name="wtro")
                    nc.sync.dma_start(
                        wt[:], wro16[hq * 512:(hq + 1) * 512, cs].rearrange(
                            "(q p) c -> p q c", p=P))
                    for q4 in range(4):
                        hi = hq * 4 + q4
                        for bt in range(NBT):
                            nc.tensor.matmul(
                                psro[bt][:], blT[:, bt, hi, :], wt[:, q4],
                                start=(hi == 0), stop=(hi == HT - 1))
                for bt in range(NBT):
                    nc.scalar.activation(xo_all[:, bt, cs], psro[bt][:],
                                         AF.Copy,
                                         accum_out=msum[bt][:, jc:jc + 1])
                    sqc = s512p.tile([P, 512], F32, tag="sqn", name="sqc")
                    nc.scalar.activation(sqc[:, :], psro[bt][:], AF.Square,
                                         accum_out=vsum[bt][:, jc:jc + 1])
                    if jc == H // 512 - 1:
                        emit_ln_stats(bt)
            for bt in range(NBT):
                emit_ln_final(bt)

    nc.finalize()
    return nc


_NC_CACHE = None
LAST_EXEC_NS = None


def _pack_xpair(x):
    """[R,H] f32 -> [R//512, P, 2, HT, 512] bf16 pair, pre-transposed to
    the on-chip tile layout: pk[ch, p, half, hi, r] = split(x)[half][
    ch*512+r, hi*128+p]."""
    hi_, lo_ = _split_bf16(x)
    def lay(a):
        return a.reshape(-1, HT, P).transpose(2, 1, 0)   # [P, HT, R]
    pk = np.stack([lay(hi_), lay(lo_)], axis=1)          # [P, 2, HT, R]
    R = x.shape[0]
    return np.ascontiguousarray(
        np.stack([pk[..., i * 512:(i + 1) * 512]
                  for i in range(R // 512)], axis=0))


def _pack_wpair(w):
    """[H,H] f32 -> [HT, P, 2, HT, P] bf16 pair in wcP tile layout:
    packed[j, p, half, hi, c] = split(W)[half][hi*128+p, j*128+c]."""
    hi_, lo_ = _split_bf16(w)
    def lay(a):
        # [hi, p, j, c] -> [j, p, hi, c]
        return np.ascontiguousarray(
            a.reshape(HT, P, HT, P).transpose(2, 1, 0, 3))
    return np.ascontiguousarray(
        np.stack([lay(hi_), lay(lo_)], axis=2))


def _pack_gwk(gate_W1, work_slots):
    """hstack(gate_W1 [H,64], work_slots.T [H,64]) -> [P, 2, HT, 128] pair:
    pk[p, half, hi, c] = split(gw)[half][hi*128+p, c]."""
    gw = np.hstack([np.asarray(gate_W1, np.float32),
                    np.ascontiguousarray(np.asarray(work_slots, np.float32).T)])
    hi_, lo_ = _split_bf16(gw)
    def lay(a):
        return a.reshape(HT, P, 128).transpose(1, 0, 2)   # [P, HT, 128]
    return np.ascontiguousarray(np.stack([lay(hi_), lay(lo_)], axis=1))


def _split_bf16(x):
    """two-term bf16 decomposition: x ~= hi + lo to ~16 mantissa bits."""
    import ml_dtypes
    bf = ml_dtypes.bfloat16
    x = np.ascontiguousarray(np.asarray(x), dtype=np.float32)
    hi = x.astype(bf)
    lo = (x - hi.astype(np.float32)).astype(bf)
    return hi, lo


def kernel(**inputs) -> np.ndarray:
    global _NC_CACHE
    if _NC_CACHE is None:
        _NC_CACHE = build()
    nc = _NC_CACHE

    def arr(x):
        return np.ascontiguousarray(np.asarray(x), dtype=np.float32)

    wq_pk = _pack_wpair(inputs["W_query"])
    wek_pk = _pack_wpair(inputs["W_ek"])
    wsq_pk = _pack_wpair(inputs["W_sq"])
    wsk_pk = _pack_wpair(inputs["W_sk"])
    wev16, _ = _split_bf16(inputs["W_ev"])
    weo16, _ = _split_bf16(inputs["W_eo"])
    wso16, _ = _split_bf16(inputs["W_so"])
    wro16, _ = _split_bf16(inputs["W_ro"])
    semv16, _ = _split_bf16(inputs["sem_values"])
    ws16, _ = _split_bf16(inputs["work_slots"])
    gwk_pk = _pack_gwk(inputs["gate_W1"], inputs["work_slots"])

    in_maps = []
    for c in range(NCORES):
        in_maps.append({
            "query_pk": _pack_xpair(inputs["query"][c * BL:(c + 1) * BL]),
            "ep_pk": _pack_xpair(inputs["ep_store"][c * NL:(c + 1) * NL]),
            "semk_pk": _pack_xpair(inputs["sem_keys"][c * ML:(c + 1) * ML]),
            "ep_imp_s": arr(inputs["ep_importance"][c * NL:(c + 1) * NL]),
            "ep_ts_s": arr(inputs["ep_timestamps"][c * NL:(c + 1) * NL]),
            "ep_imp": arr(inputs["ep_importance"]),
            "ep_ts": arr(inputs["ep_timestamps"]),
            "semv16": semv16,
            "wq_pk": wq_pk,
            "wek_pk": wek_pk,
            "wsq_pk": wsq_pk,
            "wsk_pk": wsk_pk,
            "wev16": wev16,
            "weo16": weo16,
            "wso16": wso16,
            "wro16": wro16,
            "ws16": ws16,
            "gwk_pk": gwk_pk,
            "gate_b1": arr(inputs["gate_b1"]),
            "gate_W2": arr(inputs["gate_W2"]),
            "gate_b2": arr(inputs["gate_b2"]),
            "ln_gamma": arr(inputs["ln_gamma"]),
            "ln_beta": arr(inputs["ln_beta"]),
        })
    res = run_bass_kernel_spmd(nc, in_maps, core_ids=list(range(NCORES)))
    return np.concatenate(
        [np.asarray(res.results[c]["out_s"], dtype=np.float32)
         for c in range(NCORES)], axis=0)


# revision 65
# speedup vs baseline: 1.0409x; 1.0037x over previous
"""ONIMemoryHub kernel for 8 Trainium2 NeuronCores (Bass/Tile).

Sharding: data-parallel over batch for the query side; episodic store and
semantic memory sharded across cores for the key/value projections, with
AllGathers of the projected (normalized, pre-scaled) keys/values.

Schedule notes (v2): the PE instruction stream is kept free of stalls by
emitting off-engine work (top-k merges, norms, layernorm) interleaved
between matmul blocks whose inputs are already resident:
  - projection column norms are fused into the projection evacuation
  - gate/work blocks interleave with the episodic sim chunks
  - episodic merges/transposes interleave with the semantic sim chunks
  - semantic merges interleave with the W_eo output pass
  - W_so/W_ro passes run b-tile-major with a streaming layernorm so the
    kernel tail is one b-tile's LN instead of a full LN pass.

kernel(**inputs) takes FULL inputs (as produced by reference.setup_inputs())
and returns the FULL [4096, 2048] output.
"""
import math

import numpy as np

import concourse.bass as bass
import concourse.mybir as mybir
import concourse.tile as tile
from concourse import bacc
from concourse.bass_utils import run_bass_kernel_spmd
from concourse.masks import make_identity

AF = mybir.ActivationFunctionType
AXL = mybir.AxisListType
ALU = mybir.AluOpType

NCORES = 8
B, H, N, M, S = 4096, 2048, 4096, 16384, 64
BL, NL, ML = B // NCORES, N // NCORES, M // NCORES   # 512, 512, 2048
HT = H // 128                                        # 16 h-tiles
P = 128
NBT = BL // P                                        # 4 b-tiles
EP_K = 8
SEM_K = 4
LN_EPS = 1e-5
RECENCY = 0.01   # 1 - RECENCY_DECAY

F32 = mybir.dt.float32
BF16 = mybir.dt.bfloat16
U32 = mybir.dt.uint32


def build():
    nc = bacc.Bacc("TRN2", target_bir_lowering=False, debug=False,
                   num_devices=NCORES)

    def din(name, shape, dt=F32):
        return nc.dram_tensor(name, shape, dt, kind="ExternalInput").ap()

    # per-core slices: host-split bf16 pairs, pre-transposed to tile layout
    query_pk = din("query_pk", [1, P, 2, HT, 512], BF16)
    ep_pk = din("ep_pk", [1, P, 2, HT, 512], BF16)
    semk_pk = din("semk_pk", [ML // 512, P, 2, HT, 512], BF16)
    ep_imp_s = din("ep_imp_s", [NL])
    ep_ts_s = din("ep_ts_s", [NL])
    # replicated
    ep_imp = din("ep_imp", [N])
    ep_ts = din("ep_ts", [N])
    semv16 = din("semv16", [M, H], BF16)
    wq_pk = din("wq_pk", [HT, P, 2, HT, P], BF16)
    wek_pk = din("wek_pk", [HT, P, 2, HT, P], BF16)
    wsq_pk = din("wsq_pk", [HT, P, 2, HT, P], BF16)
    wsk_pk = din("wsk_pk", [HT, P, 2, HT, P], BF16)
    wev16 = din("wev16", [H, H], BF16)
    weo16 = din("weo16", [H, H], BF16)
    wso16 = din("wso16", [H, H], BF16)
    wro16 = din("wro16", [H, H], BF16)
    ws16 = din("ws16", [S, H], BF16)
    gwk_pk = din("gwk_pk", [P, 2, HT, 128], BF16)
    gate_b1 = din("gate_b1", [64])
    gate_W2 = din("gate_W2", [64, 3])
    gate_b2 = din("gate_b2", [3])
    ln_gamma = din("ln_gamma", [H])
    ln_beta = din("ln_beta", [H])

    out_s = nc.dram_tensor("out_s", [BL, H], BF16, kind="ExternalOutput").ap()

    with tile.TileContext(nc) as tc:
        with (
            tc.tile_pool(name="cst", bufs=1) as cst,
            tc.tile_pool(name="big", bufs=1) as big,
            tc.tile_pool(name="rows", bufs=1) as rows,
            tc.tile_pool(name="s512", bufs=2) as s512p,
            tc.tile_pool(name="wcol", bufs=2) as wcolp,
            tc.tile_pool(name="wtile", bufs=2) as wtp,
            tc.tile_pool(name="sm", bufs=2) as sm,
            tc.tile_pool(name="tiny", bufs=2) as tiny,
            tc.tile_pool(name="simb", bufs=2) as simb,
            tc.tile_pool(name="gath", bufs=2) as gath,
            tc.tile_pool(name="ps_mm", bufs=7, space="PSUM") as ps_mm,
            tc.tile_pool(name="ps_sml", bufs=1, space="PSUM") as ps_sml,
            tc.tile_pool(name="dram", bufs=1, space="DRAM") as dram,
        ):
            ident = cst.tile([P, P], F32)
            make_identity(nc, ident[:])
            ident16 = cst.tile([P, P], BF16)
            nc.scalar.activation(ident16[:], ident[:], AF.Copy)
            ones_col = cst.tile([P, 1], F32)
            nc.vector.memset(ones_col[:], 1.0)

            # ---------- helpers ----------
            # big slot chains (explicit liveness via shared tags):
            #   xTin: epT -> skT(x4) -> qsT -> accTs
            #   kT  : ekT -> ksT(x4) -> accTe -> blT
            #   bl  : qTp -> bl_all
            def emit_split(dst_hi, dst_lo, src_f32, tmp32):
                """bf16 two-term split: hi = bf16(x), lo = bf16(x - hi).

                The upconvert copy runs on DVE, not gpsimd: the Pool queue
                carries the collectives, which would head-block a gpsimd
                copy (and everything after it) for a whole AllGather.
                """
                nc.scalar.activation(dst_hi, src_f32, AF.Copy)
                nc.vector.tensor_copy(tmp32, dst_hi)
                nc.vector.tensor_tensor(out=tmp32, in0=src_f32, in1=tmp32,
                                        op=ALU.subtract)
                nc.scalar.activation(dst_lo, tmp32, AF.Copy)

            def wcol_pair(w_pk, j):
                wcP = wcolp.tile([P, 2, HT, P], BF16, tag="wcp", name="wcp")
                nc.sync.dma_start(wcP[:], w_pk[j])
                return wcP

            def norm_row_finish(psn, extra_row=None):
                """[1,512] inv-norm row from accumulated sum-of-squares."""
                row = rows.tile([1, 512], F32, tag="nrow", name="nrow", bufs=2)
                nc.vector.tensor_copy(row[:1, :], psn[:1, :])
                nc.scalar.sqrt(row[:1, :], row[:1, :])
                nc.vector.tensor_scalar_max(row[:1, :], row[:1, :], 1e-12)
                nc.vector.reciprocal(row[:1, :], row[:1, :])
                if extra_row is not None:
                    nc.vector.tensor_mul(row[:1, :], row[:1, :], extra_row)
                return row

            def project3(xP, w_pk, name, tag, mode, mid_emit=None,
                         first_wc=None):
                """(x @ W).T via 3-term bf16 split matmuls; xP is a pair.

                mode "f32": returns (yT, psn) — f32 tile + sum-of-squares
                psum row (norm fused into the evacuation).
                mode "pair": returns (yP, psn) — bf16 pair tile + norm psum.
                mid_emit() is called after the j==3 block so a prefetch DMA
                can ride the SP queue behind the first few weight columns.
                """
                psn = ps_sml.tile([1, 512], F32, tag="sml", name="npsum")
                if mode == "f32":
                    yT = big.tile([P, HT, 512], F32, tag=tag, name=name)
                else:
                    yP = big.tile([P, 2, HT, 512], BF16, tag=tag, name=name)
                # norm accumulation runs at lag 1 so the PE never waits on the
                # ACT square of the chunk it just produced.
                sqs = [None] * HT
                for j in range(HT):
                    wcP = first_wc if (j == 0 and first_wc is not None) \
                        else wcol_pair(w_pk, j)
                    pst = ps_mm.tile([P, 512], F32, tag="mm", name="projps")
                    for hi in range(HT):
                        nc.tensor.matmul(
                            pst[:], wcP[:, 0, hi, :], xP[:, 0, hi, :],
                            start=(hi == 0), stop=False)
                        nc.tensor.matmul(
                            pst[:], wcP[:, 0, hi, :], xP[:, 1, hi, :],
                            start=False, stop=False)
                        nc.tensor.matmul(
                            pst[:], wcP[:, 1, hi, :], xP[:, 0, hi, :],
                            start=False, stop=(hi == HT - 1))
                    sq = s512p.tile([P, 512], F32, tag="sqn", name="sqn")
                    nc.scalar.square(sq[:, :], pst[:])
                    sqs[j] = sq
                    if mode == "f32":
                        nc.scalar.activation(yT[:, j, :], pst[:], AF.Copy)
                    else:
                        tmp32 = s512p.tile([P, 512], F32, tag="s512",
                                           name="spj32")
                        emit_split(yP[:, 0, j, :], yP[:, 1, j, :], pst[:],
                                   tmp32[:])
                    if j >= 1:
                        nc.tensor.matmul(psn[:1, :], ones_col[:],
                                         sqs[j - 1][:, :],
                                         start=(j == 1), stop=False)
                    if mid_emit is not None and j in (3, 6, 9, 12):
                        mid_emit((j - 3) // 3)
                nc.tensor.matmul(psn[:1, :], ones_col[:], sqs[HT - 1][:, :],
                                 start=False, stop=True)
                if mode == "f32":
                    return yT, psn
                return yP, psn

            def store_pair_to_ag(xT, ag_in):
                """split scaled f32 keys and store bf16 pair to AG input."""
                for hi in range(HT):
                    sth = s512p.tile([P, 512], BF16, tag="st16h", name="sth",
                                     bufs=4)
                    stl = s512p.tile([P, 512], BF16, tag="st16l", name="stl",
                                     bufs=4)
                    tmp32 = s512p.tile([P, 512], F32, tag="s512", name="spg32")
                    emit_split(sth[:], stl[:], xT[:, hi, :], tmp32[:])
                    # stores ride the ACT queue (which paces them via the
                    # splits), keeping the SP queue free for weight loads
                    nc.scalar.dma_start(ag_in[0, hi * P:(hi + 1) * P, :], sth[:])
                    nc.scalar.dma_start(ag_in[1, hi * P:(hi + 1) * P, :], stl[:])

            def scale_cols(xT, scale_row):
                bc = s512p.tile([P, 512], F32, tag="s512", name="bcn")
                nc.gpsimd.partition_broadcast(bc[:, :], scale_row[:1, :])
                for hi in range(HT):
                    nc.vector.tensor_mul(xT[:, hi, :], xT[:, hi, :], bc[:, :])

            # ===================================================================
            # Phase M: sharded memory-side projections + AllGathers
            # ===================================================================
            ag_nek_in = dram.tile([2, H, NL], BF16, name="ag_nek_in")
            ag_nek_out = dram.tile([NCORES, 2, H, NL], BF16,
                                   addr_space="Shared", name="ag_nek_out")
            ag_ev_in = dram.tile([NL, H], BF16, name="ag_ev_in")
            ag_ev_out = dram.tile([N, H], BF16, addr_space="Shared",
                                  name="ag_ev_out")
            ag_nks_in = [dram.tile([2, H, 512], BF16, name=f"ag_nks_in{i}")
                         for i in range(ML // 512)]
            ag_nks_out = [dram.tile([NCORES, 2, H, 512], BF16,
                                    addr_space="Shared", name=f"ag_nks_out{i}")
                          for i in range(ML // 512)]

            # semantic-key chunk loads: double-buffered on alternating big
            # slots (bl/xTin), emitted via project3 mid_emit hooks so each
            # 12.6us DMA hides under the previous projection.
            skPs = [None] * (ML // 512)
            qTin_box = [None]

            def load_sk(mc, piece):
                """quarter-piece prefetch of a semantic-key chunk."""
                if piece == 0:
                    skPs[mc] = big.tile([P, 2, HT, 512], BF16,
                                        tag="bl" if mc % 2 == 0 else "xTin",
                                        name=f"skT{mc}")
                hs = slice(piece * 4, (piece + 1) * 4)
                nc.sync.dma_start(skPs[mc][:, :, hs], semk_pk[mc, :, :, hs])

            def load_qTin(piece):
                if piece == 0:
                    qTin_box[0] = big.tile([P, 2, HT, 512], BF16, tag="bl",
                                           name="qTin")
                hs = slice(piece * 4, (piece + 1) * 4)
                nc.sync.dma_start(qTin_box[0][:, :, hs], query_pk[0, :, :, hs])

            # First weight column rides the DMA queue ahead of epP so the very
            # first matmul chain starts as soon as epP's first piece lands;
            # epP is split so early hi-tiles arrive (and compute) first.
            wc_ek0 = wcol_pair(wek_pk, 0)
            epP = big.tile([P, 2, HT, 512], BF16, tag="xTin", name="epT")
            nc.sync.dma_start(epP[:, :, :HT // 4], ep_pk[0, :, :, :HT // 4])
            nc.sync.dma_start(epP[:, :, HT // 4:HT // 2],
                              ep_pk[0, :, :, HT // 4:HT // 2])
            nc.sync.dma_start(epP[:, :, HT // 2:], ep_pk[0, :, :, HT // 2:])

            # ---- episodic recency/importance weights (off-PE, overlaps ekT)
            def rec_weight(imp_ap, ts_ap, shape, tagb):
                """(1+imp)*exp(-|1-ts|*RECENCY) elementwise; returns tile."""
                impt = rows.tile(shape, F32, tag=tagb + "i", name="impt")
                tst = rows.tile(shape, F32, tag=tagb + "t", name="tst")
                nc.sync.dma_start(impt[:shape[0], :], imp_ap)
                nc.sync.dma_start(tst[:shape[0], :], ts_ap)
                s = tst[:shape[0], :]
                nc.scalar.activation(s, s, AF.Copy, bias=0.0, scale=-1.0)
                nc.vector.tensor_scalar_add(s, s, 1.0)
                nc.scalar.activation(s, s, AF.Abs)
                nc.scalar.activation(s, s, AF.Exp, scale=-RECENCY)
                si = impt[:shape[0], :]
                nc.vector.tensor_scalar_add(si, si, 1.0)
                nc.vector.tensor_mul(si, si, s)
                return impt

            wfull = rec_weight(ep_imp.rearrange("(p c) -> p c", p=P),
                               ep_ts.rearrange("(p c) -> p c", p=P),
                               [P, N // P], "wf")
            wpart = rows.tile([P, 1], F32, tag="wpart", name="wpart")
            nc.vector.reduce_sum(wpart[:, :], wfull[:, :], axis=AXL.X)
            pssum = ps_sml.tile([1, 512], F32, tag="sml", name="wsps")
            nc.tensor.matmul(pssum[:1, :1], ones_col[:], wpart[:, :],
                             start=True, stop=True)
            wsum = rows.tile([1, 1], F32, tag="wsum", name="wsum")
            nc.vector.tensor_copy(wsum[:1, :], pssum[:1, :1])
            nc.vector.tensor_scalar_add(wsum[:1, :], wsum[:1, :], 1e-8)
            nc.vector.reciprocal(wsum[:1, :], wsum[:1, :])
            wloc = rec_weight(ep_imp_s[None, :], ep_ts_s[None, :], [1, NL], "wl")
            nc.vector.tensor_scalar(wloc[:1, :], wloc[:1, :], wsum[:1, :1], None,
                                    op0=ALU.mult)

            # ---- episodic keys: project (norm fused), scale, store, AG;
            # skT0's load rides behind the early ek weight columns.
            ekT, psn_ek = project3(epP, wek_pk, "ekT", "kT", "f32",
                                   mid_emit=lambda p: load_sk(0, p),
                                   first_wc=wc_ek0)
            # ---- e_vals natural layout [NL, H]; bf16 single term
            for jc in range(H // 512):
                psts = [ps_mm.tile([P, 512], F32, tag="mm", name=f"evps{i}")
                        for i in range(NL // P)]
                for h2 in range(HT // 2):
                    wt16 = wtp.tile([P, 2, 512], BF16, tag="wt16", name="wt16",
                                    bufs=2)
                    nc.sync.dma_start(
                        wt16[:],
                        wev16[h2 * 256:(h2 + 1) * 256,
                              jc * 512:(jc + 1) * 512].rearrange(
                                  "(q p) c -> p q c", p=P))
                    for q2 in range(2):
                        hi = h2 * 2 + q2
                        for nt in range(NL // P):
                            ns = slice(nt * P, (nt + 1) * P)
                            nc.tensor.matmul(
                                psts[nt][:], epP[:, 0, hi, ns], wt16[:, q2],
                                start=(hi == 0), stop=(hi == HT - 1))
                for nt in range(NL // P):
                    evs = s512p.tile([P, 512], BF16, tag="evo16", name="evout", bufs=1)
                    nc.vector.tensor_copy(evs[:], psts[nt][:])
                    nc.scalar.dma_start(
                        ag_ev_in[nt * P:(nt + 1) * P, jc * 512:(jc + 1) * 512],
                        evs[:])
)],
                ins=[ag_ev_in.opt()], outs=[ag_ev_out.opt()])

            inv_ek = norm_row_finish(psn_ek, extra_row=wloc[:1, :])
            scale_cols(ekT, inv_ek)
            store_pair_to_ag(ekT, ag_nek_in)
            nc.gpsimd.collective_compute(
                "AllGather", ALU.bypass,
                replica_groups=[list(range(NCORES))],
                ins=[ag_nek_in.opt()], outs=[ag_nek_out.opt()])
            nc.gpsimd.collective_compute(
                "AllGather", ALU.bypass,
                replica_groups=[list(range(NCORES)

            # ---- semantic keys: 4 chunks of 512 (loads via mid_emit hooks)
            wc0_box = {"wc": None}
            for mc in range(ML // 512):
                nxt = (lambda p, m=mc + 1: load_sk(m, p)) \
                    if mc + 1 < ML // 512 else load_qTin
                ksT, psn_ks = project3(skPs[mc], wsk_pk, f"ksT{mc}", "kT",
                                       "f32", mid_emit=nxt,
                                       first_wc=wc0_box["wc"])
                # prefetch the next projection's first weight column ahead
                # of this chunk's norm/scale/store emission
                wc0_box["wc"] = wcol_pair(
                    wsk_pk if mc + 1 < ML // 512 else wq_pk, 0)
                inv_ks = norm_row_finish(psn_ks)
                scale_cols(ksT, inv_ks)
                store_pair_to_ag(ksT, ag_nks_in[mc])
                nc.gpsimd.collective_compute(
                    "AllGather", ALU.bypass,
                    replica_groups=[list(range(NCORES))],
                    ins=[ag_nks_in[mc].opt()], outs=[ag_nks_out[mc].opt()])

            # ===================================================================
            # Phase Q: query-side projections (norms fused)
            # ===================================================================
            qTinP = qTin_box[0]
            qTp, psn_q = project3(qTinP, wq_pk, "qT", "kT", "pair",
                                  first_wc=wc0_box["wc"])
            wc0_qs = wcol_pair(wsq_pk, 0)
            inv_q = norm_row_finish(psn_q)
            qsP, psn_qs = project3(qTp, wsq_pk, "qsT", "xTin", "pair",
                                   first_wc=wc0_qs)
            inv_qs = norm_row_finish(psn_qs)

            # transpose inv rows -> per-partition [128, NBT] via DRAM bounce
            invq_p = cst.tile([P, NBT], F32, name="invq_p")
            invqs_p = cst.tile([P, NBT], F32, name="invqs_p")
            bounce = dram.tile([2, BL], F32, name="bounce")
            nc.sync.dma_start(bounce[0:1, :], inv_q[:1, :])
            nc.sync.dma_start(bounce[1:2, :], inv_qs[:1, :])
            nc.sync.dma_start(
                invq_p[:, :], bounce[0:1, :].rearrange("o (t p) -> (o p) t", p=P))
            nc.sync.dma_start(
                invqs_p[:, :], bounce[1:2, :].rearrange("o (t p) -> (o p) t", p=P))

            def bcast_row(dram_row, width, pool, tag, name, dt=F32):
                row = rows.tile([1, width], F32, tag="crow", name="crow", bufs=1)
                nc.sync.dma_start(row[:1, :], dram_row)
                src = row[:1, :]
                if dt != F32:
                    row16 = rows.tile([1, width], dt, tag="crow16",
                                      name="crow16", bufs=1)
                    nc.scalar.activation(row16[:1, :], row[:1, :], AF.Copy)
                    src = row16[:1, :]
                t = pool.tile([P, width], dt, tag=tag, name=name, bufs=1)
                nc.gpsimd.partition_broadcast(t[:, :], src)
                return t

            b1bc = bcast_row(gate_b1[None, :], 64, cst, "b1bc", "b1bc")
            b2bc = bcast_row(gate_b2[None, :], 3, cst, "b2bc", "b2bc")
            # gate+work concatenated projection weights (bf16 pair); rides the
            # wcol ring slot freed after the last qs weight column.
            gwk = wcolp.tile([P, 2, HT, 128], BF16, tag="wcp", name="gwk")
            nc.sync.dma_start(gwk[:], gwk_pk)
            gw2 = cst.tile([64, 3], F32, name="gw2")
            nc.sync.dma_start(gw2[:, :], gate_W2)

            inv_sqrt_h = 1.0 / math.sqrt(H)
            ewT_pre = [None] * NBT
            gw_pre = [None] * NBT

            gate_st = [None] * NBT

            def emit_gate_a(bt):
                """Gate/work stage A: fused matmul (cols 0:64 gate hidden,
                64:128 work logits; stationary q bf16-hi, moving bf16 pair of
                hstack(gate_W1, work_slots.T)) + the off-PE softmax chain."""
                psg = ps_sml.tile([P, 128], F32, tag="sml", name="psg")
                bs = slice(bt * P, (bt + 1) * P)
                for hi in range(HT):
                    nc.tensor.matmul(
                        psg[:, :], qTp[:, 0, hi, bs], gwk[:, 0, hi, :],
                        start=(hi == 0), stop=False)
                    nc.tensor.matmul(
                        psg[:, :], qTp[:, 0, hi, bs], gwk[:, 1, hi, :],
                        start=False, stop=(hi == HT - 1))
                hid = tiny.tile([P, 64], F32, tag="c64", name="hid")
                nc.vector.tensor_add(hid[:, :], psg[:, :64], b1bc[:, :])
                nc.scalar.activation(hid[:, :], hid[:, :], AF.Silu)
                wmax = tiny.tile([P, 1], F32, tag="c1", name="wmax")
                nc.vector.reduce_max(wmax[:, :], psg[:, 64:], axis=AXL.X)
                nc.vector.tensor_scalar_mul(wmax[:, :], wmax[:, :], -inv_sqrt_h)
                ew = tiny.tile([P, S], F32, tag="cew", name="ew")
                nc.scalar.activation(ew[:, :], psg[:, 64:], AF.Exp,
                                     bias=wmax[:, :1], scale=inv_sqrt_h)
                zw = tiny.tile([P, 1], F32, tag="czw", name="zw")
                nc.vector.reduce_sum(zw[:, :], ew[:, :], axis=AXL.X)
                nc.vector.reciprocal(zw[:, :], zw[:, :])
                gate_st[bt] = (hid, ew, zw)

            def emit_gate_b(bt):
                """Gate/work stage B: transposes + gate MLP tail; its PE ops
                depend only on stage-A results finished a sim chunk ago."""
                hid, ew, zw = gate_st[bt]
                psht = ps_sml.tile([64, P], F32, tag="sml", name="hidtp")
                nc.tensor.transpose(out=psht[:64, :], in_=hid[:, :],
                                    identity=ident[:])
                hidT = tiny.tile([64, P], F32, tag="c128", name="hidT")
                nc.vector.tensor_copy(hidT[:, :], psht[:64, :])
                psg2 = ps_sml.tile([P, 3], F32, tag="sml", name="psg2")
                nc.tensor.matmul(psg2[:, :3], hidT[:, :], gw2[:, :],
                                 start=True, stop=True)
                gl = cst.tile([P, 3], F32, name=f"gl{bt}")
                nc.vector.tensor_add(gl[:, :], psg2[:, :3], b2bc[:, :])
                gmax = tiny.tile([P, 1], F32, tag="c1", name="gmax")
                nc.vector.reduce_max(gmax[:, :], gl[:, :], axis=AXL.X)
                nc.vector.tensor_scalar_mul(gmax[:, :], gmax[:, :], -1.0)
                nc.scalar.activation(gl[:, :], gl[:, :], AF.Exp, bias=gmax[:, :1])
                gz = tiny.tile([P, 1], F32, tag="c1", name="gz")
                nc.vector.reduce_sum(gz[:, :], gl[:, :], axis=AXL.X)
                nc.vector.reciprocal(gz[:, :], gz[:, :])
                nc.vector.tensor_scalar(gl[:, :], gl[:, :], gz[:, :1], None,
                                        op0=ALU.mult)
                gw_pre[bt] = gl
                # fold softmax normalization AND gate weight 0 into ew
                nc.vector.tensor_tensor(out=zw[:, :], in0=zw[:, :],
                                        in1=gl[:, 0:1], op=ALU.mult)
                nc.vector.tensor_scalar(ew[:, :], ew[:, :], zw[:, :1], None,
                                        op0=ALU.mult)

            def emit_gate_c(bt):
                """Gate/work stage C: transpose of the folded work probs."""
                _, ew, _ = gate_st[bt]
                pset = ps_sml.tile([S, P], F32, tag="sml", name="ewtp")
                nc.tensor.transpose(out=pset[:S, :], in_=ew[:, :],
                                    identity=ident[:])
                ewT = cst.tile([S, P], BF16, name=f"ewT{bt}")
                nc.vector.tensor_copy(ewT[:, :], pset[:S, :])
                ewT_pre[bt] = ewT

            # ===================================================================
            # Phase S: similarity + per-chunk top-8 candidates
            # ===================================================================
            cand_v_e = [big.tile([P, (N // 512) * 8], F32, tag=f"cve{bt}",
                                 name=f"cve{bt}") for bt in range(NBT)]
            cand_i_e = [big.tile([P, (N // 512) * 8], F32, tag=f"cie{bt}",
                                 name=f"cie{bt}") for bt in range(NBT)]
            cand_v_s = [big.tile([P, (M // 512) * 8], F32, tag=f"cvs{bt}",
                                 name=f"cvs{bt}") for bt in range(NBT)]
            cand_i_s = [big.tile([P, (M // 512) * 8], F32, tag=f"cis{bt}",
                                 name=f"cis{bt}") for bt in range(NBT)]

            def sim_chunk(xP, kd, r, ch, cand_v, cand_i, base):
                """sims of all 4 b-tiles vs bf16-pair keys kd[r, :, h, :]."""
                psts = [ps_mm.tile([P, 512], F32, tag="mm", name=f"simps{i}")
                        for i in range(NBT)]
                for hi in range(HT):
                    kth = s512p.tile([P, 512], BF16, tag="st16h", name="kth",
                                     bufs=4)
                    ktl = s512p.tile([P, 512], BF16, tag="st16l", name="ktl",
                                     bufs=4)
                    nc.sync.dma_start(
                        kth[:], kd[r, 0, hi * P:(hi + 1) * P, :])
                    nc.sync.dma_start(
                        ktl[:], kd[r, 1, hi * P:(hi + 1) * P, :])
                    for bt in range(NBT):
                        bs = slice(bt * P, (bt + 1) * P)
                        nc.tensor.matmul(
                            psts[bt][:], xP[:, 0, hi, bs], kth[:],
                            start=(hi == 0), stop=False)
                        nc.tensor.matmul(
                            psts[bt][:], xP[:, 0, hi, bs], ktl[:],
                            start=False, stop=False)
                        nc.tensor.matmul(
                            psts[bt][:], xP[:, 1, hi, bs], kth[:],
                            start=False, stop=(hi == HT - 1))
                for bt in range(NBT):
                    sc = simb.tile([P, 512], F32, tag="simc", name="simc",
                                   bufs=2)
                    nc.scalar.activation(sc[:], psts[bt][:], AF.Copy)
                    mx = simb.tile([P, 8], F32, tag="mx", name="mx")
                    mi = simb.tile([P, 8], U32, tag="mi", name="mi")
                    nc.vector.max(out=mx[:], in_=sc[:])
                    nc.vector.max_index(out=mi[:], in_max=mx[:], in_values=sc[:])
                    nc.vector.tensor_copy(cand_v[bt][:, ch * 8:(ch + 1) * 8],
                                          mx[:])
                    mif = simb.tile([P, 8], F32, tag="mif", name="mif")
                    nc.vector.tensor_copy(mif[:], mi[:])
                    nc.vector.tensor_scalar_add(
                        cand_i[bt][:, ch * 8:(ch + 1) * 8], mif[:],
                        float(base))

            def topk_attend(cand_v, cand_i, k, inv_p, bt, vals_dram, gscale,
                            acc_tag, bufs=2):
                """Merged top-k -> softmax (x gscale) -> gather + weighted sum."""
                top8 = tiny.tile([P, 8], F32, tag="c8", name="top8")
                nc.vector.max(out=top8[:], in_=cand_v[:])
                idxf = tiny.tile([P, 8], F32, tag="c8", name="idxf")
                eqm = s512p.tile([P, 256], F32, tag="sqn", name="eqm")
                for kk in range(k):
                    w = cand_v.shape[-1]
                    nc.vector.tensor_scalar(
                        eqm[:, :w], cand_v[:], top8[:, kk:kk + 1], None,
                        op0=ALU.is_equal)
                    nc.vector.tensor_tensor(
                        out=eqm[:, :w], in0=eqm[:, :w], in1=cand_i[:], op=ALU.mult)
                    nc.vector.reduce_sum(idxf[:, kk:kk + 1], eqm[:, :w], axis=AXL.X)
                idxu = tiny.tile([P, 8], U32, tag="c8u", name="idxu")
                nc.vector.tensor_copy(idxu[:, :k], idxf[:, :k])
                sc8 = tiny.tile([P, 8], F32, tag="c8", name="sc8")
                nc.vector.tensor_scalar(
                    sc8[:, :k], top8[:, :k], inv_p[:, bt:bt + 1], None,
                    op0=ALU.mult)
                negm = tiny.tile([P, 1], F32, tag="c1", name="negm")
                nc.vector.tensor_scalar_mul(negm[:, :], sc8[:, 0:1], -1.0)
                nc.scalar.activation(sc8[:, :k], sc8[:, :k], AF.Exp,
                                     bias=negm[:, :1])
                zs = tiny.tile([P, 1], F32, tag="c1", name="zs")
                nc.vector.reduce_sum(zs[:, :], sc8[:, :k], axis=AXL.X)
                nc.vector.reciprocal(zs[:, :], zs[:, :])
                nc.vector.tensor_scalar(zs[:, :], zs[:, :], gscale, None,
                                        op0=ALU.mult)
                nc.vector.tensor_scalar(sc8[:, :k], sc8[:, :k], zs[:, :1], None,
                                        op0=ALU.mult)
                acc = sm.tile([P, H], BF16, tag=acc_tag, name="acc" + acc_tag,
                              bufs=bufs)
                nc.vector.memset(acc[:, :], 0.0)
                for kk in range(k):
                    g = gath.tile([P, H], BF16, tag="g", name="g")
                    nc.gpsimd.indirect_dma_start(
                        out=g[:, :], out_offset=None, in_=vals_dram,
                        in_offset=bass.IndirectOffsetOnAxis(
                            ap=idxu[:, kk:kk + 1], axis=0))
                    nc.vector.scalar_tensor_tensor(
                        out=acc[:, :], in0=g[:, :], scalar=sc8[:, kk:kk + 1],
                        in1=acc[:, :], op0=ALU.mult, op1=ALU.add)
                return acc

            def transpose_into(dst, src, dt=BF16):
                """dst [P, HT, P] view <- transpose of src [P, H]; psum
                evacuation alternates ACT/DVE so neither sequencer's
                per-op dispatch overhead paces the chain."""
                idn = ident if dt == F32 else ident16
                for hi in range(HT):
                    pst = ps_mm.tile([P, P], dt, tag="mm", name="trf")
                    nc.tensor.transpose(out=pst[:], in_=src[:, hi * P:(hi + 1) * P],
                                        identity=idn[:])
                    nc.scalar.activation(dst[:, hi, :], pst[:], AF.Copy)

            # episodic sims: one gathered buffer, rank-major global indices;
            # gate/work stages (off-PE-latency-heavy) interleave with chunks
            # so each stage's PE ops only see dependencies already finished.
            gbc2 = [None] * (H // 512)
            bbc2 = [None] * (H // 512)
            for ch in range(N // 512):
                if 2 <= ch <= NBT + 1:
                    emit_gate_c(ch - 2)
                if 1 <= ch <= NBT:
                    emit_gate_b(ch - 1)
                if ch < NBT:
                    emit_gate_a(ch)
                if ch == 4:
                    # LN gamma/beta broadcast tiles (full row, loaded in 512
                    # chunks); the serial DMA<->POOL ping-pong hides under
                    # the remaining sim chunks.
                    gbc2[0] = sm.tile([P, H], BF16, tag="gbc", name="gbc",
                                      bufs=1)
                    bbc2[0] = sm.tile([P, H], BF16, tag="bbc", name="bbc",
                                      bufs=1)
                    for t, dsrc in ((gbc2[0], ln_gamma), (bbc2[0], ln_beta)):
                        for jq in range(H // 512):
                            cq = slice(jq * 512, (jq + 1) * 512)
                            row = rows.tile([1, 512], F32, tag="crow",
                                            name="crow", bufs=1)
                            nc.sync.dma_start(row[:1, :], dsrc[None, cq])
                            row16 = rows.tile([1, 512], BF16, tag="crow16",
                                              name="crow16", bufs=1)
                            nc.scalar.activation(row16[:1, :], row[:1, :],
                                                 AF.Copy)
                            nc.gpsimd.partition_broadcast(t[:, cq],
                                                          row16[:1, :])
                sim_chunk(qTp, ag_nek_out, ch, ch, cand_v_e, cand_i_e,
                          ch * 512)

            # episodic merges (DVE/gathers) overlap semantic sims (PE); the
            # accT_e transposes are emitted after a sem chunk each so the PE
            # queue never waits on a merge.
            accT_e = big.tile([P, NBT, HT, P], BF16, tag="kT", name="accTe")
            accT_s = big.tile([P, NBT, HT, P], BF16, tag="xTin", name="accTs")
            acc_e = [None] * NBT
            acc_s = [None] * NBT

            def emit_merge_e(bt):
                acc_e[bt] = topk_attend(cand_v_e[bt][:], cand_i_e[bt][:], EP_K,
                                        invq_p, bt, ag_ev_out[:, :],
                                        gw_pre[bt][:, 1:2], "sl1")

            def emit_merge_s(bt):
                acc_s[bt] = topk_attend(cand_v_s[bt][:], cand_i_s[bt][:], SEM_K,
                                        invqs_p, bt, semv16, gw_pre[bt][:, 2:3],
                                        "sl2", bufs=3)

            sem_seq = [(i, r) for i in range(ML // 512) for r in range(NCORES)]

            def emit_sem_chunk(ch):
                i, r = sem_seq[ch]
                sim_chunk(qsP, ag_nks_out[i], r, ch, cand_v_s, cand_i_s,
                          r * ML + i * 512)

            emit_merge_e(0)
            emit_merge_e(1)
            p2a_pre = {}
            for ch in range(len(sem_seq)):
                if ch == len(sem_seq) - 2:
                    # prefetch Pass 2a's first moving tiles so its opening
                    # matmuls don't wait on the DMA queue draining
                    wsn0 = s512p.tile([S, 512], BF16, tag="s512", name="wsn2")
                    nc.sync.dma_start(wsn0[:S, :], ws16[:, :512])
                    wt0 = wtp.tile([P, 4, 512], BF16, tag="wt", name="wto")
                    nc.sync.dma_start(
                        wt0[:], weo16[:512, :512].rearrange(
                            "(q p) c -> p q c", p=P))
                    p2a_pre["wsn"] = wsn0
                    p2a_pre["wt"] = wt0
                emit_sem_chunk(ch)
                if ch < NBT:
                    transpose_into(accT_e[:, ch], acc_e[ch])
                    if ch + 2 < NBT:
                        emit_merge_e(ch + 2)

            # ===================================================================
            # Phase F: blend + output projections + streaming layernorm
            # ===================================================================
            bl_all = big.tile([P, NBT, H], BF16, tag="bl", name="bl_all")

            emit_merge_s(0)
            emit_merge_s(1)
            emit_merge_s(2)
            # Pass 2a: bl = gate0*w_out + acc_e @ W_eo (jc-major, weights read
            # once); ACT evacuates so DVE stays free for the semantic merges,
            # which run concurrently on DVE. The accT_s transposes interleave
            # between jc blocks: Ts_k lands right after merge k finishes, and
            # releasing acc_s[0] lets merge 3's ring slot allocate.
            for jc in range(H // 512):
                cs = slice(jc * 512, (jc + 1) * 512)
                if jc == 0:
                    wsn = p2a_pre["wsn"]
                else:
                    wsn = s512p.tile([S, 512], BF16, tag="s512", name="wsn2")
                    nc.sync.dma_start(wsn[:S, :], ws16[:, cs])
                psos = [ps_mm.tile([P, 512], F32, tag="mm", name=f"pso{i}")
                        for i in range(NBT)]
                for bt in range(NBT):
                    nc.tensor.matmul(psos[bt][:], ewT_pre[bt][:, :],
                                     wsn[:S, :], start=True, stop=False)
                for hq in range(HT // 4):
                    if jc == 0 and hq == 0:
                        wt = p2a_pre["wt"]
                    else:
                        wt = wtp.tile([P, 4, 512], BF16, tag="wt", name="wto")
                        nc.sync.dma_start(
                            wt[:], weo16[hq * 512:(hq + 1) * 512, cs].rearrange(
                                "(q p) c -> p q c", p=P))
                    for q4 in range(4):
                        hi = hq * 4 + q4
                        for bt in range(NBT):
                            nc.tensor.matmul(
                                psos[bt][:], accT_e[:, bt, hi, :], wt[:, q4],
                                start=False, stop=(hi == HT - 1))
                for bt in range(NBT):
                    nc.scalar.activation(bl_all[:, bt, cs], psos[bt][:], AF.Copy)
                if 1 <= jc:
                    transpose_into(accT_s[:, jc - 1], acc_s[jc - 1])
                    if jc == H // 512 - 1:
                        transpose_into(accT_s[:, jc], acc_s[jc])
                if jc == 0:
                    emit_merge_s(3)

            blT = big.tile([P, NBT, HT, P], BF16, tag="kT", name="blT")
            # Pass 2b: bl += acc_s @ W_so (jc-major, weights read once)
            for jc in range(H // 512):
                cs = slice(jc * 512, (jc + 1) * 512)
                psob = [ps_mm.tile([P, 512], F32, tag="mm", name=f"psob{i}")
                        for i in range(NBT)]
                for hq in range(HT // 4):
                    wt = wtp.tile([P, 4, 512], BF16, tag="wt", name="wtob")
                    nc.sync.dma_start(
                        wt[:], wso16[hq * 512:(hq + 1) * 512, cs].rearrange(
                            "(q p) c -> p q c", p=P))
                    for q4 in range(4):
                        hi = hq * 4 + q4
                        for bt in range(NBT):
                            nc.tensor.matmul(
                                psob[bt][:], accT_s[:, bt, hi, :], wt[:, q4],
                                start=(hi == 0), stop=(hi == HT - 1))
                for bt in range(NBT):
                    nc.vector.tensor_add(bl_all[:, bt, cs],
                                         bl_all[:, bt, cs], psob[bt][:])
                    if jc == H // 512 - 1:
                        # bl_all[bt] now complete: start its transpose DMA
                        nc.sync.dma_start_transpose(blT[:, bt],
                                                    bl_all[:, bt, :])

            # Pass 3: xo = bl @ W_ro (jc-major) with streamed LN stats
            xo_all = big.tile([P, NBT, H], BF16, tag="xTin", name="xo_all")
            msum = [tiny.tile([P, 4], F32, tag=f"cms{i}", name=f"msum{i}",
                              bufs=1) for i in range(NBT)]
            vsum = [tiny.tile([P, 4], F32, tag=f"cvs{i}", name=f"vsum{i}",
                              bufs=1) for i in range(NBT)]
            ln_stats = [None] * NBT

            def emit_ln_stats(bt):
                """inv-std and -mu*inv-std per-partition scalars for one bt."""
                mu = tiny.tile([P, 1], F32, tag="c1", name="mu")
                nc.vector.reduce_sum(mu[:, :], msum[bt][:, :], axis=AXL.X)
                nc.vector.tensor_scalar_mul(mu[:, :], mu[:, :], 1.0 / H)
                vs = tiny.tile([P, 1], F32, tag=f"cvv{bt}", name="vs", bufs=1)
                nc.vector.reduce_sum(vs[:, :], vsum[bt][:, :], axis=AXL.X)
                nc.vector.tensor_scalar_mul(vs[:, :], vs[:, :], 1.0 / H)
                mu2 = tiny.tile([P, 1], F32, tag="c1", name="mu2")
                nc.vector.tensor_tensor(out=mu2[:, :], in0=mu[:, :],
                                        in1=mu[:, :], op=ALU.mult)
                nc.vector.tensor_tensor(out=vs[:, :], in0=vs[:, :],
                                        in1=mu2[:, :], op=ALU.subtract)
                nc.vector.tensor_scalar_add(vs[:, :], vs[:, :], LN_EPS)
                nc.scalar.sqrt(vs[:, :], vs[:, :])
                nc.vector.reciprocal(vs[:, :], vs[:, :])
                nmu = tiny.tile([P, 1], F32, tag=f"cnm{bt}", name="nmu",
                                bufs=1)
                nc.vector.tensor_tensor(out=nmu[:, :], in0=mu[:, :],
                                        in1=vs[:, :], op=ALU.mult)
                nc.vector.tensor_scalar_mul(nmu[:, :], nmu[:, :], -1.0)
                ln_stats[bt] = (vs, nmu)
            def emit_ln_final(bt):
                """Normalize + affine + store for one b-tile: full-row bf16
                DVE ops (2x throughput, minimal dispatch count) on the gather
                ring, which is idle by this point."""
                vs, nmu = ln_stats[bt]
                on16 = gath.tile([P, H], BF16, tag="g", name="on16")
                nc.vector.tensor_scalar(on16[:, :], xo_all[:, bt, :],
                                        vs[:, :1], nmu[:, :1],
                                        op0=ALU.mult, op1=ALU.add)
                nc.vector.tensor_mul(on16[:, :], on16[:, :], gbc2[0][:, :])
                on = gath.tile([P, H], BF16, tag="g", name="on")
                nc.vector.tensor_add(on[:, :], on16[:, :], bbc2[0][:, :])
                nc.sync.dma_start(out_s[bt * P:(bt + 1) * P, :], on[:])

            for jc in range(H // 512):
                cs = slice(jc * 512, (jc + 1) * 512)
                psro = [ps_mm.tile([P, 512], F32, tag="mm", name=f"psro{i}")
                        for i in range(NBT)]
                for hq in range(HT // 4):
                    wt = wtp.tile([P, 4, 512], BF16, tag="wt", name="wtro")
                    nc.sync.dma_start(
                        wt[:], wro16[hq * 512:(hq + 1) * 512, cs].rearrange(
                            "(q p) c -> p q c", p=P))
                    for q4 in range(4):
                        hi = hq * 4 + q4
                        for bt in range(NBT):
                            nc.tensor.matmul(
                                psro[bt][:], blT[:, bt, hi, :], wt[:, q4],
                                start=(hi == 0), stop=(hi == HT - 1))
                for bt in range(NBT):
                    nc.scalar.activation(xo_all[:, bt, cs], psro[bt][:],
                                         AF.Copy,
                                         accum_out=msum[bt][:, jc:jc + 1])
                    sqc = s512p.tile([P, 512], F32, tag="sqn", name="sqc")
                    nc.scalar.activation(sqc[:, :], psro[bt][:], AF.Square,
                                         accum_out=vsum[bt][:, jc:jc + 1])
                    if jc == H // 512 - 1:
                        emit_ln_stats(bt)
            for bt in range(NBT):
                emit_ln_final(bt)

    nc.finalize()
    return nc


_NC_CACHE = None
LAST_EXEC_NS = None


def _pack_xpair(x):
    """[R,H] f32 -> [R//512, P, 2, HT, 512] bf16 pair, pre-transposed to
    the on-chip tile layout: pk[ch, p, half, hi, r] = split(x)[half][
    ch*512+r, hi*128+p]."""
    hi_, lo_ = _split_bf16(x)
    def lay(a):
        return a.reshape(-1, HT, P).transpose(2, 1, 0)   # [P, HT, R]
    pk = np.stack([lay(hi_), lay(lo_)], axis=1)          # [P, 2, HT, R]
    R = x.shape[0]
    return np.ascontiguousarray(
        np.stack([pk[..., i * 512:(i + 1) * 512]
                  for i in range(R // 512)], axis=0))


def _pack_wpair(w):
    """[H,H] f32 -> [HT, P, 2, HT, P] bf16 pair in wcP tile layout:
    packed[j, p, half, hi, c] = split(W)[half][hi*128+p, j*128+c]."""
    hi_, lo_ = _split_bf16(w)
    def lay(a):
        # [hi, p, j, c] -> [j, p, hi, c]
        return np.ascontiguousarray(
            a.reshape(HT, P, HT, P).transpose(2, 1, 0, 3))
    return np.ascontiguousarray(
        np.stack([lay(hi_), lay(lo_)], axis=2))


def _pack_gwk(gate_W1, work_slots):
    """hstack(gate_W1 [H,64], work_slots.T [H,64]) -> [P, 2, HT, 128] pair:
    pk[p, half, hi, c] = split(gw)[half][hi*128+p, c]."""
    gw = np.hstack([np.asarray(gate_W1, np.float32),
                    np.ascontiguousarray(np.asarray(work_slots, np.float32).T)])
    hi_, lo_ = _split_bf16(gw)
    def lay(a):
        return a.reshape(HT, P, 128).transpose(1, 0, 2)   # [P, HT, 128]
    return np.ascontiguousarray(np.stack([lay(hi_), lay(lo_)], axis=1))


def _split_bf16(x):
    """two-term bf16 decomposition: x ~= hi + lo to ~16 mantissa bits."""
    import ml_dtypes
    bf = ml_dtypes.bfloat16
    x = np.ascontiguousarray(np.asarray(x), dtype=np.float32)
    hi = x.astype(bf)
    lo = (x - hi.astype(np.float32)).astype(bf)
    return hi, lo


def kernel(**inputs) -> np.ndarray:
    global _NC_CACHE
    if _NC_CACHE is None:
        _NC_CACHE = build()
    nc = _NC_CACHE

    def arr(x):
        return np.ascontiguousarray(np.asarray(x), dtype=np.float32)

    wq_pk = _pack_wpair(inputs["W_query"])
    wek_pk = _pack_wpair(inputs["W_ek"])
    wsq_pk = _pack_wpair(inputs["W_sq"])
    wsk_pk = _pack_wpair(inputs["W_sk"])
    wev16, _ = _split_bf16(inputs["W_ev"])
    weo16, _ = _split_bf16(inputs["W_eo"])
    wso16, _ = _split_bf16(inputs["W_so"])
    wro16, _ = _split_bf16(inputs["W_ro"])
    semv16, _ = _split_bf16(inputs["sem_values"])
    ws16, _ = _split_bf16(inputs["work_slots"])
    gwk_pk = _pack_gwk(inputs["gate_W1"], inputs["work_slots"])

    in_maps = []
    for c in range(NCORES):
        in_maps.append({
            "query_pk": _pack_xpair(inputs["query"][c * BL:(c + 1) * BL]),
            "ep_pk": _pack_xpair(inputs["ep_store"][c * NL:(c + 1) * NL]),
            "semk_pk": _pack_xpair(inputs["sem_keys"][c * ML:(c + 1) * ML]),
            "ep_imp_s": arr(inputs["ep_importance"][c * NL:(c + 1) * NL]),
            "ep_ts_s": arr(inputs["ep_timestamps"][c * NL:(c + 1) * NL]),
            "ep_imp": arr(inputs["ep_importance"]),
            "ep_ts": arr(inputs["ep_timestamps"]),
            "semv16": semv16,
            "wq_pk": wq_pk,
            "wek_pk": wek_pk,
            "wsq_pk": wsq_pk,
            "wsk_pk": wsk_pk,
            "wev16": wev16,
            "weo16": weo16,
            "wso16": wso16,
            "wro16": wro16,
            "ws16": ws16,
            "gwk_pk": gwk_pk,
            "gate_b1": arr(inputs["gate_b1"]),
            "gate_W2": arr(inputs["gate_W2"]),
            "gate_b2": arr(inputs["gate_b2"]),
            "ln_gamma": arr(inputs["ln_gamma"]),
            "ln_beta": arr(inputs["ln_beta"]),
        })
    res = run_bass_kernel_spmd(nc, in_maps, core_ids=list(range(NCORES)))
    return np.concatenate(
        [np.asarray(res.results[c]["out_s"], dtype=np.float32)
         for c in range(NCORES)], axis=0)


# revision 69
# speedup vs baseline: 1.0499x; 1.0086x over previous
"""ONIMemoryHub kernel for 8 Trainium2 NeuronCores (Bass/Tile).

Sharding: data-parallel over batch for the query side; episodic store and
semantic memory sharded across cores for the key/value projections, with
AllGathers of the projected (normalized, pre-scaled) keys/values.

Schedule notes (v2): the PE instruction stream is kept free of stalls by
emitting off-engine work (top-k merges, norms, layernorm) interleaved
between matmul blocks whose inputs are already resident:
  - projection column norms are fused into the projection evacuation
  - gate/work blocks interleave with the episodic sim chunks
  - episodic merges/transposes interleave with the semantic sim chunks
  - semantic merges interleave with the W_eo output pass
  - W_so/W_ro passes run b-tile-major with a streaming layernorm so the
    kernel tail is one b-tile's LN instead of a full LN pass.

kernel(**inputs) takes FULL inputs (as produced by reference.setup_inputs())
and returns the FULL [4096, 2048] output.
"""
import math

import numpy as np

import concourse.bass as bass
import concourse.mybir as mybir
import concourse.tile as tile
from concourse import bacc
from concourse.bass_utils import run_bass_kernel_spmd
from concourse.masks import make_identity

AF = mybir.ActivationFunctionType
AXL = mybir.AxisListType
ALU = mybir.AluOpType

NCORES = 8
B, H, N, M, S = 4096, 2048, 4096, 16384, 64
BL, NL, ML = B // NCORES, N // NCORES, M // NCORES   # 512, 512, 2048
HT = H // 128                                        # 16 h-tiles
P = 128
NBT = BL // P                                        # 4 b-tiles
EP_K = 8
SEM_K = 4
LN_EPS = 1e-5
RECENCY = 0.01   # 1 - RECENCY_DECAY

F32 = mybir.dt.float32
BF16 = mybir.dt.bfloat16
U32 = mybir.dt.uint32


def build():
    nc = bacc.Bacc("TRN2", target_bir_lowering=False, debug=False,
                   num_devices=NCORES)

    def din(name, shape, dt=F32):
        return nc.dram_tensor(name, shape, dt, kind="ExternalInput").ap()

    # per-core slices: host-split bf16 pairs, pre-transposed to tile layout
    query_pk = din("query_pk", [1, P, 2, HT, 512], BF16)
    ep_pk = din("ep_pk", [1, P, 2, HT, 512], BF16)
    semk_pk = din("semk_pk", [ML // 512, P, 2, HT, 512], BF16)
    ep_imp_s = din("ep_imp_s", [NL])
    ep_ts_s = din("ep_ts_s", [NL])
    # replicated
    ep_imp = din("ep_imp", [N])
    ep_ts = din("ep_ts", [N])
    semv16 = din("semv16", [M, H], BF16)
    wq_pk = din("wq_pk", [HT, P, 2, HT, P], BF16)
    wek_pk = din("wek_pk", [HT, P, 2, HT, P], BF16)
    wsq_pk = din("wsq_pk", [HT, P, 2, HT, P], BF16)
    wsk_pk = din("wsk_pk", [HT, P, 2, HT, P], BF16)
    wev16 = din("wev16", [H, H], BF16)
    weo16 = din("weo16", [H, H], BF16)
    wso16 = din("wso16", [H, H], BF16)
    wro16 = din("wro16", [H, H], BF16)
    ws16 = din("ws16", [S, H], BF16)
    gwk_pk = din("gwk_pk", [P, 2, HT, 128], BF16)
    gate_b1 = din("gate_b1", [64])
    gate_W2 = din("gate_W2", [64, 3])
    gate_b2 = din("gate_b2", [3])
    ln_gamma = din("ln_gamma", [H])
    ln_beta = din("ln_beta", [H])

    out_s = nc.dram_tensor("out_s", [BL, H], BF16, kind="ExternalOutput").ap()

    with tile.TileContext(nc) as tc:
        with (
            tc.tile_pool(name="cst", bufs=1) as cst,
            tc.tile_pool(name="big", bufs=1) as big,
            tc.tile_pool(name="rows", bufs=1) as rows,
            tc.tile_pool(name="s512", bufs=2) as s512p,
            tc.tile_pool(name="wcol", bufs=2) as wcolp,
            tc.tile_pool(name="wtile", bufs=2) as wtp,
            tc.tile_pool(name="sm", bufs=2) as sm,
            tc.tile_pool(name="tiny", bufs=2) as tiny,
            tc.tile_pool(name="simb", bufs=2) as simb,
            tc.tile_pool(name="gath", bufs=2) as gath,
            tc.tile_pool(name="ps_mm", bufs=7, space="PSUM") as ps_mm,
            tc.tile_pool(name="ps_sml", bufs=1, space="PSUM") as ps_sml,
            tc.tile_pool(name="dram", bufs=1, space="DRAM") as dram,
        ):
            ident = cst.tile([P, P], F32)
            make_identity(nc, ident[:])
            ident16 = cst.tile([P, P], BF16)
            nc.scalar.activation(ident16[:], ident[:], AF.Copy)
            ones_col = cst.tile([P, 1], F32)
            nc.vector.memset(ones_col[:], 1.0)

            # ---------- helpers ----------
            # big slot chains (explicit liveness via shared tags):
            #   xTin: epT -> skT(x4) -> qsT -> accTs
            #   kT  : ekT -> ksT(x4) -> accTe -> blT
            #   bl  : qTp -> bl_all
            def emit_split(dst_hi, dst_lo, src_f32, tmp32):
                """bf16 two-term split: hi = bf16(x), lo = bf16(x - hi).

                The upconvert copy runs on DVE, not gpsimd: the Pool queue
                carries the collectives, which would head-block a gpsimd
                copy (and everything after it) for a whole AllGather.
                """
                nc.scalar.activation(dst_hi, src_f32, AF.Copy)
                nc.vector.tensor_copy(tmp32, dst_hi)
                nc.vector.tensor_tensor(out=tmp32, in0=src_f32, in1=tmp32,
                                        op=ALU.subtract)
                nc.scalar.activation(dst_lo, tmp32, AF.Copy)

            def wcol_pair(w_pk, j):
                wcP = wcolp.tile([P, 2, HT, P], BF16, tag="wcp", name="wcp")
                nc.sync.dma_start(wcP[:], w_pk[j])
                return wcP

            def norm_row_finish(psn, extra_row=None):
                """[1,512] inv-norm row from accumulated sum-of-squares."""
                row = rows.tile([1, 512], F32, tag="nrow", name="nrow", bufs=2)
                nc.vector.tensor_copy(row[:1, :], psn[:1, :])
                nc.scalar.sqrt(row[:1, :], row[:1, :])
                nc.vector.tensor_scalar_max(row[:1, :], row[:1, :], 1e-12)
                nc.vector.reciprocal(row[:1, :], row[:1, :])
                if extra_row is not None:
                    nc.vector.tensor_mul(row[:1, :], row[:1, :], extra_row)
                return row

            def project3(xP, w_pk, name, tag, mode, mid_emit=None,
                         first_wc=None):
                """(x @ W).T via 3-term bf16 split matmuls; xP is a pair.

                mode "f32": returns (yT, psn) — f32 tile + sum-of-squares
                psum row (norm fused into the evacuation).
                mode "pair": returns (yP, psn) — bf16 pair tile + norm psum.
                mid_emit() is called after the j==3 block so a prefetch DMA
                can ride the SP queue behind the first few weight columns.
                """
                psn = ps_sml.tile([1, 512], F32, tag="sml", name="npsum")
                if mode == "f32":
                    yT = big.tile([P, HT, 512], F32, tag=tag, name=name)
                else:
                    yP = big.tile([P, 2, HT, 512], BF16, tag=tag, name=name)
                # norm accumulation runs at lag 1 so the PE never waits on the
                # ACT square of the chunk it just produced.
                sqs = [None] * HT
                for j in range(HT):
                    wcP = first_wc if (j == 0 and first_wc is not None) \
                        else wcol_pair(w_pk, j)
                    pst = ps_mm.tile([P, 512], F32, tag="mm", name="projps")
                    for hi in range(HT):
                        nc.tensor.matmul(
                            pst[:], wcP[:, 0, hi, :], xP[:, 0, hi, :],
                            start=(hi == 0), stop=False)
                        nc.tensor.matmul(
                            pst[:], wcP[:, 0, hi, :], xP[:, 1, hi, :],
                            start=False, stop=False)
                        nc.tensor.matmul(
                            pst[:], wcP[:, 1, hi, :], xP[:, 0, hi, :],
                            start=False, stop=(hi == HT - 1))
                    sq = s512p.tile([P, 512], F32, tag="sqn", name="sqn")
                    nc.scalar.square(sq[:, :], pst[:])
                    sqs[j] = sq
                    if mode == "f32":
                        nc.scalar.activation(yT[:, j, :], pst[:], AF.Copy)
                    else:
                        tmp32 = s512p.tile([P, 512], F32, tag="s512",
                                           name="spj32")
                        emit_split(yP[:, 0, j, :], yP[:, 1, j, :], pst[:],
                                   tmp32[:])
                    if j >= 1:
                        nc.tensor.matmul(psn[:1, :], ones_col[:],
                                         sqs[j - 1][:, :],
                                         start=(j == 1), stop=False)
                    if mid_emit is not None and j in (3, 6, 9, 12):
                        mid_emit((j - 3) // 3)
                nc.tensor.matmul(psn[:1, :], ones_col[:], sqs[HT - 1][:, :],
                                 start=False, stop=True)
                if mode == "f32":
                    return yT, psn
                return yP, psn

            def store_pair_to_ag(xT, ag_in):
                """split scaled f32 keys and store bf16 pair to AG input."""
                for hi in range(HT):
                    sth = s512p.tile([P, 512], BF16, tag="st16h", name="sth",
                                     bufs=4)
                    stl = s512p.tile([P, 512], BF16, tag="st16l", name="stl",
                                     bufs=4)
                    tmp32 = s512p.tile([P, 512], F32, tag="s512", name="spg32")
                    emit_split(sth[:], stl[:], xT[:, hi, :], tmp32[:])
                    # stores ride the ACT queue (which paces them via the
                    # splits), keeping the SP queue free for weight loads
                    nc.scalar.dma_start(ag_in[0, hi * P:(hi + 1) * P, :], sth[:])
                    nc.scalar.dma_start(ag_in[1, hi * P:(hi + 1) * P, :], stl[:])

            def scale_cols(xT, scale_row):
                bc = s512p.tile([P, 512], F32, tag="s512", name="bcn")
                nc.gpsimd.partition_broadcast(bc[:, :], scale_row[:1, :])
                for hi in range(HT):
                    nc.vector.tensor_mul(xT[:, hi, :], xT[:, hi, :], bc[:, :])

            # ===================================================================
            # Phase M: sharded memory-side projections + AllGathers
            # ===================================================================
            ag_nek_in = dram.tile([2, H, NL], BF16, name="ag_nek_in")
            ag_nek_out = dram.tile([NCORES, 2, H, NL], BF16,
                                   addr_space="Shared", name="ag_nek_out")
            ag_ev_in = dram.tile([NL, H], BF16, name="ag_ev_in")
            ag_ev_out = dram.tile([N, H], BF16, addr_space="Shared",
                                  name="ag_ev_out")
            ag_nks_in = [dram.tile([2, H, 512], BF16, name=f"ag_nks_in{i}")
                         for i in range(ML // 512)]
            ag_nks_out = [dram.tile([NCORES, 2, H, 512], BF16,
                                    addr_space="Shared", name=f"ag_nks_out{i}")
                          for i in range(ML // 512)]

            # semantic-key chunk loads: double-buffered on alternating big
            # slots (bl/xTin), emitted via project3 mid_emit hooks so each
            # 12.6us DMA hides under the previous projection.
            skPs = [None] * (ML // 512)
            qTin_box = [None]

            def load_sk(mc, piece):
                """quarter-piece prefetch of a semantic-key chunk."""
                if piece == 0:
                    skPs[mc] = big.tile([P, 2, HT, 512], BF16,
                                        tag="bl" if mc % 2 == 0 else "xTin",
                                        name=f"skT{mc}")
                hs = slice(piece * 4, (piece + 1) * 4)
                nc.sync.dma_start(skPs[mc][:, :, hs], semk_pk[mc, :, :, hs])

            def load_qTin(piece):
                if piece == 0:
                    qTin_box[0] = big.tile([P, 2, HT, 512], BF16, tag="bl",
                                           name="qTin")
                hs = slice(piece * 4, (piece + 1) * 4)
                nc.sync.dma_start(qTin_box[0][:, :, hs], query_pk[0, :, :, hs])

            # First weight column rides the DMA queue ahead of epP so the very
            # first matmul chain starts as soon as epP's first piece lands;
            # epP is split so early hi-tiles arrive (and compute) first.
            wc_ek0 = wcol_pair(wek_pk, 0)
            epP = big.tile([P, 2, HT, 512], BF16, tag="xTin", name="epT")
            nc.sync.dma_start(epP[:, :, :HT // 4], ep_pk[0, :, :, :HT // 4])
            nc.sync.dma_start(epP[:, :, HT // 4:HT // 2],
                              ep_pk[0, :, :, HT // 4:HT // 2])
            nc.sync.dma_start(epP[:, :, HT // 2:], ep_pk[0, :, :, HT // 2:])

            # ---- episodic recency/importance weights (off-PE, overlaps ekT)
            def rec_weight(imp_ap, ts_ap, shape, tagb):
                """(1+imp)*exp(-|1-ts|*RECENCY) elementwise; returns tile."""
                impt = rows.tile(shape, F32, tag=tagb + "i", name="impt")
                tst = rows.tile(shape, F32, tag=tagb + "t", name="tst")
                nc.sync.dma_start(impt[:shape[0], :], imp_ap)
                nc.sync.dma_start(tst[:shape[0], :], ts_ap)
                s = tst[:shape[0], :]
                nc.scalar.activation(s, s, AF.Copy, bias=0.0, scale=-1.0)
                nc.vector.tensor_scalar_add(s, s, 1.0)
                nc.scalar.activation(s, s, AF.Abs)
                nc.scalar.activation(s, s, AF.Exp, scale=-RECENCY)
                si = impt[:shape[0], :]
                nc.vector.tensor_scalar_add(si, si, 1.0)
                nc.vector.tensor_mul(si, si, s)
                return impt

            wfull = rec_weight(ep_imp.rearrange("(p c) -> p c", p=P),
                               ep_ts.rearrange("(p c) -> p c", p=P),
                               [P, N // P], "wf")
            wpart = rows.tile([P, 1], F32, tag="wpart", name="wpart")
            nc.vector.reduce_sum(wpart[:, :], wfull[:, :], axis=AXL.X)
            pssum = ps_sml.tile([1, 512], F32, tag="sml", name="wsps")
            nc.tensor.matmul(pssum[:1, :1], ones_col[:], wpart[:, :],
                             start=True, stop=True)
            wsum = rows.tile([1, 1], F32, tag="wsum", name="wsum")
            nc.vector.tensor_copy(wsum[:1, :], pssum[:1, :1])
            nc.vector.tensor_scalar_add(wsum[:1, :], wsum[:1, :], 1e-8)
            nc.vector.reciprocal(wsum[:1, :], wsum[:1, :])
            wloc = rec_weight(ep_imp_s[None, :], ep_ts_s[None, :], [1, NL], "wl")
            nc.vector.tensor_scalar(wloc[:1, :], wloc[:1, :], wsum[:1, :1], None,
                                    op0=ALU.mult)

            # ---- episodic keys: project (norm fused), scale, store, AG;
            # skT0's load rides behind the early ek weight columns.
            ekT, psn_ek = project3(epP, wek_pk, "ekT", "kT", "f32",
                                   mid_emit=lambda p: load_sk(0, p),
                                   first_wc=wc_ek0)
            # ---- e_vals natural layout [NL, H]; bf16 single term
            for jc in range(H // 512):
                psts = [ps_mm.tile([P, 512], F32, tag="mm", name=f"evps{i}")
                        for i in range(NL // P)]
                for h2 in range(HT // 2):
                    wt16 = wtp.tile([P, 2, 512], BF16, tag="wt16", name="wt16",
                                    bufs=2)
                    nc.sync.dma_start(
                        wt16[:],
                        wev16[h2 * 256:(h2 + 1) * 256,
                              jc * 512:(jc + 1) * 512].rearrange(
                                  "(q p) c -> p q c", p=P))
                    for q2 in range(2):
                        hi = h2 * 2 + q2
                        for nt in range(NL // P):
                            ns = slice(nt * P, (nt + 1) * P)
                            nc.tensor.matmul(
                                psts[nt][:], epP[:, 0, hi, ns], wt16[:, q2],
                                start=(hi == 0), stop=(hi == HT - 1))
                for nt in range(NL // P):
                    evs = s512p.tile([P, 512], BF16, tag="evo16", name="evout", bufs=1)
                    nc.vector.tensor_copy(evs[:], psts[nt][:])
                    nc.scalar.dma_start(
                        ag_ev_in[nt * P:(nt + 1) * P, jc * 512:(jc + 1) * 512],
                        evs[:])

            inv_ek = norm_row_finish(psn_ek, extra_row=wloc[:1, :])
            scale_cols(ekT, inv_ek)
            store_pair_to_ag(ekT, ag_nek_in)
            nc.gpsimd.collective_compute(
                "AllGather", ALU.bypass,
                replica_groups=[list(range(NCORES))],
                ins=[ag_nek_in.opt()], outs=[ag_nek_out.opt()])
            nc.gpsimd.collective_compute(
                "AllGather", ALU.bypass,
                replica_groups=[list(range(NCORES))],
                ins=[ag_ev_in.opt()], outs=[ag_ev_out.opt()])

            # ---- semantic keys: 4 chunks of 512 (loads via mid_emit hooks)
            wc0_box = {"wc": None}
            for mc in range(ML // 512):
                nxt = (lambda p, m=mc + 1: load_sk(m, p)) \
                    if mc + 1 < ML // 512 else load_qTin
                ksT, psn_ks = project3(skPs[mc], wsk_pk, f"ksT{mc}", "kT",
                                       "f32", mid_emit=nxt,
                                       first_wc=wc0_box["wc"])
                # prefetch the next projection's first weight column ahead
                # of this chunk's norm/scale/store emission
                wc0_box["wc"] = wcol_pair(
                    wsk_pk if mc + 1 < ML // 512 else wq_pk, 0)
                inv_ks = norm_row_finish(psn_ks)
                scale_cols(ksT, inv_ks)
                store_pair_to_ag(ksT, ag_nks_in[mc])
                nc.gpsimd.collective_compute(
                    "AllGather", ALU.bypass,
                    replica_groups=[list(range(NCORES))],
                    ins=[ag_nks_in[mc].opt()], outs=[ag_nks_out[mc].opt()])

            # ===================================================================
            # Phase Q: query-side projections (norms fused)
            # ===================================================================
            qTinP = qTin_box[0]
            qTp, psn_q = project3(qTinP, wq_pk, "qT", "kT", "pair",
                                  first_wc=wc0_box["wc"])
            wc0_qs = wcol_pair(wsq_pk, 0)
            inv_q = norm_row_finish(psn_q)
            qsP, psn_qs = project3(qTp, wsq_pk, "qsT", "xTin", "pair",
                                   first_wc=wc0_qs)
            inv_qs = norm_row_finish(psn_qs)

            # transpose inv rows -> per-partition [128, NBT] via DRAM bounce
            invq_p = cst.tile([P, NBT], F32, name="invq_p")
            invqs_p = cst.tile([P, NBT], F32, name="invqs_p")
            bounce = dram.tile([2, BL], F32, name="bounce")
            nc.sync.dma_start(bounce[0:1, :], inv_q[:1, :])
            nc.sync.dma_start(bounce[1:2, :], inv_qs[:1, :])
            nc.sync.dma_start(
                invq_p[:, :], bounce[0:1, :].rearrange("o (t p) -> (o p) t", p=P))
            nc.sync.dma_start(
                invqs_p[:, :], bounce[1:2, :].rearrange("o (t p) -> (o p) t", p=P))

            def bcast_row(dram_row, width, pool, tag, name, dt=F32):
                row = rows.tile([1, width], F32, tag="crow", name="crow", bufs=1)
                nc.sync.dma_start(row[:1, :], dram_row)
                src = row[:1, :]
                if dt != F32:
                    row16 = rows.tile([1, width], dt, tag="crow16",
                                      name="crow16", bufs=1)
                    nc.scalar.activation(row16[:1, :], row[:1, :], AF.Copy)
                    src = row16[:1, :]
                t = pool.tile([P, width], dt, tag=tag, name=name, bufs=1)
                nc.gpsimd.partition_broadcast(t[:, :], src)
                return t

            b1bc = bcast_row(gate_b1[None, :], 64, cst, "b1bc", "b1bc")
            b2bc = bcast_row(gate_b2[None, :], 3, cst, "b2bc", "b2bc")
            # gate+work concatenated projection weights (bf16 pair); rides the
            # wcol ring slot freed after the last qs weight column.
            gwk = wcolp.tile([P, 2, HT, 128], BF16, tag="wcp", name="gwk")
            nc.sync.dma_start(gwk[:], gwk_pk)
            gw2 = cst.tile([64, 3], F32, name="gw2")
            nc.sync.dma_start(gw2[:, :], gate_W2)

            inv_sqrt_h = 1.0 / math.sqrt(H)
            ewT_pre = [None] * NBT
            gw_pre = [None] * NBT

            gate_st = [None] * NBT

            def emit_gate_a(bt):
                """Gate/work stage A: fused matmul (cols 0:64 gate hidden,
                64:128 work logits; stationary q bf16-hi, moving bf16 pair of
                hstack(gate_W1, work_slots.T)) + the off-PE softmax chain."""
                psg = ps_sml.tile([P, 128], F32, tag="sml", name="psg")
                bs = slice(bt * P, (bt + 1) * P)
                for hi in range(HT):
                    nc.tensor.matmul(
                        psg[:, :], qTp[:, 0, hi, bs], gwk[:, 0, hi, :],
                        start=(hi == 0), stop=False)
                    nc.tensor.matmul(
                        psg[:, :], qTp[:, 0, hi, bs], gwk[:, 1, hi, :],
                        start=False, stop=(hi == HT - 1))
                hid = tiny.tile([P, 64], F32, tag="c64", name="hid")
                nc.vector.tensor_add(hid[:, :], psg[:, :64], b1bc[:, :])
                nc.scalar.activation(hid[:, :], hid[:, :], AF.Silu)
                wmax = tiny.tile([P, 1], F32, tag="c1", name="wmax")
                nc.vector.reduce_max(wmax[:, :], psg[:, 64:], axis=AXL.X)
                nc.vector.tensor_scalar_mul(wmax[:, :], wmax[:, :], -inv_sqrt_h)
                ew = tiny.tile([P, S], F32, tag="cew", name="ew")
                nc.scalar.activation(ew[:, :], psg[:, 64:], AF.Exp,
                                     bias=wmax[:, :1], scale=inv_sqrt_h)
                zw = tiny.tile([P, 1], F32, tag="czw", name="zw")
                nc.vector.reduce_sum(zw[:, :], ew[:, :], axis=AXL.X)
                nc.vector.reciprocal(zw[:, :], zw[:, :])
                gate_st[bt] = (hid, ew, zw)

            def emit_gate_b(bt):
                """Gate/work stage B: transposes + gate MLP tail; its PE ops
                depend only on stage-A results finished a sim chunk ago."""
                hid, ew, zw = gate_st[bt]
                psht = ps_sml.tile([64, P], F32, tag="sml", name="hidtp")
                nc.tensor.transpose(out=psht[:64, :], in_=hid[:, :],
                                    identity=ident[:])
                hidT = tiny.tile([64, P], F32, tag="c128", name="hidT")
                nc.vector.tensor_copy(hidT[:, :], psht[:64, :])
                psg2 = ps_sml.tile([P, 3], F32, tag="sml", name="psg2")
                nc.tensor.matmul(psg2[:, :3], hidT[:, :], gw2[:, :],
                                 start=True, stop=True)
                gl = cst.tile([P, 3], F32, name=f"gl{bt}")
                nc.vector.tensor_add(gl[:, :], psg2[:, :3], b2bc[:, :])
                gmax = tiny.tile([P, 1], F32, tag="c1", name="gmax")
                nc.vector.reduce_max(gmax[:, :], gl[:, :], axis=AXL.X)
                nc.vector.tensor_scalar_mul(gmax[:, :], gmax[:, :], -1.0)
                nc.scalar.activation(gl[:, :], gl[:, :], AF.Exp, bias=gmax[:, :1])
                gz = tiny.tile([P, 1], F32, tag="c1", name="gz")
                nc.vector.reduce_sum(gz[:, :], gl[:, :], axis=AXL.X)
                nc.vector.reciprocal(gz[:, :], gz[:, :])
                nc.vector.tensor_scalar(gl[:, :], gl[:, :], gz[:, :1], None,
                                        op0=ALU.mult)
                gw_pre[bt] = gl
                # fold softmax normalization AND gate weight 0 into ew
                nc.vector.tensor_tensor(out=zw[:, :], in0=zw[:, :],
                                        in1=gl[:, 0:1], op=ALU.mult)
                nc.vector.tensor_scalar(ew[:, :], ew[:, :], zw[:, :1], None,
                                        op0=ALU.mult)

            def emit_gate_c(bt):
                """Gate/work stage C: transpose of the folded work probs."""
                _, ew, _ = gate_st[bt]
                pset = ps_sml.tile([S, P], F32, tag="sml", name="ewtp")
                nc.tensor.transpose(out=pset[:S, :], in_=ew[:, :],
                                    identity=ident[:])
                ewT = cst.tile([S, P], BF16, name=f"ewT{bt}")
                nc.vector.tensor_copy(ewT[:, :], pset[:S, :])
                ewT_pre[bt] = ewT

            # ===================================================================
            # Phase S: similarity + per-chunk top-8 candidates
            # ===================================================================
            cand_v_e = [big.tile([P, (N // 512) * 8], F32, tag=f"cve{bt}",
                                 name=f"cve{bt}") for bt in range(NBT)]
            cand_i_e = [big.tile([P, (N // 512) * 8], F32, tag=f"cie{bt}",
                                 name=f"cie{bt}") for bt in range(NBT)]
            cand_v_s = [big.tile([P, (M // 512) * 8], F32, tag=f"cvs{bt}",
                                 name=f"cvs{bt}") for bt in range(NBT)]
            cand_i_s = [big.tile([P, (M // 512) * 8], F32, tag=f"cis{bt}",
                                 name=f"cis{bt}") for bt in range(NBT)]

            def sim_chunk(xP, kd, r, ch, cand_v, cand_i, base):
                """sims of all 4 b-tiles vs bf16-pair keys kd[r, :, h, :]."""
                psts = [ps_mm.tile([P, 512], F32, tag="mm", name=f"simps{i}")
                        for i in range(NBT)]
                for hi in range(HT):
                    kth = s512p.tile([P, 512], BF16, tag="st16h", name="kth",
                                     bufs=4)
                    ktl = s512p.tile([P, 512], BF16, tag="st16l", name="ktl",
                                     bufs=4)
                    nc.sync.dma_start(
                        kth[:], kd[r, 0, hi * P:(hi + 1) * P, :])
                    nc.sync.dma_start(
                        ktl[:], kd[r, 1, hi * P:(hi + 1) * P, :])
                    for bt in range(NBT):
                        bs = slice(bt * P, (bt + 1) * P)
                        nc.tensor.matmul(
                            psts[bt][:], xP[:, 0, hi, bs], kth[:],
                            start=(hi == 0), stop=False)
                        nc.tensor.matmul(
                            psts[bt][:], xP[:, 0, hi, bs], ktl[:],
                            start=False, stop=False)
                        nc.tensor.matmul(
                            psts[bt][:], xP[:, 1, hi, bs], kth[:],
                            start=False, stop=(hi == HT - 1))
                for bt in range(NBT):
                    sc = simb.tile([P, 512], F32, tag="simc", name="simc",
                                   bufs=2)
                    nc.scalar.activation(sc[:], psts[bt][:], AF.Copy)
                    mx = simb.tile([P, 8], F32, tag="mx", name="mx")
                    mi = simb.tile([P, 8], U32, tag="mi", name="mi")
                    nc.vector.max(out=mx[:], in_=sc[:])
                    nc.vector.max_index(out=mi[:], in_max=mx[:], in_values=sc[:])
                    nc.vector.tensor_copy(cand_v[bt][:, ch * 8:(ch + 1) * 8],
                                          mx[:])
                    mif = simb.tile([P, 8], F32, tag="mif", name="mif")
                    nc.vector.tensor_copy(mif[:], mi[:])
                    nc.vector.tensor_scalar_add(
                        cand_i[bt][:, ch * 8:(ch + 1) * 8], mif[:],
                        float(base))

            def topk_attend(cand_v, cand_i, k, inv_p, bt, vals_dram, gscale,
                            acc_tag, bufs=2):
                """Merged top-k -> softmax (x gscale) -> gather + weighted sum."""
                top8 = tiny.tile([P, 8], F32, tag="c8", name="top8")
                nc.vector.max(out=top8[:], in_=cand_v[:])
                idxf = tiny.tile([P, 8], F32, tag="c8", name="idxf")
                eqm = s512p.tile([P, 256], F32, tag="sqn", name="eqm")
                for kk in range(k):
                    w = cand_v.shape[-1]
                    nc.vector.tensor_scalar(
                        eqm[:, :w], cand_v[:], top8[:, kk:kk + 1], None,
                        op0=ALU.is_equal)
                    nc.vector.tensor_tensor(
                        out=eqm[:, :w], in0=eqm[:, :w], in1=cand_i[:], op=ALU.mult)
                    nc.vector.reduce_sum(idxf[:, kk:kk + 1], eqm[:, :w], axis=AXL.X)
                idxu = tiny.tile([P, 8], U32, tag="c8u", name="idxu")
                nc.vector.tensor_copy(idxu[:, :k], idxf[:, :k])
                sc8 = tiny.tile([P, 8], F32, tag="c8", name="sc8")
                nc.vector.tensor_scalar(
                    sc8[:, :k], top8[:, :k], inv_p[:, bt:bt + 1], None,
                    op0=ALU.mult)
                negm = tiny.tile([P, 1], F32, tag="c1", name="negm")
                nc.vector.tensor_scalar_mul(negm[:, :], sc8[:, 0:1], -1.0)
                nc.scalar.activation(sc8[:, :k], sc8[:, :k], AF.Exp,
                                     bias=negm[:, :1])
                zs = tiny.tile([P, 1], F32, tag="c1", name="zs")
                nc.vector.reduce_sum(zs[:, :], sc8[:, :k], axis=AXL.X)
                nc.vector.reciprocal(zs[:, :], zs[:, :])
                nc.vector.tensor_scalar(zs[:, :], zs[:, :], gscale, None,
                                        op0=ALU.mult)
                nc.vector.tensor_scalar(sc8[:, :k], sc8[:, :k], zs[:, :1], None,
                                        op0=ALU.mult)
                acc = sm.tile([P, H], BF16, tag=acc_tag, name="acc" + acc_tag,
                              bufs=bufs)
                nc.vector.memset(acc[:, :], 0.0)
                for kk in range(k):
                    g = gath.tile([P, H], BF16, tag="g", name="g")
                    nc.gpsimd.indirect_dma_start(
                        out=g[:, :], out_offset=None, in_=vals_dram,
                        in_offset=bass.IndirectOffsetOnAxis(
                            ap=idxu[:, kk:kk + 1], axis=0))
                    nc.vector.scalar_tensor_tensor(
                        out=acc[:, :], in0=g[:, :], scalar=sc8[:, kk:kk + 1],
                        in1=acc[:, :], op0=ALU.mult, op1=ALU.add)
                return acc

            def transpose_into(dst, src, dt=BF16):
                """dst [P, HT, P] view <- transpose of src [P, H]; psum
                evacuation alternates ACT/DVE so neither sequencer's
                per-op dispatch overhead paces the chain."""
                idn = ident if dt == F32 else ident16
                for hi in range(HT):
                    pst = ps_mm.tile([P, P], dt, tag="mm", name="trf")
                    nc.tensor.transpose(out=pst[:], in_=src[:, hi * P:(hi + 1) * P],
                                        identity=idn[:])
                    nc.scalar.activation(dst[:, hi, :], pst[:], AF.Copy)

            # episodic sims: one gathered buffer, rank-major global indices;
            # gate/work stages (off-PE-latency-heavy) interleave with chunks
            # so each stage's PE ops only see dependencies already finished.
            gbc2 = [None] * (H // 512)
            bbc2 = [None] * (H // 512)
            for ch in range(N // 512):
                if 2 <= ch <= NBT + 1:
                    emit_gate_c(ch - 2)
                if 1 <= ch <= NBT:
                    emit_gate_b(ch - 1)
                if ch < NBT:
                    emit_gate_a(ch)
                if ch == 4:
                    # LN gamma/beta broadcast tiles (full row, loaded in 512
                    # chunks); the serial DMA<->POOL ping-pong hides under
                    # the remaining sim chunks.
                    gbc2[0] = sm.tile([P, H], BF16, tag="gbc", name="gbc",
                                      bufs=1)
                    bbc2[0] = sm.tile([P, H], BF16, tag="bbc", name="bbc",
                                      bufs=1)
                    for t, dsrc in ((gbc2[0], ln_gamma), (bbc2[0], ln_beta)):
                        for jq in range(H // 512):
                            cq = slice(jq * 512, (jq + 1) * 512)
                            row = rows.tile([1, 512], F32, tag="crow",
                                            name="crow", bufs=1)
                            nc.sync.dma_start(row[:1, :], dsrc[None, cq])
                            row16 = rows.tile([1, 512], BF16, tag="crow16",
                                              name="crow16", bufs=1)
                            nc.scalar.activation(row16[:1, :], row[:1, :],
                                                 AF.Copy)
                            nc.gpsimd.partition_broadcast(t[:, cq],
                                                          row16[:1, :])
                sim_chunk(qTp, ag_nek_out, ch, ch, cand_v_e, cand_i_e,
                          ch * 512)

            # episodic merges (DVE/gathers) overlap semantic sims (PE); the
            # accT_e transposes are emitted after a sem chunk each so the PE
            # queue never waits on a merge.
            accT_e = big.tile([P, NBT, HT, P], BF16, tag="kT", name="accTe")
            accT_s = big.tile([P, NBT, HT, P], BF16, tag="xTin", name="accTs")
            acc_e = [None] * NBT
            acc_s = [None] * NBT

            def emit_merge_e(bt):
                acc_e[bt] = topk_attend(cand_v_e[bt][:], cand_i_e[bt][:], EP_K,
                                        invq_p, bt, ag_ev_out[:, :],
                                        gw_pre[bt][:, 1:2], "sl1")

            def emit_merge_s(bt):
                acc_s[bt] = topk_attend(cand_v_s[bt][:], cand_i_s[bt][:], SEM_K,
                                        invqs_p, bt, semv16, gw_pre[bt][:, 2:3],
                                        "sl2", bufs=3)

            sem_seq = [(i, r) for i in range(ML // 512) for r in range(NCORES)]

            def emit_sem_chunk(ch):
                i, r = sem_seq[ch]
                sim_chunk(qsP, ag_nks_out[i], r, ch, cand_v_s, cand_i_s,
                          r * ML + i * 512)

            emit_merge_e(0)
            emit_merge_e(1)
            p2a_pre = {}
            for ch in range(len(sem_seq)):
                if ch == len(sem_seq) - 2:
                    # prefetch Pass 2a's first moving tiles so its opening
                    # matmuls don't wait on the DMA queue draining
                    wsn0 = s512p.tile([S, 512], BF16, tag="s512", name="wsn2")
                    nc.sync.dma_start(wsn0[:S, :], ws16[:, :512])
                    wt0 = wtp.tile([P, 4, 512], BF16, tag="wt", name="wto")
                    nc.sync.dma_start(
                        wt0[:], weo16[:512, :512].rearrange(
                            "(q p) c -> p q c", p=P))
                    p2a_pre["wsn"] = wsn0
                    p2a_pre["wt"] = wt0
                emit_sem_chunk(ch)
                if ch < NBT:
                    transpose_into(accT_e[:, ch], acc_e[ch])
                    if ch + 2 < NBT:
                        emit_merge_e(ch + 2)

            # ===================================================================
            # Phase F: blend + output projections + streaming layernorm
            # ===================================================================
            bl_all = big.tile([P, NBT, H], BF16, tag="bl", name="bl_all")

            emit_merge_s(0)
            emit_merge_s(1)
            emit_merge_s(2)
            # Pass 2a: bl = gate0*w_out + acc_e @ W_eo (jc-major, weights read
            # once); ACT evacuates so DVE stays free for the semantic merges,
            # which run concurrently on DVE. The accT_s transposes interleave
            # between jc blocks: Ts_k lands right after merge k finishes, and
            # releasing acc_s[0] lets merge 3's ring slot allocate.
            for jc in range(H // 512):
                cs = slice(jc * 512, (jc + 1) * 512)
                if jc == 0:
                    wsn = p2a_pre["wsn"]
                else:
                    wsn = s512p.tile([S, 512], BF16, tag="s512", name="wsn2")
                    nc.sync.dma_start(wsn[:S, :], ws16[:, cs])
                psos = [ps_mm.tile([P, 512], F32, tag="mm", name=f"pso{i}")
                        for i in range(NBT)]
                for bt in range(NBT):
                    nc.tensor.matmul(psos[bt][:], ewT_pre[bt][:, :],
                                     wsn[:S, :], start=True, stop=False)
                for hq in range(HT // 4):
                    if jc == 0 and hq == 0:
                        wt = p2a_pre["wt"]
                    else:
                        wt = wtp.tile([P, 4, 512], BF16, tag="wt", name="wto")
                        nc.sync.dma_start(
                            wt[:], weo16[hq * 512:(hq + 1) * 512, cs].rearrange(
                                "(q p) c -> p q c", p=P))
                    for q4 in range(4):
                        hi = hq * 4 + q4
                        for bt in range(NBT):
                            nc.tensor.matmul(
                                psos[bt][:], accT_e[:, bt, hi, :], wt[:, q4],
                                start=False, stop=(hi == HT - 1))
                for bt in range(NBT):
                    nc.scalar.activation(bl_all[:, bt, cs], psos[bt][:], AF.Copy)
                if 1 <= jc:
                    transpose_into(accT_s[:, jc - 1], acc_s[jc - 1])
                    if jc == H // 512 - 1:
                        transpose_into(accT_s[:, jc], acc_s[jc])
                if jc == 0:
                    emit_merge_s(3)

            blT = big.tile([P, NBT, HT, P], BF16, tag="kT", name="blT")
            # Pass 2b: bl += acc_s @ W_so (jc-major, weights read once)
            for jc in range(H // 512):
                cs = slice(jc * 512, (jc + 1) * 512)
                psob = [ps_mm.tile([P, 512], F32, tag="mm", name=f"psob{i}")
                        for i in range(NBT)]
                for hq in range(HT // 4):
                    wt = wtp.tile([P, 4, 512], BF16, tag="wt", name="wtob")
                    nc.sync.dma_start(
                        wt[:], wso16[hq * 512:(hq + 1) * 512, cs].rearrange(
                            "(q p) c -> p q c", p=P))
                    for q4 in range(4):
                        hi = hq * 4 + q4
                        for bt in range(NBT):
                            nc.tensor.matmul(
                                psob[bt][:], accT_s[:, bt, hi, :], wt[:, q4],
                                start=(hi == 0), stop=(hi == HT - 1))
                for bt in range(NBT):
                    nc.vector.tensor_add(bl_all[:, bt, cs],
                                         bl_all[:, bt, cs], psob[bt][:])
                    if jc == H // 512 - 1:
                        # bl_all[bt] now complete: start its transpose DMA
                        nc.sync.dma_start_transpose(blT[:, bt],
                                                    bl_all[:, bt, :])

            # Pass 3: xo = bl @ W_ro (jc-major) with streamed LN stats
            xo_all = big.tile([P, NBT, H], BF16, tag="xTin", name="xo_all")
            msum = [tiny.tile([P, 4], F32, tag=f"cms{i}", name=f"msum{i}",
                              bufs=1) for i in range(NBT)]
            vsum = [tiny.tile([P, 4], F32, tag=f"cvs{i}", name=f"vsum{i}",
                              bufs=1) for i in range(NBT)]
            ln_stats = [None] * NBT

            def emit_ln_stats(bt):
                """inv-std and -mu*inv-std per-partition scalars for one bt."""
                mu = tiny.tile([P, 1], F32, tag="c1", name="mu")
                nc.vector.reduce_sum(mu[:, :], msum[bt][:, :], axis=AXL.X)
                nc.vector.tensor_scalar_mul(mu[:, :], mu[:, :], 1.0 / H)
                vs = tiny.tile([P, 1], F32, tag=f"cvv{bt}", name="vs", bufs=1)
                nc.vector.reduce_sum(vs[:, :], vsum[bt][:, :], axis=AXL.X)
                nc.vector.tensor_scalar_mul(vs[:, :], vs[:, :], 1.0 / H)
                mu2 = tiny.tile([P, 1], F32, tag="c1", name="mu2")
                nc.vector.tensor_tensor(out=mu2[:, :], in0=mu[:, :],
                                        in1=mu[:, :], op=ALU.mult)
                nc.vector.tensor_tensor(out=vs[:, :], in0=vs[:, :],
                                        in1=mu2[:, :], op=ALU.subtract)
                nc.vector.tensor_scalar_add(vs[:, :], vs[:, :], LN_EPS)
                nc.scalar.sqrt(vs[:, :], vs[:, :])
                nc.vector.reciprocal(vs[:, :], vs[:, :])
                nmu = tiny.tile([P, 1], F32, tag=f"cnm{bt}", name="nmu",
                                bufs=1)
                nc.vector.tensor_tensor(out=nmu[:, :], in0=mu[:, :],
                                        in1=vs[:, :], op=ALU.mult)
                nc.vector.tensor_scalar_mul(nmu[:, :], nmu[:, :], -1.0)
                ln_stats[bt] = (vs, nmu)
            def emit_ln_final(bt):
                """Normalize + affine + store for one b-tile: full-row bf16
                DVE ops (2x throughput, minimal dispatch count) on the gather
                ring, which is idle by this point."""
                vs, nmu = ln_stats[bt]
                on16 = gath.tile([P, H], BF16, tag="g", name="on16")
                nc.vector.tensor_scalar(on16[:, :], xo_all[:, bt, :],
                                        vs[:, :1], nmu[:, :1],
                                        op0=ALU.mult, op1=ALU.add)
                nc.vector.tensor_mul(on16[:, :], on16[:, :], gbc2[0][:, :])
                on = gath.tile([P, H], BF16, tag="g", name="on")
                nc.vector.tensor_add(on[:, :], on16[:, :], bbc2[0][:, :])
                nc.sync.dma_start(out_s[bt * P:(bt + 1) * P, :], on[:])

            for jc in range(H // 512):
                cs = slice(jc * 512, (jc + 1) * 512)
                psro = [ps_mm.tile([P, 512], F32, tag="mm", name=f"psro{i}")
                        for i in range(NBT)]
                for hq in range(HT // 4):
                    wt = wtp.tile([P, 4, 512], BF16, tag="wt", name="wtro")
                    nc.sync.dma_start(
                        wt[:], wro16[hq * 512:(hq + 1) * 512, cs].rearrange(
                            "(q p) c -> p q c", p=P))
                    for q4 in range(4):
                        hi = hq * 4 + q4
                        for bt in range(NBT):
                            nc.tensor.matmul(
                                psro[bt][:], blT[:, bt, hi, :], wt[:, q4],
                                start=(hi == 0), stop=(hi == HT - 1))
                for bt in range(NBT):
                    nc.scalar.activation(xo_all[:, bt, cs], psro[bt][:],
                                         AF.Copy,
                                         accum_out=msum[bt][:, jc:jc + 1])
                    sqc = s512p.tile([P, 512], F32, tag="sqn", name="sqc")
                    nc.scalar.activation(sqc[:, :], psro[bt][:], AF.Square,
                                         accum_out=vsum[bt][:, jc:jc + 1])
                    if jc == H // 512 - 1:
                        emit_ln_stats(bt)
            for bt in range(NBT):
                emit_ln_final(bt)

    nc.finalize()
    return nc


_NC_CACHE = None
LAST_EXEC_NS = None


def _pack_xpair(x):
    """[R,H] f32 -> [R//512, P, 2, HT, 512] bf16 pair, pre-transposed to
    the on-chip tile layout: pk[ch, p, half, hi, r] = split(x)[half][
    ch*512+r, hi*128+p]."""
    hi_, lo_ = _split_bf16(x)
    def lay(a):
        return a.reshape(-1, HT, P).transpose(2, 1, 0)   # [P, HT, R]
    pk = np.stack([lay(hi_), lay(lo_)], axis=1)          # [P, 2, HT, R]
    R = x.shape[0]
    return np.ascontiguousarray(
        np.stack([pk[..., i * 512:(i + 1) * 512]
                  for i in range(R // 512)], axis=0))


def _pack_wpair(w):
    """[H,H] f32 -> [HT, P, 2, HT, P] bf16 pair in wcP tile layout:
    packed[j, p, half, hi, c] = split(W)[half][hi*128+p, j*128+c]."""
    hi_, lo_ = _split_bf16(w)
    def lay(a):
        # [hi, p, j, c] -> [j, p, hi, c]
        return np.ascontiguousarray(
            a.reshape(HT, P, HT, P).transpose(2, 1, 0, 3))
    return np.ascontiguousarray(
        np.stack([lay(hi_), lay(lo_)], axis=2))


def _pack_gwk(gate_W1, work_slots):
    """hstack(gate_W1 [H,64], work_slots.T [H,64]) -> [P, 2, HT, 128] pair:
    pk[p, half, hi, c] = split(gw)[half][hi*128+p, c]."""
    gw = np.hstack([np.asarray(gate_W1, np.float32),
                    np.ascontiguousarray(np.asarray(work_slots, np.float32).T)])
    hi_, lo_ = _split_bf16(gw)
    def lay(a):
        return a.reshape(HT, P, 128).transpose(1, 0, 2)   # [P, HT, 128]
    return np.ascontiguousarray(np.stack([lay(hi_), lay(lo_)], axis=1))


def _split_bf16(x):
    """two-term bf16 decomposition: x ~= hi + lo to ~16 mantissa bits."""
    import ml_dtypes
    bf = ml_dtypes.bfloat16
    x = np.ascontiguousarray(np.asarray(x), dtype=np.float32)
    hi = x.astype(bf)
    lo = (x - hi.astype(np.float32)).astype(bf)
    return hi, lo


def kernel(**inputs) -> np.ndarray:
    global _NC_CACHE
    if _NC_CACHE is None:
        _NC_CACHE = build()
    nc = _NC_CACHE

    def arr(x):
        return np.ascontiguousarray(np.asarray(x), dtype=np.float32)

    wq_pk = _pack_wpair(inputs["W_query"])
    wek_pk = _pack_wpair(inputs["W_ek"])
    wsq_pk = _pack_wpair(inputs["W_sq"])
    wsk_pk = _pack_wpair(inputs["W_sk"])
    wev16, _ = _split_bf16(inputs["W_ev"])
    weo16, _ = _split_bf16(inputs["W_eo"])
    wso16, _ = _split_bf16(inputs["W_so"])
    wro16, _ = _split_bf16(inputs["W_ro"])
    semv16, _ = _split_bf16(inputs["sem_values"])
    ws16, _ = _split_bf16(inputs["work_slots"])
    gwk_pk = _pack_gwk(inputs["gate_W1"], inputs["work_slots"])

    in_maps = []
    for c in range(NCORES):
        in_maps.append({
            "query_pk": _pack_xpair(inputs["query"][c * BL:(c + 1) * BL]),
            "ep_pk": _pack_xpair(inputs["ep_store"][c * NL:(c + 1) * NL]),
            "semk_pk": _pack_xpair(inputs["sem_keys"][c * ML:(c + 1) * ML]),
            "ep_imp_s": arr(inputs["ep_importance"][c * NL:(c + 1) * NL]),
            "ep_ts_s": arr(inputs["ep_timestamps"][c * NL:(c + 1) * NL]),
            "ep_imp": arr(inputs["ep_importance"]),
            "ep_ts": arr(inputs["ep_timestamps"]),
            "semv16": semv16,
            "wq_pk": wq_pk,
            "wek_pk": wek_pk,
            "wsq_pk": wsq_pk,
            "wsk_pk": wsk_pk,
            "wev16": wev16,
            "weo16": weo16,
            "wso16": wso16,
            "wro16": wro16,
            "ws16": ws16,
            "gwk_pk": gwk_pk,
            "gate_b1": arr(inputs["gate_b1"]),
            "gate_W2": arr(inputs["gate_W2"]),
            "gate_b2": arr(inputs["gate_b2"]),
            "ln_gamma": arr(inputs["ln_gamma"]),
            "ln_beta": arr(inputs["ln_beta"]),
        })
    res = run_bass_kernel_spmd(nc, in_maps, core_ids=list(range(NCORES)))
    return np.concatenate(
        [np.asarray(res.results[c]["out_s"], dtype=np.float32)
         for c in range(NCORES)], axis=0)


# revision 77
# speedup vs baseline: 1.0504x; 1.0004x over previous
"""ONIMemoryHub kernel for 8 Trainium2 NeuronCores (Bass/Tile).

Sharding: data-parallel over batch for the query side; episodic store and
semantic memory sharded across cores for the key/value projections, with
AllGathers of the projected (normalized, pre-scaled) keys/values.

Schedule notes (v2): the PE instruction stream is kept free of stalls by
emitting off-engine work (top-k merges, norms, layernorm) interleaved
between matmul blocks whose inputs are already resident:
  - projection column norms are fused into the projection evacuation
  - gate/work blocks interleave with the episodic sim chunks
  - episodic merges/transposes interleave with the semantic sim chunks
  - semantic merges interleave with the W_eo output pass
  - layernorm stats stream out of the W_ro pass via ACT accumulators and
    the finals run as full-row bf16 DVE ops, shrinking the kernel tail
  - blend transposes go through the DMA crossbar, store-side DMAs ride
    the ACT queue, and the Pool queue is kept clear of the collectives'
    head-of-line blocking.

kernel(**inputs) takes FULL inputs (as produced by reference.setup_inputs())
and returns the FULL [4096, 2048] output.
"""
import math

import numpy as np

import concourse.bass as bass
import concourse.mybir as mybir
import concourse.tile as tile
from concourse import bacc
from concourse.bass_utils import run_bass_kernel_spmd
from concourse.masks import make_identity

AF = mybir.ActivationFunctionType
AXL = mybir.AxisListType
ALU = mybir.AluOpType

NCORES = 8
B, H, N, M, S = 4096, 2048, 4096, 16384, 64
BL, NL, ML = B // NCORES, N // NCORES, M // NCORES   # 512, 512, 2048
HT = H // 128                                        # 16 h-tiles
P = 128
NBT = BL // P                                        # 4 b-tiles
EP_K = 8
SEM_K = 4
LN_EPS = 1e-5
RECENCY = 0.01   # 1 - RECENCY_DECAY

F32 = mybir.dt.float32
BF16 = mybir.dt.bfloat16
U32 = mybir.dt.uint32


def build():
    nc = bacc.Bacc("TRN2", target_bir_lowering=False, debug=False,
                   num_devices=NCORES)

    def din(name, shape, dt=F32):
        return nc.dram_tensor(name, shape, dt, kind="ExternalInput").ap()

    # per-core slices: host-split bf16 pairs, pre-transposed to tile layout
    query_pk = din("query_pk", [1, P, 2, HT, 512], BF16)
    ep_pk = din("ep_pk", [1, P, 2, HT, 512], BF16)
    semk_pk = din("semk_pk", [ML // 512, P, 2, HT, 512], BF16)
    ep_imp_s = din("ep_imp_s", [NL])
    ep_ts_s = din("ep_ts_s", [NL])
    # replicated
    ep_imp = din("ep_imp", [N])
    ep_ts = din("ep_ts", [N])
    semv16 = din("semv16", [M, H], BF16)
    wq_pk = din("wq_pk", [HT, P, 2, HT, P], BF16)
    wek_pk = din("wek_pk", [HT, P, 2, HT, P], BF16)
    wsq_pk = din("wsq_pk", [HT, P, 2, HT, P], BF16)
    wsk_pk = din("wsk_pk", [HT, P, 2, HT, P], BF16)
    wev16 = din("wev16", [H, H], BF16)
    weo16 = din("weo16", [H, H], BF16)
    wso16 = din("wso16", [H, H], BF16)
    wro16 = din("wro16", [H, H], BF16)
    ws16 = din("ws16", [S, H], BF16)
    gwk_pk = din("gwk_pk", [P, 2, HT, 128], BF16)
    gate_b1 = din("gate_b1", [64])
    gate_W2 = din("gate_W2", [64, 3])
    gate_b2 = din("gate_b2", [3])
    ln_gamma = din("ln_gamma", [H])
    ln_beta = din("ln_beta", [H])

    out_s = nc.dram_tensor("out_s", [BL, H], BF16, kind="ExternalOutput").ap()

    with tile.TileContext(nc) as tc:
        with (
            tc.tile_pool(name="cst", bufs=1) as cst,
            tc.tile_pool(name="big", bufs=1) as big,
            tc.tile_pool(name="rows", bufs=1) as rows,
            tc.tile_pool(name="s512", bufs=2) as s512p,
            tc.tile_pool(name="wcol", bufs=2) as wcolp,
            tc.tile_pool(name="wtile", bufs=2) as wtp,
            tc.tile_pool(name="sm", bufs=2) as sm,
            tc.tile_pool(name="tiny", bufs=2) as tiny,
            tc.tile_pool(name="simb", bufs=2) as simb,
            tc.tile_pool(name="gath", bufs=2) as gath,
            tc.tile_pool(name="ps_mm", bufs=7, space="PSUM") as ps_mm,
            tc.tile_pool(name="ps_sml", bufs=1, space="PSUM") as ps_sml,
            tc.tile_pool(name="dram", bufs=1, space="DRAM") as dram,
        ):
            ident = cst.tile([P, P], F32)
            make_identity(nc, ident[:])
            ident16 = cst.tile([P, P], BF16)
            nc.scalar.activation(ident16[:], ident[:], AF.Copy)
            ones_col = cst.tile([P, 1], F32)
            nc.vector.memset(ones_col[:], 1.0)

            # ---------- helpers ----------
            # big slot chains (explicit liveness via shared tags):
            #   xTin: epT -> skT(x4) -> qsT -> accTs
            #   kT  : ekT -> ksT(x4) -> accTe -> blT
            #   bl  : qTp -> bl_all
            def emit_split(dst_hi, dst_lo, src_f32, tmp32):
                """bf16 two-term split: hi = bf16(x), lo = bf16(x - hi).

                The upconvert copy runs on DVE, not gpsimd: the Pool queue
                carries the collectives, which would head-block a gpsimd
                copy (and everything after it) for a whole AllGather.
                """
                nc.scalar.activation(dst_hi, src_f32, AF.Copy)
                nc.vector.tensor_copy(tmp32, dst_hi)
                nc.vector.tensor_tensor(out=tmp32, in0=src_f32, in1=tmp32,
                                        op=ALU.subtract)
                nc.scalar.activation(dst_lo, tmp32, AF.Copy)

            def wcol_pair(w_pk, j):
                wcP = wcolp.tile([P, 2, HT, P], BF16, tag="wcp", name="wcp")
                nc.sync.dma_start(wcP[:], w_pk[j])
                return wcP

            def norm_row_finish(psn, extra_row=None):
                """[1,512] inv-norm row from accumulated sum-of-squares."""
                row = rows.tile([1, 512], F32, tag="nrow", name="nrow", bufs=2)
                nc.vector.tensor_copy(row[:1, :], psn[:1, :])
                nc.scalar.sqrt(row[:1, :], row[:1, :])
                nc.vector.tensor_scalar_max(row[:1, :], row[:1, :], 1e-12)
                nc.vector.reciprocal(row[:1, :], row[:1, :])
                if extra_row is not None:
                    nc.vector.tensor_mul(row[:1, :], row[:1, :], extra_row)
                return row

            def project3(xP, w_pk, name, tag, mode, mid_emit=None,
                         first_wc=None):
                """(x @ W).T via 3-term bf16 split matmuls; xP is a pair.

                mode "f32": returns (yT, psn) — f32 tile + sum-of-squares
                psum row (norm fused into the evacuation).
                mode "pair": returns (yP, psn) — bf16 pair tile + norm psum.
                mid_emit() is called after the j==3 block so a prefetch DMA
                can ride the SP queue behind the first few weight columns.
                """
                psn = ps_sml.tile([1, 512], F32, tag="sml", name="npsum")
                if mode == "f32":
                    yT = big.tile([P, HT, 512], F32, tag=tag, name=name)
                else:
                    yP = big.tile([P, 2, HT, 512], BF16, tag=tag, name=name)
                # norm accumulation runs at lag 1 so the PE never waits on the
                # ACT square of the chunk it just produced.
                sqs = [None] * HT
                for j in range(HT):
                    wcP = first_wc if (j == 0 and first_wc is not None) \
                        else wcol_pair(w_pk, j)
                    pst = ps_mm.tile([P, 512], F32, tag="mm", name="projps")
                    for hi in range(HT):
                        nc.tensor.matmul(
                            pst[:], wcP[:, 0, hi, :], xP[:, 0, hi, :],
                            start=(hi == 0), stop=False)
                        nc.tensor.matmul(
                            pst[:], wcP[:, 0, hi, :], xP[:, 1, hi, :],
                            start=False, stop=False)
                        nc.tensor.matmul(
                            pst[:], wcP[:, 1, hi, :], xP[:, 0, hi, :],
                            start=False, stop=(hi == HT - 1))
                    sq = s512p.tile([P, 512], F32, tag="sqn", name="sqn")
                    nc.scalar.square(sq[:, :], pst[:])
                    sqs[j] = sq
                    if mode == "f32":
                        nc.scalar.activation(yT[:, j, :], pst[:], AF.Copy)
                    else:
                        tmp32 = s512p.tile([P, 512], F32, tag="s512",
                                           name="spj32")
                        emit_split(yP[:, 0, j, :], yP[:, 1, j, :], pst[:],
                                   tmp32[:])
                    if j >= 1:
                        nc.tensor.matmul(psn[:1, :], ones_col[:],
                                         sqs[j - 1][:, :],
                                         start=(j == 1), stop=False)
                    if mid_emit is not None and j in (3, 6, 9, 12):
                        mid_emit((j - 3) // 3)
                nc.tensor.matmul(psn[:1, :], ones_col[:], sqs[HT - 1][:, :],
                                 start=False, stop=True)
                if mode == "f32":
                    return yT, psn
                return yP, psn

            def store_pair_to_ag(xT, ag_in):
                """split scaled f32 keys and store bf16 pair to AG input."""
                for hi in range(HT):
                    sth = s512p.tile([P, 512], BF16, tag="st16h", name="sth",
                                     bufs=5)
                    stl = s512p.tile([P, 512], BF16, tag="st16l", name="stl",
                                     bufs=5)
                    tmp32 = s512p.tile([P, 512], F32, tag="s512", name="spg32")
                    emit_split(sth[:], stl[:], xT[:, hi, :], tmp32[:])
                    # stores ride the ACT queue (which paces them via the
                    # splits), keeping the SP queue free for weight loads
                    nc.scalar.dma_start(ag_in[0, hi * P:(hi + 1) * P, :], sth[:])
                    nc.scalar.dma_start(ag_in[1, hi * P:(hi + 1) * P, :], stl[:])

            def scale_cols(xT, scale_row):
                bc = s512p.tile([P, 512], F32, tag="s512", name="bcn")
                nc.gpsimd.partition_broadcast(bc[:, :], scale_row[:1, :])
                for hi in range(HT):
                    nc.vector.tensor_mul(xT[:, hi, :], xT[:, hi, :], bc[:, :])

            # ===================================================================
            # Phase M: sharded memory-side projections + AllGathers
            # ===================================================================
            ag_nek_in = dram.tile([2, H, NL], BF16, name="ag_nek_in")
            ag_nek_out = dram.tile([NCORES, 2, H, NL], BF16,
                                   addr_space="Shared", name="ag_nek_out")
            ag_ev_in = dram.tile([NL, H], BF16, name="ag_ev_in")
            ag_ev_out = dram.tile([N, H], BF16, addr_space="Shared",
                                  name="ag_ev_out")
            ag_nks_in = [dram.tile([2, H, 512], BF16, name=f"ag_nks_in{i}")
                         for i in range(ML // 512)]
            ag_nks_out = [dram.tile([NCORES, 2, H, 512], BF16,
                                    addr_space="Shared", name=f"ag_nks_out{i}")
                          for i in range(ML // 512)]

            # semantic-key chunk loads: double-buffered on alternating big
            # slots (bl/xTin), emitted via project3 mid_emit hooks so each
            # 12.6us DMA hides under the previous projection.
            skPs = [None] * (ML // 512)
            qTin_box = [None]

            def load_sk(mc, piece):
                """quarter-piece prefetch of a semantic-key chunk."""
                if piece == 0:
                    skPs[mc] = big.tile([P, 2, HT, 512], BF16,
                                        tag="bl" if mc % 2 == 0 else "xTin",
                                        name=f"skT{mc}")
                hs = slice(piece * 4, (piece + 1) * 4)
                nc.sync.dma_start(skPs[mc][:, :, hs], semk_pk[mc, :, :, hs])

            def load_qTin(piece):
                if piece == 0:
                    qTin_box[0] = big.tile([P, 2, HT, 512], BF16, tag="bl",
                                           name="qTin")
                hs = slice(piece * 4, (piece + 1) * 4)
                nc.sync.dma_start(qTin_box[0][:, :, hs], query_pk[0, :, :, hs])

            # First weight column rides the DMA queue ahead of epP so the very
            # first matmul chain starts as soon as epP's first piece lands;
            # epP is split so early hi-tiles arrive (and compute) first.
            wc_ek0 = wcol_pair(wek_pk, 0)
            epP = big.tile([P, 2, HT, 512], BF16, tag="xTin", name="epT")
            nc.sync.dma_start(epP[:, :, :HT // 4], ep_pk[0, :, :, :HT // 4])
            nc.sync.dma_start(epP[:, :, HT // 4:HT // 2],
                              ep_pk[0, :, :, HT // 4:HT // 2])
            nc.sync.dma_start(epP[:, :, HT // 2:], ep_pk[0, :, :, HT // 2:])

            # ---- episodic recency/importance weights (off-PE, overlaps ekT)
            def rec_weight(imp_ap, ts_ap, shape, tagb):
                """(1+imp)*exp(-|1-ts|*RECENCY) elementwise; returns tile."""
                impt = rows.tile(shape, F32, tag=tagb + "i", name="impt")
                tst = rows.tile(shape, F32, tag=tagb + "t", name="tst")
                nc.sync.dma_start(impt[:shape[0], :], imp_ap)
                nc.sync.dma_start(tst[:shape[0], :], ts_ap)
                s = tst[:shape[0], :]
                nc.scalar.activation(s, s, AF.Copy, bias=0.0, scale=-1.0)
                nc.vector.tensor_scalar_add(s, s, 1.0)
                nc.scalar.activation(s, s, AF.Abs)
                nc.scalar.activation(s, s, AF.Exp, scale=-RECENCY)
                si = impt[:shape[0], :]
                nc.vector.tensor_scalar_add(si, si, 1.0)
                nc.vector.tensor_mul(si, si, s)
                return impt

            wfull = rec_weight(ep_imp.rearrange("(p c) -> p c", p=P),
                               ep_ts.rearrange("(p c) -> p c", p=P),
                               [P, N // P], "wf")
            wpart = rows.tile([P, 1], F32, tag="wpart", name="wpart")
            nc.vector.reduce_sum(wpart[:, :], wfull[:, :], axis=AXL.X)
            pssum = ps_sml.tile([1, 512], F32, tag="sml", name="wsps")
            nc.tensor.matmul(pssum[:1, :1], ones_col[:], wpart[:, :],
                             start=True, stop=True)
            wsum = rows.tile([1, 1], F32, tag="wsum", name="wsum")
            nc.vector.tensor_copy(wsum[:1, :], pssum[:1, :1])
            nc.vector.tensor_scalar_add(wsum[:1, :], wsum[:1, :], 1e-8)
            nc.vector.reciprocal(wsum[:1, :], wsum[:1, :])
            wloc = rec_weight(ep_imp_s[None, :], ep_ts_s[None, :], [1, NL], "wl")
            nc.vector.tensor_scalar(wloc[:1, :], wloc[:1, :], wsum[:1, :1], None,
                                    op0=ALU.mult)

            # ---- episodic keys: project (norm fused), scale, store, AG;
            # skT0's load rides behind the early ek weight columns.
            ekT, psn_ek = project3(epP, wek_pk, "ekT", "kT", "f32",
                                   mid_emit=lambda p: load_sk(0, p),
                                   first_wc=wc_ek0)
            # ---- e_vals natural layout [NL, H]; bf16 single term
            for jc in range(H // 512):
                psts = [ps_mm.tile([P, 512], F32, tag="mm", name=f"evps{i}")
                        for i in range(NL // P)]
                for h4 in range(HT // 4):
                    wt16 = wtp.tile([P, 4, 512], BF16, tag="wt", name="wt16")
                    nc.sync.dma_start(
                        wt16[:],
                        wev16[h4 * 512:(h4 + 1) * 512,
                              jc * 512:(jc + 1) * 512].rearrange(
                                  "(q p) c -> p q c", p=P))
                    for q4 in range(4):
                        hi = h4 * 4 + q4
                        for nt in range(NL // P):
                            ns = slice(nt * P, (nt + 1) * P)
                            nc.tensor.matmul(
                                psts[nt][:], epP[:, 0, hi, ns], wt16[:, q4],
                                start=(hi == 0), stop=(hi == HT - 1))
                for nt in range(NL // P):
                    evs = s512p.tile([P, 512], BF16, tag="evo16", name="evout", bufs=1)
                    nc.vector.tensor_copy(evs[:], psts[nt][:])
                    nc.scalar.dma_start(
                        ag_ev_in[nt * P:(nt + 1) * P, jc * 512:(jc + 1) * 512],
                        evs[:])

            inv_ek = norm_row_finish(psn_ek, extra_row=wloc[:1, :])
            scale_cols(ekT, inv_ek)
            store_pair_to_ag(ekT, ag_nek_in)
            nc.gpsimd.collective_compute(
                "AllGather", ALU.bypass,
                replica_groups=[list(range(NCORES))],
                ins=[ag_nek_in.opt()], outs=[ag_nek_out.opt()])
            nc.gpsimd.collective_compute(
                "AllGather", ALU.bypass,
                replica_groups=[list(range(NCORES))],
                ins=[ag_ev_in.opt()], outs=[ag_ev_out.opt()])

            # ---- semantic keys: 4 chunks of 512 (loads via mid_emit hooks)
            wc0_box = {"wc": None}
            for mc in range(ML // 512):
                nxt = (lambda p, m=mc + 1: load_sk(m, p)) \
                    if mc + 1 < ML // 512 else load_qTin
                ksT, psn_ks = project3(skPs[mc], wsk_pk, f"ksT{mc}", "kT",
                                       "f32", mid_emit=nxt,
                                       first_wc=wc0_box["wc"])
                # prefetch the next projection's first weight column ahead
                # of this chunk's norm/scale/store emission
                wc0_box["wc"] = wcol_pair(
                    wsk_pk if mc + 1 < ML // 512 else wq_pk, 0)
                inv_ks = norm_row_finish(psn_ks)
                scale_cols(ksT, inv_ks)
                store_pair_to_ag(ksT, ag_nks_in[mc])
                nc.gpsimd.collective_compute(
                    "AllGather", ALU.bypass,
                    replica_groups=[list(range(NCORES))],
                    ins=[ag_nks_in[mc].opt()], outs=[ag_nks_out[mc].opt()])

            # ===================================================================
            # Phase Q: query-side projections (norms fused)
            # ===================================================================
            qTinP = qTin_box[0]
            qTp, psn_q = project3(qTinP, wq_pk, "qT", "kT", "pair",
                                  first_wc=wc0_box["wc"])
            wc0_qs = wcol_pair(wsq_pk, 0)
            inv_q = norm_row_finish(psn_q)
            qsP, psn_qs = project3(qTp, wsq_pk, "qsT", "xTin", "pair",
                                   first_wc=wc0_qs)
            inv_qs = norm_row_finish(psn_qs)

            # transpose inv rows -> per-partition [128, NBT] via DRAM bounce
            invq_p = cst.tile([P, NBT], F32, name="invq_p")
            invqs_p = cst.tile([P, NBT], F32, name="invqs_p")
            bounce = dram.tile([2, BL], F32, name="bounce")
            nc.sync.dma_start(bounce[0:1, :], inv_q[:1, :])
            nc.sync.dma_start(bounce[1:2, :], inv_qs[:1, :])
            nc.sync.dma_start(
                invq_p[:, :], bounce[0:1, :].rearrange("o (t p) -> (o p) t", p=P))
            nc.sync.dma_start(
                invqs_p[:, :], bounce[1:2, :].rearrange("o (t p) -> (o p) t", p=P))

            def bcast_row(dram_row, width, pool, tag, name, dt=F32):
                row = rows.tile([1, width], F32, tag="crow", name="crow", bufs=1)
                nc.sync.dma_start(row[:1, :], dram_row)
                src = row[:1, :]
                if dt != F32:
                    row16 = rows.tile([1, width], dt, tag="crow16",
                                      name="crow16", bufs=1)
                    nc.scalar.activation(row16[:1, :], row[:1, :], AF.Copy)
                    src = row16[:1, :]
                t = pool.tile([P, width], dt, tag=tag, name=name, bufs=1)
                nc.gpsimd.partition_broadcast(t[:, :], src)
                return t

            b1bc = bcast_row(gate_b1[None, :], 64, cst, "b1bc", "b1bc")
            b2bc = bcast_row(gate_b2[None, :], 3, cst, "b2bc", "b2bc")
            # gate+work concatenated projection weights (bf16 pair); rides the
            # wcol ring slot freed after the last qs weight column.
            gwk = wcolp.tile([P, 2, HT, 128], BF16, tag="wcp", name="gwk")
            nc.sync.dma_start(gwk[:], gwk_pk)
            gw2 = cst.tile([64, 3], F32, name="gw2")
            nc.sync.dma_start(gw2[:, :], gate_W2)

            inv_sqrt_h = 1.0 / math.sqrt(H)
            ewT_pre = [None] * NBT
            gw_pre = [None] * NBT

            gate_st = [None] * NBT

            def emit_gate_a(bt):
                """Gate/work stage A: fused matmul (cols 0:64 gate hidden,
                64:128 work logits; stationary q bf16-hi, moving bf16 pair of
                hstack(gate_W1, work_slots.T)) + the off-PE softmax chain."""
                psg = ps_sml.tile([P, 128], F32, tag="sml", name="psg")
                bs = slice(bt * P, (bt + 1) * P)
                for hi in range(HT):
                    nc.tensor.matmul(
                        psg[:, :], qTp[:, 0, hi, bs], gwk[:, 0, hi, :],
                        start=(hi == 0), stop=False)
                    nc.tensor.matmul(
                        psg[:, :], qTp[:, 0, hi, bs], gwk[:, 1, hi, :],
                        start=False, stop=(hi == HT - 1))
                hid = tiny.tile([P, 64], F32, tag="c64", name="hid")
                nc.vector.tensor_add(hid[:, :], psg[:, :64], b1bc[:, :])
                nc.scalar.activation(hid[:, :], hid[:, :], AF.Silu)
                wmax = tiny.tile([P, 1], F32, tag="c1", name="wmax")
                nc.vector.reduce_max(wmax[:, :], psg[:, 64:], axis=AXL.X)
                nc.vector.tensor_scalar_mul(wmax[:, :], wmax[:, :], -inv_sqrt_h)
                ew = tiny.tile([P, S], F32, tag="cew", name="ew")
                nc.scalar.activation(ew[:, :], psg[:, 64:], AF.Exp,
                                     bias=wmax[:, :1], scale=inv_sqrt_h)
                zw = tiny.tile([P, 1], F32, tag="czw", name="zw")
                nc.vector.reduce_sum(zw[:, :], ew[:, :], axis=AXL.X)
                nc.vector.reciprocal(zw[:, :], zw[:, :])
                gate_st[bt] = (hid, ew, zw)

            def emit_gate_b(bt):
                """Gate/work stage B: transposes + gate MLP tail; its PE ops
                depend only on stage-A results finished a sim chunk ago."""
                hid, ew, zw = gate_st[bt]
                psht = ps_sml.tile([64, P], F32, tag="sml", name="hidtp")
                nc.tensor.transpose(out=psht[:64, :], in_=hid[:, :],
                                    identity=ident[:])
                hidT = tiny.tile([64, P], F32, tag="c128", name="hidT")
                nc.vector.tensor_copy(hidT[:, :], psht[:64, :])
                psg2 = ps_sml.tile([P, 3], F32, tag="sml", name="psg2")
                nc.tensor.matmul(psg2[:, :3], hidT[:, :], gw2[:, :],
                                 start=True, stop=True)
                gl = cst.tile([P, 3], F32, name=f"gl{bt}")
                nc.vector.tensor_add(gl[:, :], psg2[:, :3], b2bc[:, :])
                gmax = tiny.tile([P, 1], F32, tag="c1", name="gmax")
                nc.vector.reduce_max(gmax[:, :], gl[:, :], axis=AXL.X)
                nc.vector.tensor_scalar_mul(gmax[:, :], gmax[:, :], -1.0)
                nc.scalar.activation(gl[:, :], gl[:, :], AF.Exp, bias=gmax[:, :1])
                gz = tiny.tile([P, 1], F32, tag="c1", name="gz")
                nc.vector.reduce_sum(gz[:, :], gl[:, :], axis=AXL.X)
                nc.vector.reciprocal(gz[:, :], gz[:, :])
                nc.vector.tensor_scalar(gl[:, :], gl[:, :], gz[:, :1], None,
                                        op0=ALU.mult)
                gw_pre[bt] = gl
                # fold softmax normalization AND gate weight 0 into ew
                nc.vector.tensor_tensor(out=zw[:, :], in0=zw[:, :],
                                        in1=gl[:, 0:1], op=ALU.mult)
                nc.vector.tensor_scalar(ew[:, :], ew[:, :], zw[:, :1], None,
                                        op0=ALU.mult)

            def emit_gate_c(bt):
                """Gate/work stage C: transpose of the folded work probs."""
                _, ew, _ = gate_st[bt]
                pset = ps_sml.tile([S, P], F32, tag="sml", name="ewtp")
                nc.tensor.transpose(out=pset[:S, :], in_=ew[:, :],
                                    identity=ident[:])
                ewT = cst.tile([S, P], BF16, name=f"ewT{bt}")
                nc.vector.tensor_copy(ewT[:, :], pset[:S, :])
                ewT_pre[bt] = ewT

            # ===================================================================
            # Phase S: similarity + per-chunk top-8 candidates
            # ===================================================================
            cand_v_e = [big.tile([P, (N // 512) * 8], F32, tag=f"cve{bt}",
                                 name=f"cve{bt}") for bt in range(NBT)]
            cand_i_e = [big.tile([P, (N // 512) * 8], F32, tag=f"cie{bt}",
                                 name=f"cie{bt}") for bt in range(NBT)]
            cand_v_s = [big.tile([P, (M // 512) * 8], F32, tag=f"cvs{bt}",
                                 name=f"cvs{bt}") for bt in range(NBT)]
            cand_i_s = [big.tile([P, (M // 512) * 8], F32, tag=f"cis{bt}",
                                 name=f"cis{bt}") for bt in range(NBT)]

            def sim_chunk(xP, kd, r, ch, cand_v, cand_i, base):
                """sims of all 4 b-tiles vs bf16-pair keys kd[r, :, h, :]."""
                psts = [ps_mm.tile([P, 512], F32, tag="mm", name=f"simps{i}")
                        for i in range(NBT)]
                for hi in range(HT):
                    kth = s512p.tile([P, 512], BF16, tag="st16h", name="kth",
                                     bufs=5)
                    ktl = s512p.tile([P, 512], BF16, tag="st16l", name="ktl",
                                     bufs=5)
                    nc.sync.dma_start(
                        kth[:], kd[r, 0, hi * P:(hi + 1) * P, :])
                    nc.sync.dma_start(
                        ktl[:], kd[r, 1, hi * P:(hi + 1) * P, :])
                    for bt in range(NBT):
                        bs = slice(bt * P, (bt + 1) * P)
                        nc.tensor.matmul(
                            psts[bt][:], xP[:, 0, hi, bs], kth[:],
                            start=(hi == 0), stop=False)
                        nc.tensor.matmul(
                            psts[bt][:], xP[:, 0, hi, bs], ktl[:],
                            start=False, stop=False)
                        nc.tensor.matmul(
                            psts[bt][:], xP[:, 1, hi, bs], kth[:],
                            start=False, stop=(hi == HT - 1))
                for bt in range(NBT):
                    sc = simb.tile([P, 512], F32, tag="simc", name="simc",
                                   bufs=2)
                    nc.scalar.activation(sc[:], psts[bt][:], AF.Copy)
                    mx = simb.tile([P, 8], F32, tag="mx", name="mx")
                    mi = simb.tile([P, 8], U32, tag="mi", name="mi")
                    nc.vector.max(out=mx[:], in_=sc[:])
                    nc.vector.max_index(out=mi[:], in_max=mx[:], in_values=sc[:])
                    nc.vector.tensor_copy(cand_v[bt][:, ch * 8:(ch + 1) * 8],
                                          mx[:])
                    mif = simb.tile([P, 8], F32, tag="mif", name="mif")
                    nc.vector.tensor_copy(mif[:], mi[:])
                    nc.vector.tensor_scalar_add(
                        cand_i[bt][:, ch * 8:(ch + 1) * 8], mif[:],
                        float(base))

            def topk_attend(cand_v, cand_i, k, inv_p, bt, vals_dram, gscale,
                            acc_tag, bufs=2):
                """Merged top-k -> softmax (x gscale) -> gather + weighted sum."""
                top8 = tiny.tile([P, 8], F32, tag="c8", name="top8")
                nc.vector.max(out=top8[:], in_=cand_v[:])
                idxf = tiny.tile([P, 8], F32, tag="c8", name="idxf")
                eqm = s512p.tile([P, 256], F32, tag="sqn", name="eqm")
                for kk in range(k):
                    w = cand_v.shape[-1]
                    nc.vector.tensor_scalar(
                        eqm[:, :w], cand_v[:], top8[:, kk:kk + 1], None,
                        op0=ALU.is_equal)
                    nc.vector.tensor_tensor(
                        out=eqm[:, :w], in0=eqm[:, :w], in1=cand_i[:], op=ALU.mult)
                    nc.vector.reduce_sum(idxf[:, kk:kk + 1], eqm[:, :w], axis=AXL.X)
                idxu = tiny.tile([P, 8], U32, tag="c8u", name="idxu")
                nc.vector.tensor_copy(idxu[:, :k], idxf[:, :k])
                sc8 = tiny.tile([P, 8], F32, tag="c8", name="sc8")
                nc.vector.tensor_scalar(
                    sc8[:, :k], top8[:, :k], inv_p[:, bt:bt + 1], None,
                    op0=ALU.mult)
                negm = tiny.tile([P, 1], F32, tag="c1", name="negm")
                nc.vector.tensor_scalar_mul(negm[:, :], sc8[:, 0:1], -1.0)
                nc.scalar.activation(sc8[:, :k], sc8[:, :k], AF.Exp,
                                     bias=negm[:, :1])
                zs = tiny.tile([P, 1], F32, tag="c1", name="zs")
                nc.vector.reduce_sum(zs[:, :], sc8[:, :k], axis=AXL.X)
                nc.vector.reciprocal(zs[:, :], zs[:, :])
                nc.vector.tensor_scalar(zs[:, :], zs[:, :], gscale, None,
                                        op0=ALU.mult)
                nc.vector.tensor_scalar(sc8[:, :k], sc8[:, :k], zs[:, :1], None,
                                        op0=ALU.mult)
                acc = sm.tile([P, H], BF16, tag=acc_tag, name="acc" + acc_tag,
                              bufs=bufs)
                nc.vector.memset(acc[:, :], 0.0)
                for kk in range(k):
                    g = gath.tile([P, H], BF16, tag="g", name="g")
                    nc.gpsimd.indirect_dma_start(
                        out=g[:, :], out_offset=None, in_=vals_dram,
                        in_offset=bass.IndirectOffsetOnAxis(
                            ap=idxu[:, kk:kk + 1], axis=0))
                    nc.vector.scalar_tensor_tensor(
                        out=acc[:, :], in0=g[:, :], scalar=sc8[:, kk:kk + 1],
                        in1=acc[:, :], op0=ALU.mult, op1=ALU.add)
                return acc

            def transpose_into(dst, src, dt=BF16):
                """dst [P, HT, P] view <- transpose of src [P, H]; psum
                evacuation alternates ACT/DVE so neither sequencer's
                per-op dispatch overhead paces the chain."""
                idn = ident if dt == F32 else ident16
                for hi in range(HT):
                    pst = ps_mm.tile([P, P], dt, tag="mm", name="trf")
                    nc.tensor.transpose(out=pst[:], in_=src[:, hi * P:(hi + 1) * P],
                                        identity=idn[:])
                    nc.scalar.activation(dst[:, hi, :], pst[:], AF.Copy)

            # episodic sims: one gathered buffer, rank-major global indices;
            # gate/work stages (off-PE-latency-heavy) interleave with chunks
            # so each stage's PE ops only see dependencies already finished.
            gbc2 = [None] * (H // 512)
            bbc2 = [None] * (H // 512)
            for ch in range(N // 512):
                if 2 <= ch <= NBT + 1:
                    emit_gate_c(ch - 2)
                if 1 <= ch <= NBT:
                    emit_gate_b(ch - 1)
                if ch < NBT:
                    emit_gate_a(ch)
                if ch == 4:
                    # LN gamma/beta broadcast tiles (full row, loaded in 512
                    # chunks); the serial DMA<->POOL ping-pong hides under
                    # the remaining sim chunks.
                    gbc2[0] = sm.tile([P, H], BF16, tag="gbc", name="gbc",
                                      bufs=1)
                    bbc2[0] = sm.tile([P, H], BF16, tag="bbc", name="bbc",
                                      bufs=1)
                    for t, dsrc in ((gbc2[0], ln_gamma), (bbc2[0], ln_beta)):
                        for jq in range(H // 512):
                            cq = slice(jq * 512, (jq + 1) * 512)
                            row = rows.tile([1, 512], F32, tag="crow",
                                            name="crow", bufs=1)
                            nc.sync.dma_start(row[:1, :], dsrc[None, cq])
                            row16 = rows.tile([1, 512], BF16, tag="crow16",
                                              name="crow16", bufs=1)
                            nc.scalar.activation(row16[:1, :], row[:1, :],
                                                 AF.Copy)
                            nc.gpsimd.partition_broadcast(t[:, cq],
                                                          row16[:1, :])
                sim_chunk(qTp, ag_nek_out, ch, ch, cand_v_e, cand_i_e,
                          ch * 512)

            # episodic merges (DVE/gathers) overlap semantic sims (PE); the
            # accT_e transposes are emitted after a sem chunk each so the PE
            # queue never waits on a merge.
            accT_e = big.tile([P, NBT, HT, P], BF16, tag="kT", name="accTe")
            accT_s = big.tile([P, NBT, HT, P], BF16, tag="xTin", name="accTs")
            acc_e = [None] * NBT
            acc_s = [None] * NBT

            def emit_merge_e(bt):
                acc_e[bt] = topk_attend(cand_v_e[bt][:], cand_i_e[bt][:], EP_K,
                                        invq_p, bt, ag_ev_out[:, :],
                                        gw_pre[bt][:, 1:2], "sl1")

            def emit_merge_s(bt):
                acc_s[bt] = topk_attend(cand_v_s[bt][:], cand_i_s[bt][:], SEM_K,
                                        invqs_p, bt, semv16, gw_pre[bt][:, 2:3],
                                        "sl2", bufs=3)

            sem_seq = [(i, r) for i in range(ML // 512) for r in range(NCORES)]

            def emit_sem_chunk(ch):
                i, r = sem_seq[ch]
                sim_chunk(qsP, ag_nks_out[i], r, ch, cand_v_s, cand_i_s,
                          r * ML + i * 512)

            emit_merge_e(0)
            emit_merge_e(1)
            p2a_pre = {}
            for ch in range(len(sem_seq)):
                if ch == len(sem_seq) - 2:
                    # prefetch Pass 2a's first moving tiles so its opening
                    # matmuls don't wait on the DMA queue draining
                    wsn0 = s512p.tile([S, 512], BF16, tag="s512", name="wsn2")
                    nc.sync.dma_start(wsn0[:S, :], ws16[:, :512])
                    wt0 = wtp.tile([P, 4, 512], BF16, tag="wt", name="wto")
                    nc.sync.dma_start(
                        wt0[:], weo16[:512, :512].rearrange(
                            "(q p) c -> p q c", p=P))
                    p2a_pre["wsn"] = wsn0
                    p2a_pre["wt"] = wt0
                emit_sem_chunk(ch)
                if ch < NBT:
                    transpose_into(accT_e[:, ch], acc_e[ch])
                    if ch + 2 < NBT:
                        emit_merge_e(ch + 2)

            # ===================================================================
            # Phase F: blend + output projections + streaming layernorm
            # ===================================================================
            bl_all = big.tile([P, NBT, H], BF16, tag="bl", name="bl_all")

            emit_merge_s(0)
            emit_merge_s(1)
            emit_merge_s(2)
            # Pass 2a: bl = gate0*w_out + acc_e @ W_eo (jc-major, weights read
            # once); ACT evacuates so DVE stays free for the semantic merges,
            # which run concurrently on DVE. The accT_s transposes interleave
            # between jc blocks: Ts_k lands right after merge k finishes, and
            # releasing acc_s[0] lets merge 3's ring slot allocate.
            for jc in range(H // 512):
                cs = slice(jc * 512, (jc + 1) * 512)
                if jc == 0:
                    wsn = p2a_pre["wsn"]
                else:
                    wsn = s512p.tile([S, 512], BF16, tag="s512", name="wsn2")
                    nc.sync.dma_start(wsn[:S, :], ws16[:, cs])
                psos = [ps_mm.tile([P, 512], F32, tag="mm", name=f"pso{i}")
                        for i in range(NBT)]
                for bt in range(NBT):
                    nc.tensor.matmul(psos[bt][:], ewT_pre[bt][:, :],
                                     wsn[:S, :], start=True, stop=False)
                for hq in range(HT // 4):
                    if jc == 0 and hq == 0:
                        wt = p2a_pre["wt"]
                    else:
                        wt = wtp.tile([P, 4, 512], BF16, tag="wt", name="wto")
                        nc.sync.dma_start(
                            wt[:], weo16[hq * 512:(hq + 1) * 512, cs].rearrange(
                                "(q p) c -> p q c", p=P))
                    for q4 in range(4):
                        hi = hq * 4 + q4
                        for bt in range(NBT):
                            nc.tensor.matmul(
                                psos[bt][:], accT_e[:, bt, hi, :], wt[:, q4],
                                start=False, stop=(hi == HT - 1))
                for bt in range(NBT):
                    nc.scalar.activation(bl_all[:, bt, cs], psos[bt][:], AF.Copy)
                if 1 <= jc:
                    transpose_into(accT_s[:, jc - 1], acc_s[jc - 1])
                    if jc == H // 512 - 1:
                        transpose_into(accT_s[:, jc], acc_s[jc])
                if jc == 0:
                    emit_merge_s(3)

            blT = big.tile([P, NBT, HT, P], BF16, tag="kT", name="blT")
            # Pass 2b: bl += acc_s @ W_so (jc-major, weights read once)
            for jc in range(H // 512):
                cs = slice(jc * 512, (jc + 1) * 512)
                psob = [ps_mm.tile([P, 512], F32, tag="mm", name=f"psob{i}")
                        for i in range(NBT)]
                for hq in range(HT // 4):
                    wt = wtp.tile([P, 4, 512], BF16, tag="wt", name="wtob")
                    nc.sync.dma_start(
                        wt[:], wso16[hq * 512:(hq + 1) * 512, cs].rearrange(
                            "(q p) c -> p q c", p=P))
                    for q4 in range(4):
                        hi = hq * 4 + q4
                        for bt in range(NBT):
                            nc.tensor.matmul(
                                psob[bt][:], accT_s[:, bt, hi, :], wt[:, q4],
                                start=(hi == 0), stop=(hi == HT - 1))
                for bt in range(NBT):
                    nc.vector.tensor_add(bl_all[:, bt, cs],
                                         bl_all[:, bt, cs], psob[bt][:])
                    if jc == H // 512 - 1:
                        nc.sync.dma_start_transpose(blT[:, bt],
                                                    bl_all[:, bt, :])

            # Pass 3: xo = bl @ W_ro (jc-major) with streamed LN stats
            xo_all = big.tile([P, NBT, H], BF16, tag="xTin", name="xo_all")
            msum = [tiny.tile([P, 4], F32, tag=f"cms{i}", name=f"msum{i}",
                              bufs=1) for i in range(NBT)]
            vsum = [tiny.tile([P, 4], F32, tag=f"cvs{i}", name=f"vsum{i}",
                              bufs=1) for i in range(NBT)]
            ln_stats = [None] * NBT

            def emit_ln_stats(bt):
                """inv-std and -mu*inv-std per-partition scalars for one bt."""
                mu = tiny.tile([P, 1], F32, tag="c1", name="mu")
                nc.vector.reduce_sum(mu[:, :], msum[bt][:, :], axis=AXL.X)
                nc.vector.tensor_scalar_mul(mu[:, :], mu[:, :], 1.0 / H)
                vs = tiny.tile([P, 1], F32, tag=f"cvv{bt}", name="vs", bufs=1)
                nc.vector.reduce_sum(vs[:, :], vsum[bt][:, :], axis=AXL.X)
                nc.vector.tensor_scalar_mul(vs[:, :], vs[:, :], 1.0 / H)
                mu2 = tiny.tile([P, 1], F32, tag="c1", name="mu2")
                nc.vector.tensor_tensor(out=mu2[:, :], in0=mu[:, :],
                                        in1=mu[:, :], op=ALU.mult)
                nc.vector.tensor_tensor(out=vs[:, :], in0=vs[:, :],
                                        in1=mu2[:, :], op=ALU.subtract)
                nc.vector.tensor_scalar_add(vs[:, :], vs[:, :], LN_EPS)
                nc.scalar.sqrt(vs[:, :], vs[:, :])
                nc.vector.reciprocal(vs[:, :], vs[:, :])
                nmu = tiny.tile([P, 1], F32, tag=f"cnm{bt}", name="nmu",
                                bufs=1)
                nc.vector.tensor_tensor(out=nmu[:, :], in0=mu[:, :],
                                        in1=vs[:, :], op=ALU.mult)
                nc.vector.tensor_scalar_mul(nmu[:, :], nmu[:, :], -1.0)
                ln_stats[bt] = (vs, nmu)
            def emit_ln_final(bt):
                """Normalize + affine + store for one b-tile: full-row bf16
                DVE ops (2x throughput, minimal dispatch count) on the gather
                ring, which is idle by this point."""
                vs, nmu = ln_stats[bt]
                on16 = gath.tile([P, H], BF16, tag="g", name="on16")
                nc.vector.tensor_scalar(on16[:, :], xo_all[:, bt, :],
                                        vs[:, :1], nmu[:, :1],
                                        op0=ALU.mult, op1=ALU.add)
                nc.vector.tensor_mul(on16[:, :], on16[:, :], gbc2[0][:, :])
                on = gath.tile([P, H], BF16, tag="g", name="on")
                nc.vector.tensor_add(on[:, :], on16[:, :], bbc2[0][:, :])
                nc.sync.dma_start(out_s[bt * P:(bt + 1) * P, :], on[:])

            for jc in range(H // 512):
                cs = slice(jc * 512, (jc + 1) * 512)
                psro = [ps_mm.tile([P, 512], F32, tag="mm", name=f"psro{i}")
                        for i in range(NBT)]
                for hq in range(HT // 4):
                    wt = wtp.tile([P, 4, 512], BF16, tag="wt", name="wtro")
                    nc.sync.dma_start(
                        wt[:], wro16[hq * 512:(hq + 1) * 512, cs].rearrange(
                            "(q p) c -> p q c", p=P))
                    for q4 in range(4):
                        hi = hq * 4 + q4
                        for bt in range(NBT):
                            nc.tensor.matmul(
                                psro[bt][:], blT[:, bt, hi, :], wt[:, q4],
                                start=(hi == 0), stop=(hi == HT - 1))
                for bt in range(NBT):
                    nc.scalar.activation(xo_all[:, bt, cs], psro[bt][:],
                                         AF.Copy,
                                         accum_out=msum[bt][:, jc:jc + 1])
                    sqc = s512p.tile([P, 512], F32, tag="sqn", name="sqc")
                    nc.scalar.activation(sqc[:, :], psro[bt][:], AF.Square,
                                         accum_out=vsum[bt][:, jc:jc + 1])
                    if jc == H // 512 - 1:
                        emit_ln_stats(bt)
            for bt in range(NBT):
                emit_ln_final(bt)

    nc.finalize()
    return nc


_NC_CACHE = None
LAST_EXEC_NS = None


def _pack_xpair(x):
    """[R,H] f32 -> [R//512, P, 2, HT, 512] bf16 pair, pre-transposed to
    the on-chip tile layout: pk[ch, p, half, hi, r] = split(x)[half][
    ch*512+r, hi*128+p]."""
    hi_, lo_ = _split_bf16(x)
    def lay(a):
        return a.reshape(-1, HT, P).transpose(2, 1, 0)   # [P, HT, R]
    pk = np.stack([lay(hi_), lay(lo_)], axis=1)          # [P, 2, HT, R]
    R = x.shape[0]
    return np.ascontiguousarray(
        np.stack([pk[..., i * 512:(i + 1) * 512]
                  for i in range(R // 512)], axis=0))


def _pack_wpair(w):
    """[H,H] f32 -> [HT, P, 2, HT, P] bf16 pair in wcP tile layout:
    packed[j, p, half, hi, c] = split(W)[half][hi*128+p, j*128+c]."""
    hi_, lo_ = _split_bf16(w)
    def lay(a):
        # [hi, p, j, c] -> [j, p, hi, c]
        return np.ascontiguousarray(
            a.reshape(HT, P, HT, P).transpose(2, 1, 0, 3))
    return np.ascontiguousarray(
        np.stack([lay(hi_), lay(lo_)], axis=2))


def _pack_gwk(gate_W1, work_slots):
    """hstack(gate_W1 [H,64], work_slots.T [H,64]) -> [P, 2, HT, 128] pair:
    pk[p, half, hi, c] = split(gw)[half][hi*128+p, c]."""
    gw = np.hstack([np.asarray(gate_W1, np.float32),
                    np.ascontiguousarray(np.asarray(work_slots, np.float32).T)])
    hi_, lo_ = _split_bf16(gw)
    def lay(a):
        return a.reshape(HT, P, 128).transpose(1, 0, 2)   # [P, HT, 128]
    return np.ascontiguousarray(np.stack([lay(hi_), lay(lo_)], axis=1))


def _split_bf16(x):
    """two-term bf16 decomposition: x ~= hi + lo to ~16 mantissa bits."""
    import ml_dtypes
    bf = ml_dtypes.bfloat16
    x = np.ascontiguousarray(np.asarray(x), dtype=np.float32)
    hi = x.astype(bf)
    lo = (x - hi.astype(np.float32)).astype(bf)
    return hi, lo


def kernel(**inputs) -> np.ndarray:
    global _NC_CACHE
    if _NC_CACHE is None:
        _NC_CACHE = build()
    nc = _NC_CACHE

    def arr(x):
        return np.ascontiguousarray(np.asarray(x), dtype=np.float32)

    wq_pk = _pack_wpair(inputs["W_query"])
    wek_pk = _pack_wpair(inputs["W_ek"])
    wsq_pk = _pack_wpair(inputs["W_sq"])
    wsk_pk = _pack_wpair(inputs["W_sk"])
    wev16, _ = _split_bf16(inputs["W_ev"])
    weo16, _ = _split_bf16(inputs["W_eo"])
    wso16, _ = _split_bf16(inputs["W_so"])
    wro16, _ = _split_bf16(inputs["W_ro"])
    semv16, _ = _split_bf16(inputs["sem_values"])
    ws16, _ = _split_bf16(inputs["work_slots"])
    gwk_pk = _pack_gwk(inputs["gate_W1"], inputs["work_slots"])

    in_maps = []
    for c in range(NCORES):
        in_maps.append({
            "query_pk": _pack_xpair(inputs["query"][c * BL:(c + 1) * BL]),
            "ep_pk": _pack_xpair(inputs["ep_store"][c * NL:(c + 1) * NL]),
            "semk_pk": _pack_xpair(inputs["sem_keys"][c * ML:(c + 1) * ML]),
            "ep_imp_s": arr(inputs["ep_importance"][c * NL:(c + 1) * NL]),
            "ep_ts_s": arr(inputs["ep_timestamps"][c * NL:(c + 1) * NL]),
            "ep_imp": arr(inputs["ep_importance"]),
            "ep_ts": arr(inputs["ep_timestamps"]),
            "semv16": semv16,
            "wq_pk": wq_pk,
            "wek_pk": wek_pk,
            "wsq_pk": wsq_pk,
            "wsk_pk": wsk_pk,
            "wev16": wev16,
            "weo16": weo16,
            "wso16": wso16,
            "wro16": wro16,
            "ws16": ws16,
            "gwk_pk": gwk_pk,
            "gate_b1": arr(inputs["gate_b1"]),
            "gate_W2": arr(inputs["gate_W2"]),
            "gate_b2": arr(inputs["gate_b2"]),
            "ln_gamma": arr(inputs["ln_gamma"]),
            "ln_beta": arr(inputs["ln_beta"]),
        })
    res = run_bass_kernel_spmd(nc, in_maps, core_ids=list(range(NCORES)))
    return np.concatenate(
        [np.asarray(res.results[c]["out_s"], dtype=np.float32)
         for c in range(NCORES)], axis=0)


# revision 80
# speedup vs baseline: 1.0651x; 1.0141x over previous
"""ONIMemoryHub kernel for 8 Trainium2 NeuronCores (Bass/Tile).

Sharding: data-parallel over batch for the query side; episodic store and
semantic memory sharded across cores for the key/value projections, with
AllGathers of the projected (normalized, pre-scaled) keys/values.

Schedule notes (v2): the PE instruction stream is kept free of stalls by
emitting off-engine work (top-k merges, norms, layernorm) interleaved
between matmul blocks whose inputs are already resident:
  - projection column norms are fused into the projection evacuation
  - gate/work blocks interleave with the episodic sim chunks
  - episodic merges/transposes interleave with the semantic sim chunks
  - semantic merges interleave with the W_eo output pass
  - layernorm stats stream out of the W_ro pass via ACT accumulators and
    the finals run as full-row bf16 DVE ops, shrinking the kernel tail
  - blend transposes go through the DMA crossbar, store-side DMAs ride
    the ACT queue, and the Pool queue is kept clear of the collectives'
    head-of-line blocking.

kernel(**inputs) takes FULL inputs (as produced by reference.setup_inputs())
and returns the FULL [4096, 2048] output.
"""
import math

import numpy as np

import concourse.bass as bass
import concourse.mybir as mybir
import concourse.tile as tile
from concourse import bacc
from concourse.bass_utils import run_bass_kernel_spmd
from concourse.masks import make_identity

AF = mybir.ActivationFunctionType
AXL = mybir.AxisListType
ALU = mybir.AluOpType

NCORES = 8
B, H, N, M, S = 4096, 2048, 4096, 16384, 64
BL, NL, ML = B // NCORES, N // NCORES, M // NCORES   # 512, 512, 2048
HT = H // 128                                        # 16 h-tiles
P = 128
NBT = BL // P                                        # 4 b-tiles
EP_K = 8
SEM_K = 4
LN_EPS = 1e-5
RECENCY = 0.01   # 1 - RECENCY_DECAY

F32 = mybir.dt.float32
BF16 = mybir.dt.bfloat16
U32 = mybir.dt.uint32


def build():
    nc = bacc.Bacc("TRN2", target_bir_lowering=False, debug=False,
                   num_devices=NCORES)

    def din(name, shape, dt=F32):
        return nc.dram_tensor(name, shape, dt, kind="ExternalInput").ap()

    # per-core slices: host-split bf16 pairs, pre-transposed to tile layout
    query_pk = din("query_pk", [1, P, 2, HT, 512], BF16)
    ep_pk = din("ep_pk", [1, P, 2, HT, 512], BF16)
    semk_pk = din("semk_pk", [ML // 512, P, 2, HT, 512], BF16)
    ep_imp_s = din("ep_imp_s", [NL])
    ep_ts_s = din("ep_ts_s", [NL])
    # replicated
    ep_imp = din("ep_imp", [N])
    ep_ts = din("ep_ts", [N])
    semv16 = din("semv16", [M, H], BF16)
    wq_pk = din("wq_pk", [HT, P, 2, HT, P], BF16)
    wek_pk = din("wek_pk", [HT, P, 2, HT, P], BF16)
    wsq_pk = din("wsq_pk", [HT, P, 2, HT, P], BF16)
    wsk_pk = din("wsk_pk", [HT, P, 2, HT, P], BF16)
    wev16 = din("wev16", [H, H], BF16)
    weo16 = din("weo16", [H, H], BF16)
    wso16 = din("wso16", [H, H], BF16)
    wro16 = din("wro16", [H, H], BF16)
    ws16 = din("ws16", [S, H], BF16)
    gwk_pk = din("gwk_pk", [P, 2, HT, 128], BF16)
    gate_b1 = din("gate_b1", [64])
    gate_W2 = din("gate_W2", [64, 3])
    gate_b2 = din("gate_b2", [3])
    ln_gamma = din("ln_gamma", [H])
    ln_beta = din("ln_beta", [H])

    out_s = nc.dram_tensor("out_s", [BL, H], BF16, kind="ExternalOutput").ap()

    with tile.TileContext(nc) as tc:
        with (
            tc.tile_pool(name="cst", bufs=1) as cst,
            tc.tile_pool(name="big", bufs=1) as big,
            tc.tile_pool(name="rows", bufs=1) as rows,
            tc.tile_pool(name="s512", bufs=2) as s512p,
            tc.tile_pool(name="wcol", bufs=2) as wcolp,
            tc.tile_pool(name="wtile", bufs=2) as wtp,
            tc.tile_pool(name="sm", bufs=2) as sm,
            tc.tile_pool(name="tiny", bufs=2) as tiny,
            tc.tile_pool(name="simb", bufs=2) as simb,
            tc.tile_pool(name="gath", bufs=2) as gath,
            tc.tile_pool(name="ps_mm", bufs=7, space="PSUM") as ps_mm,
            tc.tile_pool(name="ps_sml", bufs=1, space="PSUM") as ps_sml,
            tc.tile_pool(name="dram", bufs=1, space="DRAM") as dram,
        ):
            ident = cst.tile([P, P], F32)
            make_identity(nc, ident[:])
            ident16 = cst.tile([P, P], BF16)
            nc.scalar.activation(ident16[:], ident[:], AF.Copy)
            ones_col = cst.tile([P, 1], F32)
            nc.vector.memset(ones_col[:], 1.0)
            ones16 = cst.tile([P, 1], BF16)
            nc.vector.memset(ones16[:], 1.0)

            def warm(n):
                """Dependency-free PE transposes that keep the tensor engine's
                p-state streak alive across a known stall (the wait exists
                either way; the following real matmuls start at full clock).
                Rides the small-PSUM bank, which is idle at every call site."""
                for _ in range(n):
                    pw = ps_sml.tile([P, P], F32, tag="sml", name="warm")
                    nc.tensor.transpose(out=pw[:], in_=ident[:],
                                        identity=ident[:])

            # ---------- helpers ----------
            # big slot chains (explicit liveness via shared tags):
            #   xTin: epT -> skT(x4) -> qsT -> accTs
            #   kT  : ekT -> ksT(x4) -> accTe -> blT
            #   bl  : qTp -> bl_all
            def emit_split(dst_hi, dst_lo, src_f32, tmp32):
                """bf16 two-term split: hi = bf16(x), lo = bf16(x - hi).

                The upconvert copy runs on DVE, not gpsimd: the Pool queue
                carries the collectives, which would head-block a gpsimd
                copy (and everything after it) for a whole AllGather.
                """
                nc.scalar.activation(dst_hi, src_f32, AF.Copy)
                nc.vector.tensor_copy(tmp32, dst_hi)
                nc.vector.tensor_tensor(out=tmp32, in0=src_f32, in1=tmp32,
                                        op=ALU.subtract)
                nc.scalar.activation(dst_lo, tmp32, AF.Copy)

            def wcol_pair(w_pk, j):
                wcP = wcolp.tile([P, 2, HT, P], BF16, tag="wcp", name="wcp")
                nc.sync.dma_start(wcP[:], w_pk[j])
                return wcP

            def norm_row_finish(psn, extra_row=None):
                """[1,512] inv-norm row from accumulated sum-of-squares."""
                row = rows.tile([1, 512], F32, tag="nrow", name="nrow", bufs=2)
                nc.vector.tensor_copy(row[:1, :], psn[:1, :])
                nc.scalar.sqrt(row[:1, :], row[:1, :])
                nc.vector.tensor_scalar_max(row[:1, :], row[:1, :], 1e-12)
                nc.vector.reciprocal(row[:1, :], row[:1, :])
                if extra_row is not None:
                    nc.vector.tensor_mul(row[:1, :], row[:1, :], extra_row)
                return row

            def project3(xP, w_pk, name, tag, mode, mid_emit=None,
                         first_wc=None):
                """(x @ W).T via 3-term bf16 split matmuls; xP is a pair.

                mode "f32": returns (yT, psn) — f32 tile + sum-of-squares
                psum row (norm fused into the evacuation).
                mode "pair": returns (yP, psn) — bf16 pair tile + norm psum.
                mid_emit() is called after the j==3 block so a prefetch DMA
                can ride the SP queue behind the first few weight columns.
                """
                psn = ps_sml.tile([1, 512], F32, tag="sml", name="npsum")
                if mode == "f32":
                    yT = big.tile([P, HT, 512], F32, tag=tag, name=name)
                else:
                    yP = big.tile([P, 2, HT, 512], BF16, tag=tag, name=name)
                # norm accumulation runs at lag 1 so the PE never waits on
                # the ACT square of the chunk it just produced. Squares are
                # split to a bf16 pair: two 1-cycle/row matmuls instead of one
                # 4-cycle/row fp32 matmul, at ~2^-17 precision (norm budget
                # needs ~1e-6).
                sqs = [None] * HT
                for j in range(HT):
                    wcP = first_wc if (j == 0 and first_wc is not None) \
                        else wcol_pair(w_pk, j)
                    pst = ps_mm.tile([P, 512], F32, tag="mm", name="projps")
                    for hi in range(HT):
                        nc.tensor.matmul(
                            pst[:], wcP[:, 0, hi, :], xP[:, 0, hi, :],
                            start=(hi == 0), stop=False)
                        nc.tensor.matmul(
                            pst[:], wcP[:, 0, hi, :], xP[:, 1, hi, :],
                            start=False, stop=False)
                        nc.tensor.matmul(
                            pst[:], wcP[:, 1, hi, :], xP[:, 0, hi, :],
                            start=False, stop=(hi == HT - 1))
                    sq32 = s512p.tile([P, 512], F32, tag="s512", name="sq32")
                    nc.scalar.square(sq32[:, :], pst[:])
                    sqp = s512p.tile([P, 2, 512], BF16, tag="sqn", name="sqp")
                    tmp32 = s512p.tile([P, 512], F32, tag="s512", name="sqt32")
                    emit_split(sqp[:, 0, :], sqp[:, 1, :], sq32[:, :],
                               tmp32[:, :])
                    sqs[j] = sqp
                    if mode == "f32":
                        nc.scalar.activation(yT[:, j, :], pst[:], AF.Copy)
                    else:
                        tmp32 = s512p.tile([P, 512], F32, tag="s512",
                                           name="spj32")
                        emit_split(yP[:, 0, j, :], yP[:, 1, j, :], pst[:],
                                   tmp32[:])
                    if j >= 1:
                        nc.tensor.matmul(psn[:1, :], ones16[:],
                                         sqs[j - 1][:, 0, :],
                                         start=(j == 1), stop=False)
                        nc.tensor.matmul(psn[:1, :], ones16[:],
                                         sqs[j - 1][:, 1, :],
                                         start=False, stop=False)
                    if mid_emit is not None and j in (3, 6, 9, 12):
                        mid_emit((j - 3) // 3)
                nc.tensor.matmul(psn[:1, :], ones16[:], sqs[HT - 1][:, 0, :],
                                 start=False, stop=False)
                nc.tensor.matmul(psn[:1, :], ones16[:], sqs[HT - 1][:, 1, :],
                                 start=False, stop=True)
                if mode == "f32":
                    return yT, psn
                return yP, psn

            def store_pair_to_ag(xT, ag_in):
                """split scaled f32 keys and store bf16 pair to AG input."""
                for hi in range(HT):
                    sth = s512p.tile([P, 512], BF16, tag="st16h", name="sth",
                                     bufs=5)
                    stl = s512p.tile([P, 512], BF16, tag="st16l", name="stl",
                                     bufs=5)
                    tmp32 = s512p.tile([P, 512], F32, tag="s512", name="spg32")
                    emit_split(sth[:], stl[:], xT[:, hi, :], tmp32[:])
                    # stores ride the ACT queue (which paces them via the
                    # splits), keeping the SP queue free for weight loads
                    nc.scalar.dma_start(ag_in[0, hi * P:(hi + 1) * P, :], sth[:])
                    nc.scalar.dma_start(ag_in[1, hi * P:(hi + 1) * P, :], stl[:])

            def scale_cols(xT, scale_row):
                bc = s512p.tile([P, 512], F32, tag="s512", name="bcn")
                nc.gpsimd.partition_broadcast(bc[:, :], scale_row[:1, :])
                for hi in range(HT):
                    nc.vector.tensor_mul(xT[:, hi, :], xT[:, hi, :], bc[:, :])

            # ===================================================================
            # Phase M: sharded memory-side projections + AllGathers
            # ===================================================================
            ag_nek_in = dram.tile([2, H, NL], BF16, name="ag_nek_in")
            ag_nek_out = dram.tile([NCORES, 2, H, NL], BF16,
                                   addr_space="Shared", name="ag_nek_out")
            ag_ev_in = dram.tile([NL, H], BF16, name="ag_ev_in")
            ag_ev_out = dram.tile([N, H], BF16, addr_space="Shared",
                                  name="ag_ev_out")
            ag_nks_in = [dram.tile([2, H, 512], BF16, name=f"ag_nks_in{i}")
                         for i in range(ML // 512)]
            ag_nks_out = [dram.tile([NCORES, 2, H, 512], BF16,
                                    addr_space="Shared", name=f"ag_nks_out{i}")
                          for i in range(ML // 512)]

            # semantic-key chunk loads: double-buffered on alternating big
            # slots (bl/xTin), emitted via project3 mid_emit hooks so each
            # 12.6us DMA hides under the previous projection.
            skPs = [None] * (ML // 512)
            qTin_box = [None]

            def load_sk(mc, piece):
                """quarter-piece prefetch of a semantic-key chunk."""
                if piece == 0:
                    skPs[mc] = big.tile([P, 2, HT, 512], BF16,
                                        tag="bl" if mc % 2 == 0 else "xTin",
                                        name=f"skT{mc}")
                hs = slice(piece * 4, (piece + 1) * 4)
                nc.sync.dma_start(skPs[mc][:, :, hs], semk_pk[mc, :, :, hs])

            def load_qTin(piece):
                if piece == 0:
                    qTin_box[0] = big.tile([P, 2, HT, 512], BF16, tag="bl",
                                           name="qTin")
                hs = slice(piece * 4, (piece + 1) * 4)
                nc.sync.dma_start(qTin_box[0][:, :, hs], query_pk[0, :, :, hs])

            # First weight column rides the DMA queue ahead of epP so the very
            # first matmul chain starts as soon as epP's first piece lands;
            # epP is split so early hi-tiles arrive (and compute) first.
            wc_ek0 = wcol_pair(wek_pk, 0)
            warm(40)
            epP = big.tile([P, 2, HT, 512], BF16, tag="xTin", name="epT")
            nc.sync.dma_start(epP[:, :, :HT // 4], ep_pk[0, :, :, :HT // 4])
            nc.sync.dma_start(epP[:, :, HT // 4:HT // 2],
                              ep_pk[0, :, :, HT // 4:HT // 2])
            nc.sync.dma_start(epP[:, :, HT // 2:], ep_pk[0, :, :, HT // 2:])

            # ---- episodic recency/importance weights (off-PE, overlaps ekT)
            def rec_weight(imp_ap, ts_ap, shape, tagb):
                """(1+imp)*exp(-|1-ts|*RECENCY) elementwise; returns tile."""
                impt = rows.tile(shape, F32, tag=tagb + "i", name="impt")
                tst = rows.tile(shape, F32, tag=tagb + "t", name="tst")
                nc.sync.dma_start(impt[:shape[0], :], imp_ap)
                nc.sync.dma_start(tst[:shape[0], :], ts_ap)
                s = tst[:shape[0], :]
                nc.scalar.activation(s, s, AF.Copy, bias=0.0, scale=-1.0)
                nc.vector.tensor_scalar_add(s, s, 1.0)
                nc.scalar.activation(s, s, AF.Abs)
                nc.scalar.activation(s, s, AF.Exp, scale=-RECENCY)
                si = impt[:shape[0], :]
                nc.vector.tensor_scalar_add(si, si, 1.0)
                nc.vector.tensor_mul(si, si, s)
                return impt

            wfull = rec_weight(ep_imp.rearrange("(p c) -> p c", p=P),
                               ep_ts.rearrange("(p c) -> p c", p=P),
                               [P, N // P], "wf")
            wpart = rows.tile([P, 1], F32, tag="wpart", name="wpart")
            nc.vector.reduce_sum(wpart[:, :], wfull[:, :], axis=AXL.X)
            pssum = ps_sml.tile([1, 512], F32, tag="sml", name="wsps")
            nc.tensor.matmul(pssum[:1, :1], ones_col[:], wpart[:, :],
                             start=True, stop=True)
            wsum = rows.tile([1, 1], F32, tag="wsum", name="wsum")
            nc.vector.tensor_copy(wsum[:1, :], pssum[:1, :1])
            nc.vector.tensor_scalar_add(wsum[:1, :], wsum[:1, :], 1e-8)
            nc.vector.reciprocal(wsum[:1, :], wsum[:1, :])
            wloc = rec_weight(ep_imp_s[None, :], ep_ts_s[None, :], [1, NL], "wl")
            nc.vector.tensor_scalar(wloc[:1, :], wloc[:1, :], wsum[:1, :1], None,
                                    op0=ALU.mult)

            # ---- episodic keys: project (norm fused), scale, store, AG;
            # skT0's load rides behind the early ek weight columns.
            ekT, psn_ek = project3(epP, wek_pk, "ekT", "kT", "f32",
                                   mid_emit=lambda p: load_sk(0, p),
                                   first_wc=wc_ek0)
            # ---- e_vals natural layout [NL, H]; bf16 single term
            for jc in range(H // 512):
                psts = [ps_mm.tile([P, 512], F32, tag="mm", name=f"evps{i}")
                        for i in range(NL // P)]
                for h4 in range(HT // 4):
                    wt16 = wtp.tile([P, 4, 512], BF16, tag="wt", name="wt16")
                    nc.sync.dma_start(
                        wt16[:],
                        wev16[h4 * 512:(h4 + 1) * 512,
                              jc * 512:(jc + 1) * 512].rearrange(
                                  "(q p) c -> p q c", p=P))
                    for q4 in range(4):
                        hi = h4 * 4 + q4
                        for nt in range(NL // P):
                            ns = slice(nt * P, (nt + 1) * P)
                            nc.tensor.matmul(
                                psts[nt][:], epP[:, 0, hi, ns], wt16[:, q4],
                                start=(hi == 0), stop=(hi == HT - 1))
                for nt in range(NL // P):
                    evs = s512p.tile([P, 512], BF16, tag="evo16", name="evout", bufs=1)
                    nc.vector.tensor_copy(evs[:], psts[nt][:])
                    nc.scalar.dma_start(
                        ag_ev_in[nt * P:(nt + 1) * P, jc * 512:(jc + 1) * 512],
                        evs[:])

            inv_ek = norm_row_finish(psn_ek, extra_row=wloc[:1, :])
            scale_cols(ekT, inv_ek)
            store_pair_to_ag(ekT, ag_nek_in)
            nc.gpsimd.collective_compute(
                "AllGather", ALU.bypass,
                replica_groups=[list(range(NCORES))],
                ins=[ag_nek_in.opt()], outs=[ag_nek_out.opt()])
            nc.gpsimd.collective_compute(
                "AllGather", ALU.bypass,
                replica_groups=[list(range(NCORES))],
                ins=[ag_ev_in.opt()], outs=[ag_ev_out.opt()])

            # ---- semantic keys: 4 chunks of 512 (loads via mid_emit hooks)
            wc0_box = {"wc": None}
            for mc in range(ML // 512):
                nxt = (lambda p, m=mc + 1: load_sk(m, p)) \
                    if mc + 1 < ML // 512 else load_qTin
                ksT, psn_ks = project3(skPs[mc], wsk_pk, f"ksT{mc}", "kT",
                                       "f32", mid_emit=nxt,
                                       first_wc=wc0_box["wc"])
                # prefetch the next projection's first weight column ahead
                # of this chunk's norm/scale/store emission
                wc0_box["wc"] = wcol_pair(
                    wsk_pk if mc + 1 < ML // 512 else wq_pk, 0)
                inv_ks = norm_row_finish(psn_ks)
                scale_cols(ksT, inv_ks)
                store_pair_to_ag(ksT, ag_nks_in[mc])
                nc.gpsimd.collective_compute(
                    "AllGather", ALU.bypass,
                    replica_groups=[list(range(NCORES))],
                    ins=[ag_nks_in[mc].opt()], outs=[ag_nks_out[mc].opt()])

            # ===================================================================
            # Phase Q: query-side projections (norms fused)
            # ===================================================================
            qTinP = qTin_box[0]
            qTp, psn_q = project3(qTinP, wq_pk, "qT", "kT", "pair",
                                  first_wc=wc0_box["wc"])
            wc0_qs = wcol_pair(wsq_pk, 0)
            inv_q = norm_row_finish(psn_q)
            qsP, psn_qs = project3(qTp, wsq_pk, "qsT", "xTin", "pair",
                                   first_wc=wc0_qs)
            inv_qs = norm_row_finish(psn_qs)

            # transpose inv rows -> per-partition [128, NBT] via DRAM bounce
            invq_p = cst.tile([P, NBT], F32, name="invq_p")
            invqs_p = cst.tile([P, NBT], F32, name="invqs_p")
            bounce = dram.tile([2, BL], F32, name="bounce")
            nc.sync.dma_start(bounce[0:1, :], inv_q[:1, :])
            nc.sync.dma_start(bounce[1:2, :], inv_qs[:1, :])
            nc.sync.dma_start(
                invq_p[:, :], bounce[0:1, :].rearrange("o (t p) -> (o p) t", p=P))
            nc.sync.dma_start(
                invqs_p[:, :], bounce[1:2, :].rearrange("o (t p) -> (o p) t", p=P))

            def bcast_row(dram_row, width, pool, tag, name, dt=F32):
                row = rows.tile([1, width], F32, tag="crow", name="crow", bufs=1)
                nc.sync.dma_start(row[:1, :], dram_row)
                src = row[:1, :]
                if dt != F32:
                    row16 = rows.tile([1, width], dt, tag="crow16",
                                      name="crow16", bufs=1)
                    nc.scalar.activation(row16[:1, :], row[:1, :], AF.Copy)
                    src = row16[:1, :]
                t = pool.tile([P, width], dt, tag=tag, name=name, bufs=1)
                nc.gpsimd.partition_broadcast(t[:, :], src)
                return t

            b1bc = bcast_row(gate_b1[None, :], 64, cst, "b1bc", "b1bc")
            b2bc = bcast_row(gate_b2[None, :], 3, cst, "b2bc", "b2bc")
            # gate+work concatenated projection weights (bf16 pair); rides the
            # wcol ring slot freed after the last qs weight column.
            gwk = wcolp.tile([P, 2, HT, 128], BF16, tag="wcp", name="gwk")
            nc.sync.dma_start(gwk[:], gwk_pk)
            gw2 = cst.tile([64, 3], F32, name="gw2")
            nc.sync.dma_start(gw2[:, :], gate_W2)

            inv_sqrt_h = 1.0 / math.sqrt(H)
            ewT_pre = [None] * NBT
            gw_pre = [None] * NBT

            gate_st = [None] * NBT

            def emit_gate_a(bt):
                """Gate/work stage A: fused matmul (cols 0:64 gate hidden,
                64:128 work logits; stationary q bf16-hi, moving bf16 pair of
                hstack(gate_W1, work_slots.T)) + the off-PE softmax chain."""
                psg = ps_sml.tile([P, 128], F32, tag="sml", name="psg")
                bs = slice(bt * P, (bt + 1) * P)
                for hi in range(HT):
                    nc.tensor.matmul(
                        psg[:, :], qTp[:, 0, hi, bs], gwk[:, 0, hi, :],
                        start=(hi == 0), stop=False)
                    nc.tensor.matmul(
                        psg[:, :], qTp[:, 0, hi, bs], gwk[:, 1, hi, :],
                        start=False, stop=(hi == HT - 1))
                hid = tiny.tile([P, 64], F32, tag="c64", name="hid")
                nc.vector.tensor_add(hid[:, :], psg[:, :64], b1bc[:, :])
                nc.scalar.activation(hid[:, :], hid[:, :], AF.Silu)
                wmax = tiny.tile([P, 1], F32, tag="c1", name="wmax")
                nc.vector.reduce_max(wmax[:, :], psg[:, 64:], axis=AXL.X)
                nc.vector.tensor_scalar_mul(wmax[:, :], wmax[:, :], -inv_sqrt_h)
                ew = tiny.tile([P, S], F32, tag="cew", name="ew")
                nc.scalar.activation(ew[:, :], psg[:, 64:], AF.Exp,
                                     bias=wmax[:, :1], scale=inv_sqrt_h)
                zw = tiny.tile([P, 1], F32, tag="czw", name="zw")
                nc.vector.reduce_sum(zw[:, :], ew[:, :], axis=AXL.X)
                nc.vector.reciprocal(zw[:, :], zw[:, :])
                gate_st[bt] = (hid, ew, zw)

            def emit_gate_b(bt):
                """Gate/work stage B: transposes + gate MLP tail; its PE ops
                depend only on stage-A results finished a sim chunk ago."""
                hid, ew, zw = gate_st[bt]
                psht = ps_sml.tile([64, P], F32, tag="sml", name="hidtp")
                nc.tensor.transpose(out=psht[:64, :], in_=hid[:, :],
                                    identity=ident[:])
                hidT = tiny.tile([64, P], F32, tag="c128", name="hidT")
                nc.vector.tensor_copy(hidT[:, :], psht[:64, :])
                psg2 = ps_sml.tile([P, 3], F32, tag="sml", name="psg2")
                nc.tensor.matmul(psg2[:, :3], hidT[:, :], gw2[:, :],
                                 start=True, stop=True)
                gl = cst.tile([P, 3], F32, name=f"gl{bt}")
                nc.vector.tensor_add(gl[:, :], psg2[:, :3], b2bc[:, :])
                gmax = tiny.tile([P, 1], F32, tag="c1", name="gmax")
                nc.vector.reduce_max(gmax[:, :], gl[:, :], axis=AXL.X)
                nc.vector.tensor_scalar_mul(gmax[:, :], gmax[:, :], -1.0)
                nc.scalar.activation(gl[:, :], gl[:, :], AF.Exp, bias=gmax[:, :1])
                gz = tiny.tile([P, 1], F32, tag="c1", name="gz")
                nc.vector.reduce_sum(gz[:, :], gl[:, :], axis=AXL.X)
                nc.vector.reciprocal(gz[:, :], gz[:, :])
                nc.vector.tensor_scalar(gl[:, :], gl[:, :], gz[:, :1], None,
                                        op0=ALU.mult)
                gw_pre[bt] = gl
                # fold softmax normalization AND gate weight 0 into ew
                nc.vector.tensor_tensor(out=zw[:, :], in0=zw[:, :],
                                        in1=gl[:, 0:1], op=ALU.mult)
                nc.vector.tensor_scalar(ew[:, :], ew[:, :], zw[:, :1], None,
                                        op0=ALU.mult)

            def emit_gate_c(bt):
                """Gate/work stage C: transpose of the folded work probs."""
                _, ew, _ = gate_st[bt]
                pset = ps_sml.tile([S, P], F32, tag="sml", name="ewtp")
                nc.tensor.transpose(out=pset[:S, :], in_=ew[:, :],
                                    identity=ident[:])
                ewT = cst.tile([S, P], BF16, name=f"ewT{bt}")
                nc.vector.tensor_copy(ewT[:, :], pset[:S, :])
                ewT_pre[bt] = ewT

            # ===================================================================
            # Phase S: similarity + per-chunk top-8 candidates
            # ===================================================================
            cand_v_e = [big.tile([P, (N // 512) * 8], F32, tag=f"cve{bt}",
                                 name=f"cve{bt}") for bt in range(NBT)]
            cand_i_e = [big.tile([P, (N // 512) * 8], F32, tag=f"cie{bt}",
                                 name=f"cie{bt}") for bt in range(NBT)]
            cand_v_s = [big.tile([P, (M // 512) * 8], F32, tag=f"cvs{bt}",
                                 name=f"cvs{bt}") for bt in range(NBT)]
            cand_i_s = [big.tile([P, (M // 512) * 8], F32, tag=f"cis{bt}",
                                 name=f"cis{bt}") for bt in range(NBT)]

            def sim_chunk(xP, kd, r, ch, cand_v, cand_i, base):
                """sims of all 4 b-tiles vs bf16-pair keys kd[r, :, h, :]."""
                psts = [ps_mm.tile([P, 512], F32, tag="mm", name=f"simps{i}")
                        for i in range(NBT)]
                for hi in range(HT):
                    kth = s512p.tile([P, 512], BF16, tag="st16h", name="kth",
                                     bufs=5)
                    ktl = s512p.tile([P, 512], BF16, tag="st16l", name="ktl",
                                     bufs=5)
                    nc.sync.dma_start(
                        kth[:], kd[r, 0, hi * P:(hi + 1) * P, :])
                    nc.sync.dma_start(
                        ktl[:], kd[r, 1, hi * P:(hi + 1) * P, :])
                    for bt in range(NBT):
                        bs = slice(bt * P, (bt + 1) * P)
                        nc.tensor.matmul(
                            psts[bt][:], xP[:, 0, hi, bs], kth[:],
                            start=(hi == 0), stop=False)
                        nc.tensor.matmul(
                            psts[bt][:], xP[:, 0, hi, bs], ktl[:],
                            start=False, stop=False)
                        nc.tensor.matmul(
                            psts[bt][:], xP[:, 1, hi, bs], kth[:],
                            start=False, stop=(hi == HT - 1))
                for bt in range(NBT):
                    sc = simb.tile([P, 512], F32, tag="simc", name="simc",
                                   bufs=2)
                    nc.scalar.activation(sc[:], psts[bt][:], AF.Copy)
                    mx = simb.tile([P, 8], F32, tag="mx", name="mx")
                    mi = simb.tile([P, 8], U32, tag="mi", name="mi")
                    nc.vector.max(out=mx[:], in_=sc[:])
                    nc.vector.max_index(out=mi[:], in_max=mx[:], in_values=sc[:])
                    nc.vector.tensor_copy(cand_v[bt][:, ch * 8:(ch + 1) * 8],
                                          mx[:])
                    mif = simb.tile([P, 8], F32, tag="mif", name="mif")
                    nc.vector.tensor_copy(mif[:], mi[:])
                    nc.vector.tensor_scalar_add(
                        cand_i[bt][:, ch * 8:(ch + 1) * 8], mif[:],
                        float(base))

            def topk_attend(cand_v, cand_i, k, inv_p, bt, vals_dram, gscale,
                            acc_tag, bufs=2):
                """Merged top-k -> softmax (x gscale) -> gather + weighted sum."""
                top8 = tiny.tile([P, 8], F32, tag="c8", name="top8")
                nc.vector.max(out=top8[:], in_=cand_v[:])
                idxf = tiny.tile([P, 8], F32, tag="c8", name="idxf")
                eqm = s512p.tile([P, 256], F32, tag="sqn", name="eqm")
                for kk in range(k):
                    w = cand_v.shape[-1]
                    nc.vector.tensor_scalar(
                        eqm[:, :w], cand_v[:], top8[:, kk:kk + 1], None,
                        op0=ALU.is_equal)
                    nc.vector.tensor_tensor(
                        out=eqm[:, :w], in0=eqm[:, :w], in1=cand_i[:], op=ALU.mult)
                    nc.vector.reduce_sum(idxf[:, kk:kk + 1], eqm[:, :w], axis=AXL.X)
                idxu = tiny.tile([P, 8], U32, tag="c8u", name="idxu")
                nc.vector.tensor_copy(idxu[:, :k], idxf[:, :k])
                sc8 = tiny.tile([P, 8], F32, tag="c8", name="sc8")
                nc.vector.tensor_scalar(
                    sc8[:, :k], top8[:, :k], inv_p[:, bt:bt + 1], None,
                    op0=ALU.mult)
                negm = tiny.tile([P, 1], F32, tag="c1", name="negm")
                nc.vector.tensor_scalar_mul(negm[:, :], sc8[:, 0:1], -1.0)
                nc.scalar.activation(sc8[:, :k], sc8[:, :k], AF.Exp,
                                     bias=negm[:, :1])
                zs = tiny.tile([P, 1], F32, tag="c1", name="zs")
                nc.vector.reduce_sum(zs[:, :], sc8[:, :k], axis=AXL.X)
                nc.vector.reciprocal(zs[:, :], zs[:, :])
                nc.vector.tensor_scalar(zs[:, :], zs[:, :], gscale, None,
                                        op0=ALU.mult)
                nc.vector.tensor_scalar(sc8[:, :k], sc8[:, :k], zs[:, :1], None,
                                        op0=ALU.mult)
                acc = sm.tile([P, H], BF16, tag=acc_tag, name="acc" + acc_tag,
                              bufs=bufs)
                nc.vector.memset(acc[:, :], 0.0)
                for kk in range(k):
                    g = gath.tile([P, H], BF16, tag="g", name="g")
                    nc.gpsimd.indirect_dma_start(
                        out=g[:, :], out_offset=None, in_=vals_dram,
                        in_offset=bass.IndirectOffsetOnAxis(
                            ap=idxu[:, kk:kk + 1], axis=0))
                    nc.vector.scalar_tensor_tensor(
                        out=acc[:, :], in0=g[:, :], scalar=sc8[:, kk:kk + 1],
                        in1=acc[:, :], op0=ALU.mult, op1=ALU.add)
                return acc

            def transpose_into(dst, src, dt=BF16):
                """dst [P, HT, P] view <- transpose of src [P, H]; psum
                evacuation alternates ACT/DVE so neither sequencer's
                per-op dispatch overhead paces the chain."""
                idn = ident if dt == F32 else ident16
                for hi in range(HT):
                    pst = ps_mm.tile([P, P], dt, tag="mm", name="trf")
                    nc.tensor.transpose(out=pst[:], in_=src[:, hi * P:(hi + 1) * P],
                                        identity=idn[:])
                    nc.scalar.activation(dst[:, hi, :], pst[:], AF.Copy)

            # episodic sims: one gathered buffer, rank-major global indices;
            # gate/work stages (off-PE-latency-heavy) interleave with chunks
            # so each stage's PE ops only see dependencies already finished.
            gbc2 = [None] * (H // 512)
            bbc2 = [None] * (H // 512)
            for ch in range(N // 512):
                if 2 <= ch <= NBT + 1:
                    emit_gate_c(ch - 2)
                if 1 <= ch <= NBT:
                    emit_gate_b(ch - 1)
                if ch < NBT:
                    emit_gate_a(ch)
                if ch == 4:
                    # LN gamma/beta broadcast tiles (full row, loaded in 512
                    # chunks); the serial DMA<->POOL ping-pong hides under
                    # the remaining sim chunks.
                    gbc2[0] = sm.tile([P, H], BF16, tag="gbc", name="gbc",
                                      bufs=1)
                    bbc2[0] = sm.tile([P, H], BF16, tag="bbc", name="bbc",
                                      bufs=1)
                    for t, dsrc in ((gbc2[0], ln_gamma), (bbc2[0], ln_beta)):
                        for jq in range(H // 512):
                            cq = slice(jq * 512, (jq + 1) * 512)
                            row = rows.tile([1, 512], F32, tag="crow",
                                            name="crow", bufs=1)
                            nc.sync.dma_start(row[:1, :], dsrc[None, cq])
                            row16 = rows.tile([1, 512], BF16, tag="crow16",
                                              name="crow16", bufs=1)
                            nc.scalar.activation(row16[:1, :], row[:1, :],
                                                 AF.Copy)
                            nc.gpsimd.partition_broadcast(t[:, cq],
                                                          row16[:1, :])
                sim_chunk(qTp, ag_nek_out, ch, ch, cand_v_e, cand_i_e,
                          ch * 512)

            # episodic merges (DVE/gathers) overlap semantic sims (PE); the
            # accT_e transposes are emitted after a sem chunk each so the PE
            # queue never waits on a merge.
            accT_e = big.tile([P, NBT, HT, P], BF16, tag="kT", name="accTe")
            accT_s = big.tile([P, NBT, HT, P], BF16, tag="xTin", name="accTs")
            acc_e = [None] * NBT
            acc_s = [None] * NBT

            def emit_merge_e(bt):
                acc_e[bt] = topk_attend(cand_v_e[bt][:], cand_i_e[bt][:], EP_K,
                                        invq_p, bt, ag_ev_out[:, :],
                                        gw_pre[bt][:, 1:2], "sl1")

            def emit_merge_s(bt):
                acc_s[bt] = topk_attend(cand_v_s[bt][:], cand_i_s[bt][:], SEM_K,
                                        invqs_p, bt, semv16, gw_pre[bt][:, 2:3],
                                        "sl2", bufs=3)

            sem_seq = [(i, r) for i in range(ML // 512) for r in range(NCORES)]

            def emit_sem_chunk(ch):
                i, r = sem_seq[ch]
                sim_chunk(qsP, ag_nks_out[i], r, ch, cand_v_s, cand_i_s,
                          r * ML + i * 512)

            emit_merge_e(0)
            emit_merge_e(1)
            p2a_pre = {}
            for ch in range(len(sem_seq)):
                if ch == len(sem_seq) - 2:
                    # prefetch Pass 2a's first moving tiles so its opening
                    # matmuls don't wait on the DMA queue draining
                    wsn0 = s512p.tile([S, 512], BF16, tag="s512", name="wsn2")
                    nc.sync.dma_start(wsn0[:S, :], ws16[:, :512])
                    wt0 = wtp.tile([P, 4, 512], BF16, tag="wt", name="wto")
                    nc.sync.dma_start(
                        wt0[:], weo16[:512, :512].rearrange(
                            "(q p) c -> p q c", p=P))
                    p2a_pre["wsn"] = wsn0
                    p2a_pre["wt"] = wt0
                emit_sem_chunk(ch)
                if ch < NBT:
                    transpose_into(accT_e[:, ch], acc_e[ch])
                    if ch + 2 < NBT:
                        emit_merge_e(ch + 2)

            # ===================================================================
            # Phase F: blend + output projections + streaming layernorm
            # ===================================================================
            bl_all = big.tile([P, NBT, H], BF16, tag="bl", name="bl_all")

            warm(50)
            emit_merge_s(0)
            emit_merge_s(1)
            emit_merge_s(2)
            # Pass 2a: bl = gate0*w_out + acc_e @ W_eo (jc-major, weights read
            # once); ACT evacuates so DVE stays free for the semantic merges,
            # which run concurrently on DVE. The accT_s transposes interleave
            # between jc blocks: Ts_k lands right after merge k finishes, and
            # releasing acc_s[0] lets merge 3's ring slot allocate.
            for jc in range(H // 512):
                cs = slice(jc * 512, (jc + 1) * 512)
                if jc == 0:
                    wsn = p2a_pre["wsn"]
                else:
                    wsn = s512p.tile([S, 512], BF16, tag="s512", name="wsn2")
                    nc.sync.dma_start(wsn[:S, :], ws16[:, cs])
                psos = [ps_mm.tile([P, 512], F32, tag="mm", name=f"pso{i}")
                        for i in range(NBT)]
                for bt in range(NBT):
                    nc.tensor.matmul(psos[bt][:], ewT_pre[bt][:, :],
                                     wsn[:S, :], start=True, stop=False)
                for hq in range(HT // 4):
                    if jc == 0 and hq == 0:
                        wt = p2a_pre["wt"]
                    else:
                        wt = wtp.tile([P, 4, 512], BF16, tag="wt", name="wto")
                        nc.sync.dma_start(
                            wt[:], weo16[hq * 512:(hq + 1) * 512, cs].rearrange(
                                "(q p) c -> p q c", p=P))
                    for q4 in range(4):
                        hi = hq * 4 + q4
                        for bt in range(NBT):
                            nc.tensor.matmul(
                                psos[bt][:], accT_e[:, bt, hi, :], wt[:, q4],
                                start=False, stop=(hi == HT - 1))
                for bt in range(NBT):
                    nc.scalar.activation(bl_all[:, bt, cs], psos[bt][:], AF.Copy)
                if 1 <= jc:
                    transpose_into(accT_s[:, jc - 1], acc_s[jc - 1])
                    if jc == H // 512 - 1:
                        transpose_into(accT_s[:, jc], acc_s[jc])
                if jc == 0:
                    emit_merge_s(3)

            blT = big.tile([P, NBT, HT, P], BF16, tag="kT", name="blT")
            # Pass 2b: bl += acc_s @ W_so (jc-major, weights read once)
            for jc in range(H // 512):
                cs = slice(jc * 512, (jc + 1) * 512)
                psob = [ps_mm.tile([P, 512], F32, tag="mm", name=f"psob{i}")
                        for i in range(NBT)]
                for hq in range(HT // 4):
                    wt = wtp.tile([P, 4, 512], BF16, tag="wt", name="wtob")
                    nc.sync.dma_start(
                        wt[:], wso16[hq * 512:(hq + 1) * 512, cs].rearrange(
                            "(q p) c -> p q c", p=P))
                    for q4 in range(4):
                        hi = hq * 4 + q4
                        for bt in range(NBT):
                            nc.tensor.matmul(
                                psob[bt][:], accT_s[:, bt, hi, :], wt[:, q4],
                                start=(hi == 0), stop=(hi == HT - 1))
                for bt in range(NBT):
                    nc.vector.tensor_add(bl_all[:, bt, cs],
                                         bl_all[:, bt, cs], psob[bt][:])
                    if jc == H // 512 - 1:
                        nc.sync.dma_start_transpose(blT[:, bt],
                                                    bl_all[:, bt, :])

            warm(45)
            # Pass 3: xo = bl @ W_ro (jc-major) with streamed LN stats
            xo_all = big.tile([P, NBT, H], BF16, tag="xTin", name="xo_all")
            msum = [tiny.tile([P, 4], F32, tag=f"cms{i}", name=f"msum{i}",
                              bufs=1) for i in range(NBT)]
            vsum = [tiny.tile([P, 4], F32, tag=f"cvs{i}", name=f"vsum{i}",
                              bufs=1) for i in range(NBT)]
            ln_stats = [None] * NBT

            def emit_ln_stats(bt):
                """inv-std and -mu*inv-std per-partition scalars for one bt."""
                mu = tiny.tile([P, 1], F32, tag="c1", name="mu")
                nc.vector.reduce_sum(mu[:, :], msum[bt][:, :], axis=AXL.X)
                nc.vector.tensor_scalar_mul(mu[:, :], mu[:, :], 1.0 / H)
                vs = tiny.tile([P, 1], F32, tag=f"cvv{bt}", name="vs", bufs=1)
                nc.vector.reduce_sum(vs[:, :], vsum[bt][:, :], axis=AXL.X)
                nc.vector.tensor_scalar_mul(vs[:, :], vs[:, :], 1.0 / H)
                mu2 = tiny.tile([P, 1], F32, tag="c1", name="mu2")
                nc.vector.tensor_tensor(out=mu2[:, :], in0=mu[:, :],
                                        in1=mu[:, :], op=ALU.mult)
                nc.vector.tensor_tensor(out=vs[:, :], in0=vs[:, :],
                                        in1=mu2[:, :], op=ALU.subtract)
                nc.vector.tensor_scalar_add(vs[:, :], vs[:, :], LN_EPS)
                nc.scalar.sqrt(vs[:, :], vs[:, :])
                nc.vector.reciprocal(vs[:, :], vs[:, :])
                nmu = tiny.tile([P, 1], F32, tag=f"cnm{bt}", name="nmu",
                                bufs=1)
                nc.vector.tensor_tensor(out=nmu[:, :], in0=mu[:, :],
                                        in1=vs[:, :], op=ALU.mult)
                nc.vector.tensor_scalar_mul(nmu[:, :], nmu[:, :], -1.0)
                ln_stats[bt] = (vs, nmu)
            def emit_ln_final(bt):
                """Normalize + affine + store for one b-tile: full-row bf16
                DVE ops (2x throughput, minimal dispatch count) on the gather
                ring, which is idle by this point."""
                vs, nmu = ln_stats[bt]
                on16 = gath.tile([P, H], BF16, tag="g", name="on16")
                nc.vector.tensor_scalar(on16[:, :], xo_all[:, bt, :],
                                        vs[:, :1], nmu[:, :1],
                                        op0=ALU.mult, op1=ALU.add)
                nc.vector.tensor_mul(on16[:, :], on16[:, :], gbc2[0][:, :])
                on = gath.tile([P, H], BF16, tag="g", name="on")
                nc.vector.tensor_add(on[:, :], on16[:, :], bbc2[0][:, :])
                nc.sync.dma_start(out_s[bt * P:(bt + 1) * P, :], on[:])

            for jc in range(H // 512):
                cs = slice(jc * 512, (jc + 1) * 512)
                psro = [ps_mm.tile([P, 512], F32, tag="mm", name=f"psro{i}")
                        for i in range(NBT)]
                for hq in range(HT // 4):
                    wt = wtp.tile([P, 4, 512], BF16, tag="wt", name="wtro")
                    nc.sync.dma_start(
                        wt[:], wro16[hq * 512:(hq + 1) * 512, cs].rearrange(
                            "(q p) c -> p q c", p=P))
                    for q4 in range(4):
                        hi = hq * 4 + q4
                        for bt in range(NBT):
                            nc.tensor.matmul(
                                psro[bt][:], blT[:, bt, hi, :], wt[:, q4],
                                start=(hi == 0), stop=(hi == HT - 1))
                for bt in range(NBT):
                    nc.scalar.activation(xo_all[:, bt, cs], psro[bt][:],
                                         AF.Copy,
                                         accum_out=msum[bt][:, jc:jc + 1])
                    sqc = s512p.tile([P, 512], F32, tag="sqn", name="sqc")
                    nc.scalar.activation(sqc[:, :], psro[bt][:], AF.Square,
                                         accum_out=vsum[bt][:, jc:jc + 1])
                    if jc == H // 512 - 1:
                        emit_ln_stats(bt)
            for bt in range(NBT):
                emit_ln_final(bt)

    nc.finalize()
    return nc


_NC_CACHE = None
LAST_EXEC_NS = None


def _pack_xpair(x):
    """[R,H] f32 -> [R//512, P, 2, HT, 512] bf16 pair, pre-transposed to
    the on-chip tile layout: pk[ch, p, half, hi, r] = split(x)[half][
    ch*512+r, hi*128+p]."""
    hi_, lo_ = _split_bf16(x)
    def lay(a):
        return a.reshape(-1, HT, P).transpose(2, 1, 0)   # [P, HT, R]
    pk = np.stack([lay(hi_), lay(lo_)], axis=1)          # [P, 2, HT, R]
    R = x.shape[0]
    return np.ascontiguousarray(
        np.stack([pk[..., i * 512:(i + 1) * 512]
                  for i in range(R // 512)], axis=0))


def _pack_wpair(w):
    """[H,H] f32 -> [HT, P, 2, HT, P] bf16 pair in wcP tile layout:
    packed[j, p, half, hi, c] = split(W)[half][hi*128+p, j*128+c]."""
    hi_, lo_ = _split_bf16(w)
    def lay(a):
        # [hi, p, j, c] -> [j, p, hi, c]
        return np.ascontiguousarray(
            a.reshape(HT, P, HT, P).transpose(2, 1, 0, 3))
    return np.ascontiguousarray(
        np.stack([lay(hi_), lay(lo_)], axis=2))


def _pack_gwk(gate_W1, work_slots):
    """hstack(gate_W1 [H,64], work_slots.T [H,64]) -> [P, 2, HT, 128] pair:
    pk[p, half, hi, c] = split(gw)[half][hi*128+p, c]."""
    gw = np.hstack([np.asarray(gate_W1, np.float32),
                    np.ascontiguousarray(np.asarray(work_slots, np.float32).T)])
    hi_, lo_ = _split_bf16(gw)
    def lay(a):
        return a.reshape(HT, P, 128).transpose(1, 0, 2)   # [P, HT, 128]
    return np.ascontiguousarray(np.stack([lay(hi_), lay(lo_)], axis=1))


def _split_bf16(x):
    """two-term bf16 decomposition: x ~= hi + lo to ~16 mantissa bits."""
    import ml_dtypes
    bf = ml_dtypes.bfloat16
    x = np.ascontiguousarray(np.asarray(x), dtype=np.float32)
    hi = x.astype(bf)
    lo = (x - hi.astype(np.float32)).astype(bf)
    return hi, lo


def kernel(**inputs) -> np.ndarray:
    global _NC_CACHE
    if _NC_CACHE is None:
        _NC_CACHE = build()
    nc = _NC_CACHE

    def arr(x):
        return np.ascontiguousarray(np.asarray(x), dtype=np.float32)

    wq_pk = _pack_wpair(inputs["W_query"])
    wek_pk = _pack_wpair(inputs["W_ek"])
    wsq_pk = _pack_wpair(inputs["W_sq"])
    wsk_pk = _pack_wpair(inputs["W_sk"])
    wev16, _ = _split_bf16(inputs["W_ev"])
    weo16, _ = _split_bf16(inputs["W_eo"])
    wso16, _ = _split_bf16(inputs["W_so"])
    wro16, _ = _split_bf16(inputs["W_ro"])
    semv16, _ = _split_bf16(inputs["sem_values"])
    ws16, _ = _split_bf16(inputs["work_slots"])
    gwk_pk = _pack_gwk(inputs["gate_W1"], inputs["work_slots"])

    in_maps = []
    for c in range(NCORES):
        in_maps.append({
            "query_pk": _pack_xpair(inputs["query"][c * BL:(c + 1) * BL]),
            "ep_pk": _pack_xpair(inputs["ep_store"][c * NL:(c + 1) * NL]),
            "semk_pk": _pack_xpair(inputs["sem_keys"][c * ML:(c + 1) * ML]),
            "ep_imp_s": arr(inputs["ep_importance"][c * NL:(c + 1) * NL]),
            "ep_ts_s": arr(inputs["ep_timestamps"][c * NL:(c + 1) * NL]),
            "ep_imp": arr(inputs["ep_importance"]),
            "ep_ts": arr(inputs["ep_timestamps"]),
            "semv16": semv16,
            "wq_pk": wq_pk,
            "wek_pk": wek_pk,
            "wsq_pk": wsq_pk,
            "wsk_pk": wsk_pk,
            "wev16": wev16,
            "weo16": weo16,
            "wso16": wso16,
            "wro16": wro16,
            "ws16": ws16,
            "gwk_pk": gwk_pk,
            "gate_b1": arr(inputs["gate_b1"]),
            "gate_W2": arr(inputs["gate_W2"]),
            "gate_b2": arr(inputs["gate_b2"]),
            "ln_gamma": arr(inputs["ln_gamma"]),
            "ln_beta": arr(inputs["ln_beta"]),
        })
    res = run_bass_kernel_spmd(nc, in_maps, core_ids=list(range(NCORES)))
    return np.concatenate(
        [np.asarray(res.results[c]["out_s"], dtype=np.float32)
         for c in range(NCORES)], axis=0)
